# revision 19
# baseline (speedup 1.0000x reference)
"""AliasFreeActivation (upsample2x -> leaky_relu -> 31x31 depthwise sinc conv
-> downsample2x) as a Trainium2 Bass/Tile kernel, data-parallel over 8 cores.

Math (per [128,128] image; B*C = 512 images, 64 per core):
  out = Dy @ Conv_F(lrelu(Uy @ x @ Ux^T)) @ Dx^T
With F = sum_r g_r h_r^T (weighted SVD of the 31x31 filter, rank 8):
  out = sum_r M_r @ act @ N_r^T
  M_r = Dy @ Toeplitz(g_r) [128,256],  N_r = Dx @ Toeplitz(h_r) [128,256]
  act = lrelu(Uy @ x @ Ux^T) [256,256]

Precision/engine split (v2):
  ranks 0..k32-1 : A-pass fp16 (banded), W kept as RAW fp32 in SBUF
                   (DVE plain copy), B-pass float32r matmuls (full PE rate
                   for >=256 output cols) with exact fp32 M constants.
  ranks k32..    : A-pass fp8 DoubleRow, W evac = PLAIN Pool copy to fp8
                   (the old s_e scale is folded into nt8), B-pass fp8 DR.
PSUM evacuation engines: ACT = lrelu + tmpT + out; DVE = W32; Pool
(gpsimd, which CAN read PSUM on TRN2) = W8 + act8.

Device dataflow per image (out[m,n] = sum_k lhsT[k,m] rhs[k,n]):
  S1a: tmpT[c,Y]    = sum_y x[y,c] UyT[y,Y]
  S1b: actT[X,Y]    = sum_c UxT[c,X] tmpT[c,Y]   (+ lrelu on evac)
  A:   W[Y,(r,j)]   = sum_X actT[X,Y] NT[X,(r,j)]
  B:   out[i,(m,j)] = sum_{r,Yc} MT[Yc,(r,i)] W[Yc,(r,m,j)]   (PSUM accum)
"""
import contextlib
import os

import numpy as np

import concourse.bass as bass
import concourse.mybir as mybir
import concourse.tile as tile
from concourse import bacc
from concourse.bass_utils import run_bass_kernel_spmd

H = 128
H2 = 256
KF = 31
LRELU_SLOPE = 0.01
RANK_ENV = os.environ.get("AFA_RANK")
N_CORES = 8
N_IMG = 64                      # images per core (512 / 8)
GROUP = int(os.environ.get("AFA_G", "4"))
K32_ENV = os.environ.get("AFA_K32")


# ---------------- host-side constants ----------------

def _ac_matrix(out_n, in_n):
    scale = (in_n - 1) / (out_n - 1)
    c = np.arange(out_n, dtype=np.float64) * scale
    i0 = np.clip(np.floor(c).astype(np.int64), 0, in_n - 2)
    w = c - i0
    M = np.zeros((out_n, in_n), dtype=np.float64)
    M[np.arange(out_n), i0] = 1.0 - w
    M[np.arange(out_n), i0 + 1] = w
    return M


def _toeplitz_same(h, n):
    T = np.zeros((n, n), dtype=np.float64)
    for u in range(len(h)):
        d = u - len(h) // 2
        if d >= 0:
            idx = np.arange(0, n - d)
        else:
            idx = np.arange(-d, n)
        T[idx, idx + d] += h[u]
    return T


def _shift_mat(n, d):
    S = np.zeros((n, n))
    idx = np.arange(0, n - d) if d >= 0 else np.arange(-d, n)
    S[idx, idx + d] = 1.0
    return S


def _weighted_rank(F, rank):
    """Rank-`rank` approx of F minimizing the end-to-end error under the
    signal model act ~ U x U^T with white x: err = ||B^1/2 (F-Fr) B^1/2||_F
    with B the Gram of the composed per-tap maps Z_u = D S_u U."""
    kf = F.shape[0]
    D = _ac_matrix(H, H2)
    Uu = _ac_matrix(H2, H)
    Zs = [D @ _shift_mat(H2, u - kf // 2) @ Uu for u in range(kf)]
    B = np.zeros((kf, kf))
    for u in range(kf):
        for v in range(u, kf):
            B[u, v] = B[v, u] = np.sum(Zs[u] * Zs[v])
    w, V = np.linalg.eigh(B)
    w = np.maximum(w, 1e-12)
    Bh = (V * np.sqrt(w)) @ V.T
    Bih = (V / np.sqrt(w)) @ V.T
    Gm = Bh @ F @ Bh
    U_, S_, Vt_ = np.linalg.svd(Gm)
    Fr = Bih @ (U_[:, :rank] * S_[:rank]) @ Vt_[:rank] @ Bih
    return Fr


def _sample_act(Uu, n=4):
    rng = np.random.default_rng(1234)
    X = rng.standard_normal((n, H, H))
    A = Uu @ X @ Uu.T
    return np.where(A >= 0, A, LRELU_SLOPE * A)


def _make_consts(filt, rank, k32):
    """Build device constants.

    Ranks [0, k32): nt fp16 in seg layout (columns (j, r_local)-major for the
    banded A-pass 2D APs), mt as RAW fp32 (used as float32r), rank-major.
    Ranks [k32, rank): nt8/mt8 fp8 DoubleRow-interleaved, with per-rank scale
    n_r chosen so that nt8, W8 (= s_a n_r act N^T, evac'd by a PLAIN copy)
    and mt8 (= M / (s_a n_r)) all sit inside the fp8e4m3 normal range.
    s_a is folded into uyt (lrelu is positively homogeneous) and 1/s_a into
    nt/mt-fp32, so every rank's B contribution lands in out PSUM at scale 1.
    """
    F = np.asarray(filt, dtype=np.float64)
    if rank < min(F.shape):
        F = _weighted_rank(F, rank)
    U, S, Vt = np.linalg.svd(F)
    D = _ac_matrix(H, H2)
    Uu = _ac_matrix(H2, H)
    uyt = np.ascontiguousarray(Uu.T)               # [128 y, 256 Y]

    nr8 = rank - k32
    Ms = [D @ _toeplitz_same(U[:, r] * np.sqrt(S[r]), H2) for r in range(rank)]
    Ns = [D @ _toeplitz_same(Vt[r, :] * np.sqrt(S[r]), H2) for r in range(rank)]

    # fp8 scales: act8 = s_a * act (s_a folded into uyt)
    act = _sample_act(Uu)
    actmax = np.abs(act).max() * 1.15
    s_a = 100.0 / actmax

    # fp16/fp32 ranks: nt gets 1/s_a (act arrives pre-scaled by s_a);
    # mt stays exact fp32.
    nt = np.zeros((2, H, k32 * H), dtype=np.float32)
    mt32 = np.zeros((2, H, k32 * H), dtype=np.float32)
    for r in range(k32):
        cols = np.arange(H) * k32 + r              # (j, r_local)-major
        for c in range(2):
            nt[c, :, cols] = (Ns[r][:, c * H:(c + 1) * H] / s_a).astype(np.float32)
            mt32[c, :, r * H:(r + 1) * H] = \
                Ms[r][:, c * H:(c + 1) * H].T.astype(np.float32)

    out = {"uyt": (uyt * s_a).astype(np.float32), "uxt": uyt.astype(np.float32),
           "nt": nt, "mt32": mt32, "k32": k32, "nr8": nr8, "s_a": float(s_a),
           "s_e": 1.0}
    if nr8 == 0:
        return out

    # ---- fp8 tail: per-rank scales (baseline scheme): wg8 = s_e * W8psum
    # (s_e applied for free by the ACT Copy evac), n_r/m_r sqrt-balanced so
    # act8, nt8, wg8, mt8 all sit inside the fp8e4m3 normal range.
    np8 = mybir.dt.np(mybir.dt.float8e4)
    nt8 = np.zeros((H, 2, nr8 * H), dtype=np.float64)
    mt8 = np.zeros((H, 2, nr8 * H), dtype=np.float64)
    wmaxs = [np.abs(act @ Ns[k32 + i].T).max() * 1.3 for i in range(nr8)]
    mmaxs = [np.abs(Ms[k32 + i]).max() for i in range(nr8)]
    nmaxs = [np.abs(Ns[k32 + i]).max() for i in range(nr8)]
    bias = 2.0
    tmp = [np.sqrt(wmaxs[i] * mmaxs[i]) * bias / (s_a * wmaxs[i])
           for i in range(nr8)]           # = n_r * s_e per rank
    s_e = float(np.exp(np.mean(np.log([tmp[i] * nmaxs[i]
                                       for i in range(nr8)]))) / 100.0)
    for i in range(nr8):
        r = k32 + i
        n_r = tmp[i] / s_e
        m_r = 1.0 / (s_a * n_r * s_e)
        for c in range(2):
            # nt8[p, xc, j*nr8+i] = n_r * N_r[j, xc*128+p]
            nt8[:, c, i::nr8] = (n_r * Ns[r][:, c * H:(c + 1) * H]).T
            # mt8[p, yc, i*H+k] = m_r * M_r[k, yc*128+p]
            mt8[:, c, i * H:(i + 1) * H] = (m_r * Ms[r][:, c * H:(c + 1) * H]).T
    out["s_e"] = s_e
    out["nt8"] = nt8.astype(np8).reshape(H, 2 * nr8 * H)
    out["mt8"] = mt8.astype(np8).reshape(H, 2 * nr8 * H)
    return out


# ---------------- device program ----------------

def _build_tile_program(tc, outs, ins, *, n_img, k32, nr8, group,
                        s_e=1.0, loop_reps=1):
    nc = tc.nc
    if nr8:
        x_d, uyt_d, uxt_d, nt_d, mt32_d, nt8_d, mt8_d = ins
    else:
        x_d, uyt_d, uxt_d, nt_d, mt32_d = ins
    out_d = outs[0]
    RC = k32 * H
    G = group
    GW = G * H
    assert G in (2, 4)
    assert n_img % G == 0
    f16 = mybir.dt.float16
    f32 = mybir.dt.float32
    f32r = mybir.dt.float32r
    fp8 = mybir.dt.float8e4
    DR = mybir.MatmulPerfMode.DoubleRow

    ctx = contextlib.ExitStack()
    with ctx:
        const_pool = ctx.enter_context(tc.tile_pool(name="consts", bufs=1))
        x_pool = ctx.enter_context(tc.tile_pool(name="x", bufs=2))
        tmp_pool = ctx.enter_context(tc.tile_pool(
            name="tmp", bufs=int(os.environ.get("AFA_TMPB", "3"))))
        act_pool = ctx.enter_context(tc.tile_pool(
            name="act", bufs=int(os.environ.get("AFA_ACTB", "8"))))
        w_pool = ctx.enter_context(tc.tile_pool(
            name="w", bufs=int(os.environ.get("AFA_WB", "2"))))
        osb_pool = ctx.enter_context(tc.tile_pool(name="osb", bufs=2))
        # PSUM banks: ps_s 3 + ps_w 2x2 + ps_out 1 = 8
        ps_s = ctx.enter_context(tc.tile_pool(
            name="ps_s", bufs=int(os.environ.get("AFA_PSS", "1")), space="PSUM"))
        ps_w = ctx.enter_context(tc.tile_pool(
            name="ps_w", bufs=int(os.environ.get("AFA_PSW", "3")), space="PSUM"))
        ps_out = ctx.enter_context(tc.tile_pool(
            name="ps_out", bufs=int(os.environ.get("AFA_PSO", "1")),
            space="PSUM"))

        uyt_sb = const_pool.tile([H, H2], f16, tag="uyt")
        nc.sync.dma_start(uyt_sb[:], uyt_d[:])
        uxt_sb = const_pool.tile([H, H2], f16, tag="uxt")
        nc.sync.dma_start(uxt_sb[:], uxt_d[:])
        nt_sb = []
        mt_sb = []
        for c in range(2):
            t = const_pool.tile([H, RC], f16, tag=f"nt{c}", name=f"nt{c}_sb")
            nc.sync.dma_start(t[:], nt_d[c])
            nt_sb.append(t)
            t = const_pool.tile([H, RC], f32r, tag=f"mt{c}", name=f"mt{c}_sb")
            nc.sync.dma_start(t[:], mt32_d[c])
            mt_sb.append(t)
        if nr8:
            nt8_sb = const_pool.tile([H, 2 * nr8 * H], fp8, tag="nt8")
            nc.sync.dma_start(nt8_sb[:], nt8_d[:])
            mt8_sb = const_pool.tile([H, 2 * nr8 * H], fp8, tag="mt8")
            nc.sync.dma_start(mt8_sb[:], mt8_d[:])

        def _fetch_x(g):
            x_sb = x_pool.tile([H, GW], f16, tag="x", name=f"x_{g}")
            xg = x_d[g * G:(g + 1) * G].rearrange("g h w -> h g w")
            nc.sync.dma_start(x_sb[:].rearrange("h (g w) -> h g w", g=G), xg)
            return x_sb

        def _emit_s1a(g, half, x_sb, st):
            """S1a matmuls for an image pair + tmpT evac (DVE)."""
            tmpT_ps = ps_s.tile([H, 2 * H2], f32, tag="sp",
                                name=f"tp_{g}_{half}")
            for u in range(2):
                m = half * 2 + u
                nc.tensor.matmul(tmpT_ps[:, u * H2:(u + 1) * H2],
                                 x_sb[:, m * H:(m + 1) * H], uyt_sb[:],
                                 start=True, stop=True)
            t_sb = tmp_pool.tile([H, 2 * H2], f16, tag="tmpT")
            nc.vector.tensor_copy(t_sb[:], tmpT_ps[:])
            st["t_sb"] = t_sb

        def _emit_s1b(g, m, st):
            """S1b matmuls + lrelu (ACT) + act8 (Pool) for one image."""
            act_ps = ps_s.tile([H, 2 * H2], f32, tag="sp", name=f"ap_{g}_{m}")
            tw = st["t_sb"][:, (m % 2) * H2:(m % 2 + 1) * H2]
            for xc in range(2):
                nc.tensor.matmul(act_ps[:, xc * H2:(xc + 1) * H2],
                                 uxt_sb[:, xc * H:(xc + 1) * H], tw,
                                 start=True, stop=True)
            act_sb = act_pool.tile([H, 2 * H2], f16, tag="act")
            nc.scalar.activation(act_sb[:], act_ps[:],
                                 mybir.ActivationFunctionType.Lrelu,
                                 alpha=LRELU_SLOPE)
            st["act"].append(act_sb)
            if nr8:
                act8_sb = act_pool.tile([H, 2 * H2], fp8, tag="act8")
                nc.gpsimd.tensor_copy(act8_sb[:], act_sb[:])
                st["act8"].append(act8_sb)

        def _emit_group(g, pending_b, acts_cur, x_next):
            wg_sb = w_pool.tile([H, 2 * k32 * GW], f32r, tag="wg",
                                name=f"wg_{g}")
            wgv = wg_sb[:].rearrange("p (c r g w) -> p c r g w", c=2, r=k32, g=G)
            wg8_sb = None
            if nr8:
                wg8_sb = w_pool.tile([H, 2 * nr8 * GW], fp8, tag="wg8",
                                     name=f"wg8_{g}")
            act_sbs = acts_cur["act"]
            act8_sbs = acts_cur["act8"]
            nxt = {"act": [], "act8": []} if x_next is not None else None

            # pass A for image m of this group; W evacs right after the
            # producing matmuls so ps_w recycles fast.
            # fp16 part: nt/W_ps columns are (j, r_local)-major, so the
            # Toeplitz j-band of each X-chunk is a CONTIGUOUS column window:
            # X-chunk0 only reaches j<=71, chunk1 only j>=56; j in [56,72)
            # accumulates, the rest first-write.
            # fp8 part: one DoubleRow matmul per yc contracts both X-chunks.
            def _emit_a_split(m):
                act_sb = act_sbs[m]
                jwin = ((0, 72), (56, H))
                dst8v = wg8_sb[:].rearrange(
                    "p (c r g w) -> p c r g w", c=2, r=nr8, g=G)
                act8 = act8_sbs[m][:].rearrange("p (x y) -> p x y", x=2)
                nt8v = nt8_sb[:].rearrange("p (x c) -> p x c", x=2)
                for yc in range(2):
                    w_ps8 = ps_w.tile([H, 512], f32, tag="wps",
                                      name=f"wps8_{g}_{m}_{yc}")
                    nc.tensor.matmul(
                        w_ps8[:], act8[:, :, yc * H:(yc + 1) * H], nt8v,
                        start=True, stop=True, perf_mode=DR)
                    src8 = w_ps8[:].rearrange("p (j r) -> p r j", r=nr8)
                    if yc == 0:
                        nc.scalar.activation(dst8v[:, yc, :, m], src8,
                                             mybir.ActivationFunctionType.Copy,
                                             scale=s_e)
                    else:
                        nc.vector.tensor_scalar_mul(dst8v[:, yc, :, m],
                                                    src8, s_e)
                for yc in range(2):
                    w_ps = ps_w.tile([H, 512], f32, tag="wps",
                                     name=f"wps_{g}_{m}_{yc}")
                    for xc in range(2):
                        j0, j1 = jwin[xc]
                        nc.tensor.matmul(
                            w_ps[:, j0 * k32:j1 * k32],
                            act_sb[:, xc * H2 + yc * H: xc * H2 + (yc + 1) * H],
                            nt_sb[xc][:, j0 * k32:j1 * k32],
                            start=(xc == 0), stop=(xc == 1),
                            skip_group_check=True)
                    src = w_ps[:, :k32 * H].rearrange("p (j r) -> p r j", r=k32)
                    if yc == 0:
                        nc.vector.tensor_copy(wgv[:, yc, :, m], src)
                    else:
                        nc.scalar.activation(wgv[:, yc, :, m], src,
                                             mybir.ActivationFunctionType.Copy)

            def _emit_a(m):
                if os.environ.get("AFA_WSPLIT", "0") == "1":
                    _emit_a_split(m)
                    return
                act_sb = act_sbs[m]
                if nr8:
                    act8 = act8_sbs[m][:].rearrange("p (x y) -> p x y", x=2)
                    nt8v = nt8_sb[:].rearrange("p (x c) -> p x c", x=2)
                    w_ps8 = ps_w.tile([H, 1024], f32, tag="wps",
                                      name=f"wps8_{g}_{m}")
                    for yc in range(2):
                        nc.tensor.matmul(
                            w_ps8[:, yc * 512:(yc + 1) * 512],
                            act8[:, :, yc * H:(yc + 1) * H],
                            nt8v,
                            start=True, stop=True, perf_mode=DR)
                    # W8 evac: PLAIN fp8 copy on ACT (scales folded into nt8)
                    dst8v = wg8_sb[:].rearrange(
                        "p (c r g w) -> p c r g w", c=2, r=nr8, g=G)
                    if nr8 == 4:
                        src8 = w_ps8[:].rearrange(
                            "p (c j r) -> p c r j", c=2, r=nr8)
                        nc.scalar.activation(dst8v[:, :, :, m], src8,
                                             mybir.ActivationFunctionType.Copy,
                                             scale=s_e)
                    else:
                        for yc in range(2):
                            src8 = w_ps8[:, yc * 512:yc * 512 + nr8 * H] \
                                .rearrange("p (j r) -> p r j", r=nr8)
                            nc.scalar.activation(
                                dst8v[:, yc, :, m], src8,
                                mybir.ActivationFunctionType.Copy, scale=s_e)
                w_ps = ps_w.tile([H, 1024], f32, tag="wps",
                                 name=f"wps_{g}_{m}")
                jwin = ((0, 72), (56, H))
                for yc in range(2):
                    for xc in range(2):
                        j0, j1 = jwin[xc]
                        nc.tensor.matmul(
                            w_ps[:, yc * 512 + j0 * k32:yc * 512 + j1 * k32],
                            act_sb[:, xc * H2 + yc * H: xc * H2 + (yc + 1) * H],
                            nt_sb[xc][:, j0 * k32:j1 * k32],
                            start=(xc == 0), stop=(xc == 1),
                            skip_group_check=True)
                # W32 evac: RAW fp32 copy on DVE (consumed as float32r)
                if k32 == 4:
                    src = w_ps[:].rearrange("p (c j r) -> p c r j", c=2, r=k32)
                    nc.vector.tensor_copy(wgv[:, :, :, m], src)
                else:
                    for yc in range(2):
                        src = w_ps[:, yc * 512:yc * 512 + k32 * H].rearrange(
                            "p (j r) -> p r j", r=k32)
                        nc.vector.tensor_copy(wgv[:, yc, :, m], src)

            # pass B over image PAIRS: each unit is a 256-col matmul touching
            # only images (p*2, p*2+1), so chunk ci only needs W of pair<=ci
            # from the previous group -> full-slot slack at pbd=0.
            state = {"nmm": [0, 0], "out_ps": None}
            units = [(kind, p, yc, r)
                     for p in range(G // 2)
                     for kind, yc, r in
                     ([("32", yc, r) for yc in range(2) for r in range(k32)]
                      + [("8", 0, rl) for rl in range(nr8)])]
            nu_pair = len(units) // (G // 2)

            def _pass_b(ci, chunks=G):
                if state["out_ps"] is None:
                    state["out_ps"] = ps_out.tile([H, GW], f32, tag="ops",
                                                  name=f"ops_{g}")
                out_ps = state["out_ps"]
                n0 = (ci * len(units)) // chunks
                n1 = ((ci + 1) * len(units)) // chunks
                for kind, p, yc, r in units[n0:n1]:
                    state["nmm"][p] += 1
                    ow = out_ps[:, p * 2 * H:(p + 1) * 2 * H]
                    first = state["nmm"][p] == 1
                    last = state["nmm"][p] == nu_pair
                    if kind == "32":
                        nc.tensor.matmul(
                            ow,
                            mt_sb[yc][:, r * H:(r + 1) * H],
                            wgv[:, yc, r, p * 2:(p + 1) * 2],
                            start=first, stop=last, skip_group_check=True)
                    else:
                        mt8v = mt8_sb[:].rearrange("p (c ri) -> p c ri", c=2)
                        wg8v = wg8_sb[:].rearrange(
                            "p (c r g w) -> p c r g w", c=2, r=nr8, g=G)
                        nc.tensor.matmul(
                            ow,
                            mt8v[:, :, r * H:(r + 1) * H],
                            wg8v[:, :, r, p * 2:(p + 1) * 2],
                            start=first, stop=last,
                            perf_mode=DR, skip_group_check=True)
                if ci == chunks - 1:
                    og = out_d[g * G:(g + 1) * G].rearrange("g h w -> h g w")
                    out_sb = osb_pool.tile([H, GW], f32, tag="osb")
                    oe = os.environ.get("AFA_OE", "v")
                    if os.environ.get("AFA_OSPLIT", "0") == "1":
                        for p in range(G // 2):
                            nc.vector.tensor_copy(
                                out_sb[:, p * 2 * H:(p + 1) * 2 * H],
                                out_ps[:, p * 2 * H:(p + 1) * 2 * H])
                    elif oe == "s":
                        nc.scalar.activation(out_sb[:], out_ps[:],
                                             mybir.ActivationFunctionType.Copy)
                    else:
                        nc.vector.tensor_copy(out_sb[:], out_ps[:])
                    nc.sync.dma_start(
                        og, out_sb[:].rearrange("h (g w) -> h g w", g=G))

            # slot loop: A+evacs, then B(g-1) chunk, then next-group S1 piece
            s1pos = os.environ.get("AFA_S1POS", "late")
            bch = int(os.environ.get("AFA_BCH", str(G)))   # B chunk count
            bst = int(os.environ.get("AFA_BST", "0"))      # B start slot
            for m in range(G):
                if s1pos == "early" and nxt is not None:
                    if m % 2 == 0:
                        _emit_s1a(g + 1, m // 2, x_next, nxt)
                    _emit_s1b(g + 1, m, nxt)
                _emit_a(m)
                if pending_b is not None and bst <= m < bst + bch:
                    pending_b(m - bst, chunks=bch)
                if s1pos != "early" and nxt is not None:
                    if m % 2 == 0:
                        _emit_s1a(g + 1, m // 2, x_next, nxt)
                    _emit_s1b(g + 1, m, nxt)

            return _pass_b, nxt

        def _emit_all_groups():
            ngroups = n_img // G
            pending = None
            x_next = _fetch_x(0)
            # prologue: S1 for group 0
            acts_next = {"act": [], "act8": []}
            for half in range(G // 2):
                _emit_s1a(0, half, x_next, acts_next)
                _emit_s1b(0, half * 2, acts_next)
                _emit_s1b(0, half * 2 + 1, acts_next)
            for g in range(ngroups):
                x_cur = x_next
                x_next = _fetch_x(g + 1) if g + 1 < ngroups else None
                acts_cur = acts_next
                pending, acts_next = _emit_group(g, pending, acts_cur, x_next)
            if pending is not None:
                bch = int(os.environ.get("AFA_BCH", str(G)))
                for ci in range(bch):
                    pending(ci, chunks=bch)

        if loop_reps > 1:
            with tc.For_i(0, loop_reps, 1):
                _emit_all_groups()
        else:
            _emit_all_groups()


_NC_CACHE = {}


def _build_nc(n_img=N_IMG, k32=4, nr8=4, group=None, s_e=1.0, loop_reps=1):
    if group is None:
        group = GROUP
    key = (n_img, k32, nr8, group, loop_reps, round(s_e, 12),
           os.environ.get("AFA_PSO", ""), os.environ.get("AFA_S1POS", ""),
           os.environ.get("AFA_BCH", ""), os.environ.get("AFA_BST", ""),
           os.environ.get("AFA_OSPLIT", ""), os.environ.get("AFA_G", ""),
           os.environ.get("AFA_WSPLIT", ""), os.environ.get("AFA_OE", ""),
           os.environ.get("AFA_PSW", ""), os.environ.get("AFA_PSS", ""),
           os.environ.get("AFA_PBD", ""), os.environ.get("AFA_TMPB", ""),
           os.environ.get("AFA_ACTB", ""), os.environ.get("AFA_WB", ""))
    if key in _NC_CACHE:
        return _NC_CACHE[key]
    nc = bacc.Bacc("TRN2", target_bir_lowering=False, debug=False)
    f16 = mybir.dt.float16
    f32 = mybir.dt.float32
    f32r = mybir.dt.float32r
    fp8 = mybir.dt.float8e4
    RC = k32 * H
    x_d = nc.dram_tensor("x", [n_img, H, H], f16, kind="ExternalInput").ap()
    uyt_d = nc.dram_tensor("uyt", [H, H2], f16, kind="ExternalInput").ap()
    uxt_d = nc.dram_tensor("uxt", [H, H2], f16, kind="ExternalInput").ap()
    nt_d = nc.dram_tensor("nt", [2, H, RC], f16, kind="ExternalInput").ap()
    mt32_d = nc.dram_tensor("mt32", [2, H, RC], f32r, kind="ExternalInput").ap()
    ins = [x_d, uyt_d, uxt_d, nt_d, mt32_d]
    if nr8:
        nt8_d = nc.dram_tensor("nt8", [H, 2 * nr8 * H], fp8,
                               kind="ExternalInput").ap()
        mt8_d = nc.dram_tensor("mt8", [H, 2 * nr8 * H], fp8,
                               kind="ExternalInput").ap()
        ins += [nt8_d, mt8_d]
    out_d = nc.dram_tensor("out", [n_img, H, H], f32, kind="ExternalOutput").ap()
    with tile.TileContext(nc) as tc:
        _build_tile_program(tc, [out_d], ins, n_img=n_img, k32=k32, nr8=nr8,
                            group=group, s_e=s_e, loop_reps=loop_reps)
    nc.compile()
    _NC_CACHE[key] = nc
    return nc


def _pick_rank(filt):
    """Smallest rank whose weighted-truncation error estimate fits the
    error budget (harness gate 2e-2; leave room for fp16/fp8 quantization).
    For the reference's sinc filter this lands on 8."""
    if RANK_ENV:
        return int(RANK_ENV)
    F = np.asarray(filt, np.float64)
    kf = F.shape[0]
    D = _ac_matrix(H, H2)
    Uu = _ac_matrix(H2, H)
    Zs = [D @ _shift_mat(H2, u - kf // 2) @ Uu for u in range(kf)]
    B = np.zeros((kf, kf))
    for u in range(kf):
        for v in range(u, kf):
            B[u, v] = B[v, u] = np.sum(Zs[u] * Zs[v])
    w, V = np.linalg.eigh(B)
    Bh = (V * np.sqrt(np.maximum(w, 1e-12))) @ V.T
    s = np.linalg.svd(Bh @ F @ Bh, compute_uv=False)
    nrm = np.sqrt(np.sum(s * s))
    for r in range(4, 16):
        if r >= len(s) or np.sqrt(np.sum(s[r:] ** 2)) <= 4e-3 * nrm:
            return r
    return 16


def _split_rank(rank):
    k32 = int(K32_ENV) if K32_ENV else max(rank - 4, rank // 2)
    k32 = min(k32, rank)
    return k32, rank - k32


def _make_in_maps(x, filt, rank, consts=None):
    k32, nr8 = _split_rank(rank)
    if consts is None:
        consts = _make_consts(filt, rank, k32)
    f16 = np.float16
    imgs = x.reshape(N_CORES, N_IMG, H, H)
    base = {
        "uyt": consts["uyt"].astype(f16), "uxt": consts["uxt"].astype(f16),
        "nt": consts["nt"].astype(f16), "mt32": consts["mt32"],
    }
    if consts["nr8"]:
        base["nt8"] = consts["nt8"]
        base["mt8"] = consts["mt8"]
    return [{"x": np.ascontiguousarray(imgs[core]).astype(f16), **base}
            for core in range(N_CORES)]


_RUNNER_CACHE = {}


def _get_runner(nc):
    """Persistent jitted 8-core runner (mirrors bass2jax.run_bass_via_pjrt's
    multi-core path) so repeated kernel() calls reuse one compiled executable."""
    if id(nc) in _RUNNER_CACHE:
        return _RUNNER_CACHE[id(nc)]
    import jax
    from jax.sharding import Mesh, PartitionSpec
    from jax.experimental.shard_map import shard_map
    from concourse.bass2jax import (_bass_exec_p, install_neuronx_cc_hook,
                                    partition_id_tensor)
    install_neuronx_cc_hook()
    in_names, out_names, out_avals, zero_outs = [], [], [], []
    for alloc in nc.m.functions[0].allocations:
        if not isinstance(alloc, mybir.MemoryLocationSet):
            continue
        name = alloc.memorylocations[0].name
        if alloc.kind == "ExternalInput":
            if nc.partition_id_tensor is not None and name == nc.partition_id_tensor.name:
                continue
            in_names.append(name)
        elif alloc.kind == "ExternalOutput":
            out_names.append(name)
            shape = tuple(alloc.tensor_shape)
            dtype = mybir.dt.np(alloc.dtype)
            out_avals.append(jax.core.ShapedArray(shape, dtype))
            zero_outs.append(np.zeros(shape, dtype))
    n_params = len(in_names)
    all_in_names = in_names + out_names
    if nc.partition_id_tensor is not None:
        all_in_names = all_in_names + [nc.partition_id_tensor.name]

    def _body(*args):
        operands = list(args)
        if nc.partition_id_tensor is not None:
            operands.append(partition_id_tensor())
        return tuple(_bass_exec_p.bind(
            *operands,
            out_avals=tuple(out_avals),
            in_names=tuple(all_in_names),
            out_names=tuple(out_names),
            lowering_input_output_aliases=(),
            sim_require_finite=True,
            sim_require_nnan=True,
            nc=nc,
        ))

    donate = tuple(range(n_params, n_params + len(out_names)))
    devices = jax.devices()[:N_CORES]
    mesh = Mesh(np.asarray(devices), ("core",))
    in_specs = (PartitionSpec("core"),) * (n_params + len(out_names))
    out_specs = (PartitionSpec("core"),) * len(out_names)
    sharded = jax.jit(
        shard_map(_body, mesh=mesh, in_specs=in_specs, out_specs=out_specs,
                  check_rep=False),
        donate_argnums=donate, keep_unused=True)
    runner = (sharded, in_names, out_names, out_avals, zero_outs)
    _RUNNER_CACHE[id(nc)] = runner
    return runner


def run(x, filt):
    """Run on 8 cores. Returns out [B,C,H,W] f32."""
    x = np.ascontiguousarray(np.asarray(x, dtype=np.float32))
    filt = np.asarray(filt, dtype=np.float32)
    B, C, Hh, Ww = x.shape
    assert (Hh, Ww) == (H, H) and B * C == N_CORES * N_IMG
    rank = _pick_rank(filt)
    k32, nr8 = _split_rank(rank)
    consts = _make_consts(filt, rank, k32)
    in_maps = _make_in_maps(x, filt, rank, consts)
    nc = _build_nc(k32=k32, nr8=nr8, s_e=consts["s_e"])
    try:
        sharded, in_names, out_names, out_avals, zero_outs = _get_runner(nc)
        concat_in = [np.concatenate([in_maps[c][nm] for c in range(N_CORES)], axis=0)
                     for nm in in_names]
        concat_zero = [np.zeros((N_CORES * z.shape[0], *z.shape[1:]), z.dtype)
                       for z in zero_outs]
        outs = sharded(*concat_in, *concat_zero)
        oi = out_names.index("out")
        out = np.asarray(outs[oi]).reshape(N_CORES, *out_avals[oi].shape)
    except Exception:
        res = run_bass_kernel_spmd(nc, in_maps, core_ids=list(range(N_CORES)))
        out = np.stack([res.results[c]["out"] for c in range(N_CORES)])
    return out.reshape(B, C, H, H).astype(np.float32, copy=False)


def kernel(x, filt):
    return run(x, filt)


# revision 20
# speedup vs baseline: 1.0173x; 1.0173x over previous
"""AliasFreeActivation (upsample2x -> leaky_relu -> 31x31 depthwise sinc conv
-> downsample2x) as a Trainium2 Bass/Tile kernel, data-parallel over 8 cores.

Math (per [128,128] image; B*C = 512 images, 64 per core):
  out = Dy @ Conv_F(lrelu(Uy @ x @ Ux^T)) @ Dx^T
With F = sum_r g_r h_r^T (weighted SVD of the 31x31 filter, rank 8):
  out = sum_r M_r @ act @ N_r^T
  M_r = Dy @ Toeplitz(g_r) [128,256],  N_r = Dx @ Toeplitz(h_r) [128,256]
  act = lrelu(Uy @ x @ Ux^T) [256,256]

Precision/engine split (v2):
  ranks 0..k32-1 : A-pass fp16 (banded), W evac'd to fp16 by a plain DVE
                   copy, B-pass fp16 matmuls over image-pair units.
  ranks k32..    : A-pass fp8 DoubleRow, W evac = PLAIN Pool copy to fp8
                   (the old s_e scale is folded into nt8), B-pass fp8 DR.
PSUM evacuation engines: ACT = lrelu + tmpT + out; DVE = W32; Pool
(gpsimd, which CAN read PSUM on TRN2) = W8 + act8.

Device dataflow per image (out[m,n] = sum_k lhsT[k,m] rhs[k,n]):
  S1a: tmpT[c,Y]    = sum_y x[y,c] UyT[y,Y]
  S1b: actT[X,Y]    = sum_c UxT[c,X] tmpT[c,Y]   (+ lrelu on evac)
  A:   W[Y,(r,j)]   = sum_X actT[X,Y] NT[X,(r,j)]
  B:   out[i,(m,j)] = sum_{r,Yc} MT[Yc,(r,i)] W[Yc,(r,m,j)]   (PSUM accum)
"""
import contextlib
import os

import numpy as np

import concourse.bass as bass
import concourse.mybir as mybir
import concourse.tile as tile
from concourse import bacc
from concourse.bass_utils import run_bass_kernel_spmd

H = 128
H2 = 256
KF = 31
LRELU_SLOPE = 0.01
RANK_ENV = os.environ.get("AFA_RANK")
N_CORES = 8
N_IMG = 64                      # images per core (512 / 8)
GROUP = int(os.environ.get("AFA_G", "4"))
K32_ENV = os.environ.get("AFA_K32")


# ---------------- host-side constants ----------------

def _ac_matrix(out_n, in_n):
    scale = (in_n - 1) / (out_n - 1)
    c = np.arange(out_n, dtype=np.float64) * scale
    i0 = np.clip(np.floor(c).astype(np.int64), 0, in_n - 2)
    w = c - i0
    M = np.zeros((out_n, in_n), dtype=np.float64)
    M[np.arange(out_n), i0] = 1.0 - w
    M[np.arange(out_n), i0 + 1] = w
    return M


def _toeplitz_same(h, n):
    T = np.zeros((n, n), dtype=np.float64)
    for u in range(len(h)):
        d = u - len(h) // 2
        if d >= 0:
            idx = np.arange(0, n - d)
        else:
            idx = np.arange(-d, n)
        T[idx, idx + d] += h[u]
    return T


def _shift_mat(n, d):
    S = np.zeros((n, n))
    idx = np.arange(0, n - d) if d >= 0 else np.arange(-d, n)
    S[idx, idx + d] = 1.0
    return S


def _weighted_rank(F, rank):
    """Rank-`rank` approx of F minimizing the end-to-end error under the
    signal model act ~ U x U^T with white x: err = ||B^1/2 (F-Fr) B^1/2||_F
    with B the Gram of the composed per-tap maps Z_u = D S_u U."""
    kf = F.shape[0]
    D = _ac_matrix(H, H2)
    Uu = _ac_matrix(H2, H)
    Zs = [D @ _shift_mat(H2, u - kf // 2) @ Uu for u in range(kf)]
    B = np.zeros((kf, kf))
    for u in range(kf):
        for v in range(u, kf):
            B[u, v] = B[v, u] = np.sum(Zs[u] * Zs[v])
    w, V = np.linalg.eigh(B)
    w = np.maximum(w, 1e-12)
    Bh = (V * np.sqrt(w)) @ V.T
    Bih = (V / np.sqrt(w)) @ V.T
    Gm = Bh @ F @ Bh
    U_, S_, Vt_ = np.linalg.svd(Gm)
    Fr = Bih @ (U_[:, :rank] * S_[:rank]) @ Vt_[:rank] @ Bih
    return Fr


def _sample_act(Uu, n=4):
    rng = np.random.default_rng(1234)
    X = rng.standard_normal((n, H, H))
    A = Uu @ X @ Uu.T
    return np.where(A >= 0, A, LRELU_SLOPE * A)


def _make_consts(filt, rank, k32):
    """Build device constants.

    Ranks [0, k32): nt fp16 in seg layout (columns (j, r_local)-major for the
    banded A-pass 2D APs), mt as RAW fp32 (used as float32r), rank-major.
    Ranks [k32, rank): nt8/mt8 fp8 DoubleRow-interleaved, with per-rank scale
    n_r chosen so that nt8, W8 (= s_a n_r act N^T, evac'd by a PLAIN copy)
    and mt8 (= M / (s_a n_r)) all sit inside the fp8e4m3 normal range.
    s_a is folded into uyt (lrelu is positively homogeneous) and 1/s_a into
    nt/mt-fp32, so every rank's B contribution lands in out PSUM at scale 1.
    """
    F = np.asarray(filt, dtype=np.float64)
    if rank < min(F.shape):
        F = _weighted_rank(F, rank)
    U, S, Vt = np.linalg.svd(F)
    D = _ac_matrix(H, H2)
    Uu = _ac_matrix(H2, H)
    uyt = np.ascontiguousarray(Uu.T)               # [128 y, 256 Y]

    nr8 = rank - k32
    Ms = [D @ _toeplitz_same(U[:, r] * np.sqrt(S[r]), H2) for r in range(rank)]
    Ns = [D @ _toeplitz_same(Vt[r, :] * np.sqrt(S[r]), H2) for r in range(rank)]

    # fp8 scales: act8 = s_a * act (s_a folded into uyt)
    act = _sample_act(Uu)
    actmax = np.abs(act).max() * 1.15
    s_a = 100.0 / actmax

    # fp16/fp32 ranks: nt gets 1/s_a (act arrives pre-scaled by s_a);
    # mt stays exact fp32.
    nt = np.zeros((2, H, k32 * H), dtype=np.float32)
    mt32 = np.zeros((2, H, k32 * H), dtype=np.float16)
    for r in range(k32):
        cols = np.arange(H) * k32 + r              # (j, r_local)-major
        for c in range(2):
            nt[c, :, cols] = (Ns[r][:, c * H:(c + 1) * H] / s_a).astype(np.float32)
            mt32[c, :, r * H:(r + 1) * H] = \
                Ms[r][:, c * H:(c + 1) * H].T.astype(np.float16)

    out = {"uyt": (uyt * s_a).astype(np.float32), "uxt": uyt.astype(np.float32),
           "nt": nt, "mt32": mt32, "k32": k32, "nr8": nr8, "s_a": float(s_a),
           "s_e": 1.0}
    if nr8 == 0:
        return out

    # ---- fp8 tail: per-rank scales (baseline scheme): wg8 = s_e * W8psum
    # (s_e applied for free by the ACT Copy evac), n_r/m_r sqrt-balanced so
    # act8, nt8, wg8, mt8 all sit inside the fp8e4m3 normal range.
    np8 = mybir.dt.np(mybir.dt.float8e4)
    nt8 = np.zeros((H, 2, nr8 * H), dtype=np.float64)
    mt8 = np.zeros((H, 2, nr8 * H), dtype=np.float64)
    wmaxs = [np.abs(act @ Ns[k32 + i].T).max() * 1.3 for i in range(nr8)]
    mmaxs = [np.abs(Ms[k32 + i]).max() for i in range(nr8)]
    nmaxs = [np.abs(Ns[k32 + i]).max() for i in range(nr8)]
    bias = 2.0
    tmp = [np.sqrt(wmaxs[i] * mmaxs[i]) * bias / (s_a * wmaxs[i])
           for i in range(nr8)]           # = n_r * s_e per rank
    s_e = float(np.exp(np.mean(np.log([tmp[i] * nmaxs[i]
                                       for i in range(nr8)]))) / 100.0)
    for i in range(nr8):
        r = k32 + i
        n_r = tmp[i] / s_e
        m_r = 1.0 / (s_a * n_r * s_e)
        for c in range(2):
            # nt8[p, xc, j*nr8+i] = n_r * N_r[j, xc*128+p]
            nt8[:, c, i::nr8] = (n_r * Ns[r][:, c * H:(c + 1) * H]).T
            # mt8[p, yc, i*H+k] = m_r * M_r[k, yc*128+p]
            mt8[:, c, i * H:(i + 1) * H] = (m_r * Ms[r][:, c * H:(c + 1) * H]).T
    out["s_e"] = s_e
    out["nt8"] = nt8.astype(np8).reshape(H, 2 * nr8 * H)
    out["mt8"] = mt8.astype(np8).reshape(H, 2 * nr8 * H)
    return out


# ---------------- device program ----------------

def _build_tile_program(tc, outs, ins, *, n_img, k32, nr8, group,
                        s_e=1.0, loop_reps=1):
    nc = tc.nc
    if nr8:
        x_d, uyt_d, uxt_d, nt_d, mt32_d, nt8_d, mt8_d = ins
    else:
        x_d, uyt_d, uxt_d, nt_d, mt32_d = ins
    out_d = outs[0]
    RC = k32 * H
    G = group
    GW = G * H
    assert G in (2, 4)
    assert n_img % G == 0
    f16 = mybir.dt.float16
    f32 = mybir.dt.float32
    f32r = mybir.dt.float32r
    fp8 = mybir.dt.float8e4
    DR = mybir.MatmulPerfMode.DoubleRow

    ctx = contextlib.ExitStack()
    with ctx:
        const_pool = ctx.enter_context(tc.tile_pool(name="consts", bufs=1))
        x_pool = ctx.enter_context(tc.tile_pool(name="x", bufs=2))
        tmp_pool = ctx.enter_context(tc.tile_pool(
            name="tmp", bufs=int(os.environ.get("AFA_TMPB", "3"))))
        act_pool = ctx.enter_context(tc.tile_pool(
            name="act", bufs=int(os.environ.get("AFA_ACTB", "8"))))
        w_pool = ctx.enter_context(tc.tile_pool(
            name="w", bufs=int(os.environ.get("AFA_WB", "2"))))
        osb_pool = ctx.enter_context(tc.tile_pool(name="osb", bufs=2))
        # PSUM banks: ps_s 3 + ps_w 2x2 + ps_out 1 = 8
        ps_s = ctx.enter_context(tc.tile_pool(
            name="ps_s", bufs=int(os.environ.get("AFA_PSS", "1")), space="PSUM"))
        ps_w = ctx.enter_context(tc.tile_pool(
            name="ps_w", bufs=int(os.environ.get("AFA_PSW", "3")), space="PSUM"))
        ps_out = ctx.enter_context(tc.tile_pool(
            name="ps_out", bufs=int(os.environ.get("AFA_PSO", "1")),
            space="PSUM"))

        uyt_sb = const_pool.tile([H, H2], f16, tag="uyt")
        nc.sync.dma_start(uyt_sb[:], uyt_d[:])
        uxt_sb = const_pool.tile([H, H2], f16, tag="uxt")
        nc.sync.dma_start(uxt_sb[:], uxt_d[:])
        nt_sb = []
        mt_sb = []
        for c in range(2):
            t = const_pool.tile([H, RC], f16, tag=f"nt{c}", name=f"nt{c}_sb")
            nc.sync.dma_start(t[:], nt_d[c])
            nt_sb.append(t)
            t = const_pool.tile([H, RC], f16, tag=f"mt{c}", name=f"mt{c}_sb")
            nc.sync.dma_start(t[:], mt32_d[c])
            mt_sb.append(t)
        if nr8:
            nt8_sb = const_pool.tile([H, 2 * nr8 * H], fp8, tag="nt8")
            nc.sync.dma_start(nt8_sb[:], nt8_d[:])
            mt8_sb = const_pool.tile([H, 2 * nr8 * H], fp8, tag="mt8")
            nc.sync.dma_start(mt8_sb[:], mt8_d[:])

        def _fetch_x(g):
            x_sb = x_pool.tile([H, GW], f16, tag="x", name=f"x_{g}")
            xg = x_d[g * G:(g + 1) * G].rearrange("g h w -> h g w")
            nc.sync.dma_start(x_sb[:].rearrange("h (g w) -> h g w", g=G), xg)
            return x_sb

        def _emit_s1a(g, half, x_sb, st):
            """S1a matmuls for an image pair + tmpT evac (DVE)."""
            tmpT_ps = ps_s.tile([H, 2 * H2], f32, tag="sp",
                                name=f"tp_{g}_{half}")
            for u in range(2):
                m = half * 2 + u
                nc.tensor.matmul(tmpT_ps[:, u * H2:(u + 1) * H2],
                                 x_sb[:, m * H:(m + 1) * H], uyt_sb[:],
                                 start=True, stop=True)
            t_sb = tmp_pool.tile([H, 2 * H2], f16, tag="tmpT")
            nc.vector.tensor_copy(t_sb[:], tmpT_ps[:])
            st["t_sb"] = t_sb

        def _emit_s1b(g, m, st):
            """S1b matmuls + lrelu (ACT) + act8 (Pool) for one image."""
            act_ps = ps_s.tile([H, 2 * H2], f32, tag="sp", name=f"ap_{g}_{m}")
            tw = st["t_sb"][:, (m % 2) * H2:(m % 2 + 1) * H2]
            for xc in range(2):
                nc.tensor.matmul(act_ps[:, xc * H2:(xc + 1) * H2],
                                 uxt_sb[:, xc * H:(xc + 1) * H], tw,
                                 start=True, stop=True)
            act_sb = act_pool.tile([H, 2 * H2], f16, tag="act")
            nc.scalar.activation(act_sb[:], act_ps[:],
                                 mybir.ActivationFunctionType.Lrelu,
                                 alpha=LRELU_SLOPE)
            st["act"].append(act_sb)
            if nr8:
                act8_sb = act_pool.tile([H, 2 * H2], fp8, tag="act8")
                nc.gpsimd.tensor_copy(act8_sb[:], act_sb[:])
                st["act8"].append(act8_sb)

        def _emit_group(g, pending_b, acts_cur, x_next):
            wg_sb = w_pool.tile([H, 2 * k32 * GW], f16, tag="wg",
                                name=f"wg_{g}")
            wgv = wg_sb[:].rearrange("p (c r g w) -> p c r g w", c=2, r=k32, g=G)
            wg8_sb = None
            if nr8:
                wg8_sb = w_pool.tile([H, 2 * nr8 * GW], fp8, tag="wg8",
                                     name=f"wg8_{g}")
            act_sbs = acts_cur["act"]
            act8_sbs = acts_cur["act8"]
            nxt = {"act": [], "act8": []} if x_next is not None else None

            # pass A for image m of this group; W evacs right after the
            # producing matmuls so ps_w recycles fast.
            # fp16 part: nt/W_ps columns are (j, r_local)-major, so the
            # Toeplitz j-band of each X-chunk is a CONTIGUOUS column window:
            # X-chunk0 only reaches j<=71, chunk1 only j>=56; j in [56,72)
            # accumulates, the rest first-write.
            # fp8 part: one DoubleRow matmul per yc contracts both X-chunks.
            def _emit_a_split(m):
                act_sb = act_sbs[m]
                jwin = ((0, 72), (56, H))
                dst8v = wg8_sb[:].rearrange(
                    "p (c r g w) -> p c r g w", c=2, r=nr8, g=G)
                act8 = act8_sbs[m][:].rearrange("p (x y) -> p x y", x=2)
                nt8v = nt8_sb[:].rearrange("p (x c) -> p x c", x=2)
                for yc in range(2):
                    w_ps8 = ps_w.tile([H, 512], f32, tag="wps",
                                      name=f"wps8_{g}_{m}_{yc}")
                    nc.tensor.matmul(
                        w_ps8[:], act8[:, :, yc * H:(yc + 1) * H], nt8v,
                        start=True, stop=True, perf_mode=DR)
                    src8 = w_ps8[:].rearrange("p (j r) -> p r j", r=nr8)
                    if yc == 0:
                        nc.scalar.activation(dst8v[:, yc, :, m], src8,
                                             mybir.ActivationFunctionType.Copy,
                                             scale=s_e)
                    else:
                        nc.vector.tensor_scalar_mul(dst8v[:, yc, :, m],
                                                    src8, s_e)
                for yc in range(2):
                    w_ps = ps_w.tile([H, 512], f32, tag="wps",
                                     name=f"wps_{g}_{m}_{yc}")
                    for xc in range(2):
                        j0, j1 = jwin[xc]
                        nc.tensor.matmul(
                            w_ps[:, j0 * k32:j1 * k32],
                            act_sb[:, xc * H2 + yc * H: xc * H2 + (yc + 1) * H],
                            nt_sb[xc][:, j0 * k32:j1 * k32],
                            start=(xc == 0), stop=(xc == 1),
                            skip_group_check=True)
                    src = w_ps[:, :k32 * H].rearrange("p (j r) -> p r j", r=k32)
                    if yc == 0:
                        nc.vector.tensor_copy(wgv[:, yc, :, m], src)
                    else:
                        nc.scalar.activation(wgv[:, yc, :, m], src,
                                             mybir.ActivationFunctionType.Copy)

            def _emit_a(m):
                if os.environ.get("AFA_WSPLIT", "0") == "1":
                    _emit_a_split(m)
                    return
                act_sb = act_sbs[m]
                if nr8:
                    act8 = act8_sbs[m][:].rearrange("p (x y) -> p x y", x=2)
                    nt8v = nt8_sb[:].rearrange("p (x c) -> p x c", x=2)
                    w_ps8 = ps_w.tile([H, 1024], f32, tag="wps",
                                      name=f"wps8_{g}_{m}")
                    for yc in range(2):
                        nc.tensor.matmul(
                            w_ps8[:, yc * 512:(yc + 1) * 512],
                            act8[:, :, yc * H:(yc + 1) * H],
                            nt8v,
                            start=True, stop=True, perf_mode=DR)
                    # W8 evac: PLAIN fp8 copy on ACT (scales folded into nt8)
                    dst8v = wg8_sb[:].rearrange(
                        "p (c r g w) -> p c r g w", c=2, r=nr8, g=G)
                    if nr8 == 4:
                        src8 = w_ps8[:].rearrange(
                            "p (c j r) -> p c r j", c=2, r=nr8)
                        nc.scalar.activation(dst8v[:, :, :, m], src8,
                                             mybir.ActivationFunctionType.Copy,
                                             scale=s_e)
                    else:
                        for yc in range(2):
                            src8 = w_ps8[:, yc * 512:yc * 512 + nr8 * H] \
                                .rearrange("p (j r) -> p r j", r=nr8)
                            nc.scalar.activation(
                                dst8v[:, yc, :, m], src8,
                                mybir.ActivationFunctionType.Copy, scale=s_e)
                w_ps = ps_w.tile([H, 1024], f32, tag="wps",
                                 name=f"wps_{g}_{m}")
                jwin = ((0, 72), (56, H))
                for yc in range(2):
                    for xc in range(2):
                        j0, j1 = jwin[xc]
                        nc.tensor.matmul(
                            w_ps[:, yc * 512 + j0 * k32:yc * 512 + j1 * k32],
                            act_sb[:, xc * H2 + yc * H: xc * H2 + (yc + 1) * H],
                            nt_sb[xc][:, j0 * k32:j1 * k32],
                            start=(xc == 0), stop=(xc == 1),
                            skip_group_check=True)
                # W32 evac: RAW fp32 copy on DVE (consumed as float32r)
                if k32 == 4:
                    src = w_ps[:].rearrange("p (c j r) -> p c r j", c=2, r=k32)
                    nc.vector.tensor_copy(wgv[:, :, :, m], src)
                else:
                    for yc in range(2):
                        src = w_ps[:, yc * 512:yc * 512 + k32 * H].rearrange(
                            "p (j r) -> p r j", r=k32)
                        nc.vector.tensor_copy(wgv[:, yc, :, m], src)

            # pass B over image PAIRS: each unit is a 256-col matmul touching
            # only images (p*2, p*2+1), so chunk ci only needs W of pair<=ci
            # from the previous group -> full-slot slack at pbd=0.
            state = {"nmm": [0, 0], "out_ps": None}
            units = [(kind, p, yc, r)
                     for p in range(G // 2)
                     for kind, yc, r in
                     ([("32", yc, r) for yc in range(2) for r in range(k32)]
                      + [("8", 0, rl) for rl in range(nr8)])]
            nu_pair = len(units) // (G // 2)

            def _pass_b(ci, chunks=G):
                if state["out_ps"] is None:
                    state["out_ps"] = ps_out.tile([H, GW], f32, tag="ops",
                                                  name=f"ops_{g}")
                out_ps = state["out_ps"]
                n0 = (ci * len(units)) // chunks
                n1 = ((ci + 1) * len(units)) // chunks
                for kind, p, yc, r in units[n0:n1]:
                    state["nmm"][p] += 1
                    ow = out_ps[:, p * 2 * H:(p + 1) * 2 * H]
                    first = state["nmm"][p] == 1
                    last = state["nmm"][p] == nu_pair
                    if kind == "32":
                        nc.tensor.matmul(
                            ow,
                            mt_sb[yc][:, r * H:(r + 1) * H],
                            wgv[:, yc, r, p * 2:(p + 1) * 2],
                            start=first, stop=last, skip_group_check=True)
                    else:
                        mt8v = mt8_sb[:].rearrange("p (c ri) -> p c ri", c=2)
                        wg8v = wg8_sb[:].rearrange(
                            "p (c r g w) -> p c r g w", c=2, r=nr8, g=G)
                        nc.tensor.matmul(
                            ow,
                            mt8v[:, :, r * H:(r + 1) * H],
                            wg8v[:, :, r, p * 2:(p + 1) * 2],
                            start=first, stop=last,
                            perf_mode=DR, skip_group_check=True)
                if ci == chunks - 1:
                    og = out_d[g * G:(g + 1) * G].rearrange("g h w -> h g w")
                    out_sb = osb_pool.tile([H, GW], f32, tag="osb")
                    oe = os.environ.get("AFA_OE", "v")
                    if os.environ.get("AFA_OSPLIT", "0") == "1":
                        for p in range(G // 2):
                            nc.vector.tensor_copy(
                                out_sb[:, p * 2 * H:(p + 1) * 2 * H],
                                out_ps[:, p * 2 * H:(p + 1) * 2 * H])
                    elif oe == "s":
                        nc.scalar.activation(out_sb[:], out_ps[:],
                                             mybir.ActivationFunctionType.Copy)
                    else:
                        nc.vector.tensor_copy(out_sb[:], out_ps[:])
                    nc.sync.dma_start(
                        og, out_sb[:].rearrange("h (g w) -> h g w", g=G))

            # slot loop: A+evacs, then B(g-1) chunk, then next-group S1 piece
            s1pos = os.environ.get("AFA_S1POS", "late")
            bch = int(os.environ.get("AFA_BCH", str(G)))   # B chunk count
            bst = int(os.environ.get("AFA_BST", "0"))      # B start slot
            for m in range(G):
                if s1pos == "early" and nxt is not None:
                    if m % 2 == 0:
                        _emit_s1a(g + 1, m // 2, x_next, nxt)
                    _emit_s1b(g + 1, m, nxt)
                _emit_a(m)
                if pending_b is not None and bst <= m < bst + bch:
                    pending_b(m - bst, chunks=bch)
                if s1pos != "early" and nxt is not None:
                    if m % 2 == 0:
                        _emit_s1a(g + 1, m // 2, x_next, nxt)
                    _emit_s1b(g + 1, m, nxt)

            return _pass_b, nxt

        def _emit_all_groups():
            ngroups = n_img // G
            pending = None
            x_next = _fetch_x(0)
            # prologue: S1 for group 0
            acts_next = {"act": [], "act8": []}
            for half in range(G // 2):
                _emit_s1a(0, half, x_next, acts_next)
                _emit_s1b(0, half * 2, acts_next)
                _emit_s1b(0, half * 2 + 1, acts_next)
            for g in range(ngroups):
                x_cur = x_next
                x_next = _fetch_x(g + 1) if g + 1 < ngroups else None
                acts_cur = acts_next
                pending, acts_next = _emit_group(g, pending, acts_cur, x_next)
            if pending is not None:
                bch = int(os.environ.get("AFA_BCH", str(G)))
                for ci in range(bch):
                    pending(ci, chunks=bch)

        if loop_reps > 1:
            with tc.For_i(0, loop_reps, 1):
                _emit_all_groups()
        else:
            _emit_all_groups()


_NC_CACHE = {}


def _build_nc(n_img=N_IMG, k32=4, nr8=4, group=None, s_e=1.0, loop_reps=1):
    if group is None:
        group = GROUP
    key = (n_img, k32, nr8, group, loop_reps, round(s_e, 12),
           os.environ.get("AFA_PSO", ""), os.environ.get("AFA_S1POS", ""),
           os.environ.get("AFA_BCH", ""), os.environ.get("AFA_BST", ""),
           os.environ.get("AFA_OSPLIT", ""), os.environ.get("AFA_G", ""),
           os.environ.get("AFA_WSPLIT", ""), os.environ.get("AFA_OE", ""),
           os.environ.get("AFA_PSW", ""), os.environ.get("AFA_PSS", ""),
           os.environ.get("AFA_PBD", ""), os.environ.get("AFA_TMPB", ""),
           os.environ.get("AFA_ACTB", ""), os.environ.get("AFA_WB", ""))
    if key in _NC_CACHE:
        return _NC_CACHE[key]
    nc = bacc.Bacc("TRN2", target_bir_lowering=False, debug=False)
    f16 = mybir.dt.float16
    f32 = mybir.dt.float32
    f32r = mybir.dt.float32r
    fp8 = mybir.dt.float8e4
    RC = k32 * H
    x_d = nc.dram_tensor("x", [n_img, H, H], f16, kind="ExternalInput").ap()
    uyt_d = nc.dram_tensor("uyt", [H, H2], f16, kind="ExternalInput").ap()
    uxt_d = nc.dram_tensor("uxt", [H, H2], f16, kind="ExternalInput").ap()
    nt_d = nc.dram_tensor("nt", [2, H, RC], f16, kind="ExternalInput").ap()
    mt32_d = nc.dram_tensor("mt32", [2, H, RC], f16, kind="ExternalInput").ap()
    ins = [x_d, uyt_d, uxt_d, nt_d, mt32_d]
    if nr8:
        nt8_d = nc.dram_tensor("nt8", [H, 2 * nr8 * H], fp8,
                               kind="ExternalInput").ap()
        mt8_d = nc.dram_tensor("mt8", [H, 2 * nr8 * H], fp8,
                               kind="ExternalInput").ap()
        ins += [nt8_d, mt8_d]
    out_d = nc.dram_tensor("out", [n_img, H, H], f32, kind="ExternalOutput").ap()
    with tile.TileContext(nc) as tc:
        _build_tile_program(tc, [out_d], ins, n_img=n_img, k32=k32, nr8=nr8,
                            group=group, s_e=s_e, loop_reps=loop_reps)
    nc.compile()
    _NC_CACHE[key] = nc
    return nc


def _pick_rank(filt):
    """Smallest rank whose weighted-truncation error estimate fits the
    error budget (harness gate 2e-2; leave room for fp16/fp8 quantization).
    For the reference's sinc filter this lands on 8."""
    if RANK_ENV:
        return int(RANK_ENV)
    F = np.asarray(filt, np.float64)
    kf = F.shape[0]
    D = _ac_matrix(H, H2)
    Uu = _ac_matrix(H2, H)
    Zs = [D @ _shift_mat(H2, u - kf // 2) @ Uu for u in range(kf)]
    B = np.zeros((kf, kf))
    for u in range(kf):
        for v in range(u, kf):
            B[u, v] = B[v, u] = np.sum(Zs[u] * Zs[v])
    w, V = np.linalg.eigh(B)
    Bh = (V * np.sqrt(np.maximum(w, 1e-12))) @ V.T
    s = np.linalg.svd(Bh @ F @ Bh, compute_uv=False)
    nrm = np.sqrt(np.sum(s * s))
    for r in range(4, 16):
        if r >= len(s) or np.sqrt(np.sum(s[r:] ** 2)) <= 4e-3 * nrm:
            return r
    return 16


def _split_rank(rank):
    k32 = int(K32_ENV) if K32_ENV else max(rank - 4, rank // 2)
    k32 = min(k32, rank)
    return k32, rank - k32


def _make_in_maps(x, filt, rank, consts=None):
    k32, nr8 = _split_rank(rank)
    if consts is None:
        consts = _make_consts(filt, rank, k32)
    f16 = np.float16
    imgs = x.reshape(N_CORES, N_IMG, H, H)
    base = {
        "uyt": consts["uyt"].astype(f16), "uxt": consts["uxt"].astype(f16),
        "nt": consts["nt"].astype(f16), "mt32": consts["mt32"],
    }
    if consts["nr8"]:
        base["nt8"] = consts["nt8"]
        base["mt8"] = consts["mt8"]
    return [{"x": np.ascontiguousarray(imgs[core]).astype(f16), **base}
            for core in range(N_CORES)]


_RUNNER_CACHE = {}


def _get_runner(nc):
    """Persistent jitted 8-core runner (mirrors bass2jax.run_bass_via_pjrt's
    multi-core path) so repeated kernel() calls reuse one compiled executable."""
    if id(nc) in _RUNNER_CACHE:
        return _RUNNER_CACHE[id(nc)]
    import jax
    from jax.sharding import Mesh, PartitionSpec
    from jax.experimental.shard_map import shard_map
    from concourse.bass2jax import (_bass_exec_p, install_neuronx_cc_hook,
                                    partition_id_tensor)
    install_neuronx_cc_hook()
    in_names, out_names, out_avals, zero_outs = [], [], [], []
    for alloc in nc.m.functions[0].allocations:
        if not isinstance(alloc, mybir.MemoryLocationSet):
            continue
        name = alloc.memorylocations[0].name
        if alloc.kind == "ExternalInput":
            if nc.partition_id_tensor is not None and name == nc.partition_id_tensor.name:
                continue
            in_names.append(name)
        elif alloc.kind == "ExternalOutput":
            out_names.append(name)
            shape = tuple(alloc.tensor_shape)
            dtype = mybir.dt.np(alloc.dtype)
            out_avals.append(jax.core.ShapedArray(shape, dtype))
            zero_outs.append(np.zeros(shape, dtype))
    n_params = len(in_names)
    all_in_names = in_names + out_names
    if nc.partition_id_tensor is not None:
        all_in_names = all_in_names + [nc.partition_id_tensor.name]

    def _body(*args):
        operands = list(args)
        if nc.partition_id_tensor is not None:
            operands.append(partition_id_tensor())
        return tuple(_bass_exec_p.bind(
            *operands,
            out_avals=tuple(out_avals),
            in_names=tuple(all_in_names),
            out_names=tuple(out_names),
            lowering_input_output_aliases=(),
            sim_require_finite=True,
            sim_require_nnan=True,
            nc=nc,
        ))

    donate = tuple(range(n_params, n_params + len(out_names)))
    devices = jax.devices()[:N_CORES]
    mesh = Mesh(np.asarray(devices), ("core",))
    in_specs = (PartitionSpec("core"),) * (n_params + len(out_names))
    out_specs = (PartitionSpec("core"),) * len(out_names)
    sharded = jax.jit(
        shard_map(_body, mesh=mesh, in_specs=in_specs, out_specs=out_specs,
                  check_rep=False),
        donate_argnums=donate, keep_unused=True)
    runner = (sharded, in_names, out_names, out_avals, zero_outs)
    _RUNNER_CACHE[id(nc)] = runner
    return runner


def run(x, filt):
    """Run on 8 cores. Returns out [B,C,H,W] f32."""
    x = np.ascontiguousarray(np.asarray(x, dtype=np.float32))
    filt = np.asarray(filt, dtype=np.float32)
    B, C, Hh, Ww = x.shape
    assert (Hh, Ww) == (H, H) and B * C == N_CORES * N_IMG
    rank = _pick_rank(filt)
    k32, nr8 = _split_rank(rank)
    consts = _make_consts(filt, rank, k32)
    in_maps = _make_in_maps(x, filt, rank, consts)
    nc = _build_nc(k32=k32, nr8=nr8, s_e=consts["s_e"])
    try:
        sharded, in_names, out_names, out_avals, zero_outs = _get_runner(nc)
        concat_in = [np.concatenate([in_maps[c][nm] for c in range(N_CORES)], axis=0)
                     for nm in in_names]
        concat_zero = [np.zeros((N_CORES * z.shape[0], *z.shape[1:]), z.dtype)
                       for z in zero_outs]
        outs = sharded(*concat_in, *concat_zero)
        oi = out_names.index("out")
        out = np.asarray(outs[oi]).reshape(N_CORES, *out_avals[oi].shape)
    except Exception:
        res = run_bass_kernel_spmd(nc, in_maps, core_ids=list(range(N_CORES)))
        out = np.stack([res.results[c]["out"] for c in range(N_CORES)])
    return out.reshape(B, C, H, H).astype(np.float32, copy=False)


def kernel(x, filt):
    return run(x, filt)


# revision 21
# speedup vs baseline: 1.7069x; 1.6778x over previous
"""AliasFreeActivation (upsample2x -> leaky_relu -> 31x31 depthwise sinc conv
-> downsample2x) as a Trainium2 Bass/Tile kernel, data-parallel over 8 cores.

Math (per [128,128] image; B*C = 512 images, 64 per core):
  out = Dy @ Conv_F(lrelu(Uy @ x @ Ux^T)) @ Dx^T
With F = sum_r g_r h_r^T (weighted SVD of the 31x31 filter, rank 8):
  out = sum_r M_r @ act @ N_r^T
  M_r = Dy @ Toeplitz(g_r) [128,256],  N_r = Dx @ Toeplitz(h_r) [128,256]
  act = lrelu(Uy @ x @ Ux^T) [256,256]

Precision/engine split (v2):
  ranks 0..k32-1 : A-pass fp16 (banded), W evac'd to fp16 by a plain DVE
                   copy, B-pass fp16 matmuls over image-pair units.
  ranks k32..    : A-pass fp8 DoubleRow, W evac = PLAIN Pool copy to fp8
                   (the old s_e scale is folded into nt8), B-pass fp8 DR.
PSUM evacuation engines: ACT = lrelu + tmpT + out; DVE = W32; Pool
(gpsimd, which CAN read PSUM on TRN2) = W8 + act8.

Device dataflow per image (out[m,n] = sum_k lhsT[k,m] rhs[k,n]):
  S1a: tmpT[c,Y]    = sum_y x[y,c] UyT[y,Y]
  S1b: actT[X,Y]    = sum_c UxT[c,X] tmpT[c,Y]   (+ lrelu on evac)
  A:   W[Y,(r,j)]   = sum_X actT[X,Y] NT[X,(r,j)]
  B:   out[i,(m,j)] = sum_{r,Yc} MT[Yc,(r,i)] W[Yc,(r,m,j)]   (PSUM accum)
"""
import contextlib
import os

import numpy as np

import concourse.bass as bass
import concourse.mybir as mybir
import concourse.tile as tile
from concourse import bacc
from concourse.bass_utils import run_bass_kernel_spmd

H = 128
H2 = 256
KF = 31
LRELU_SLOPE = 0.01
RANK_ENV = os.environ.get("AFA_RANK")
N_CORES = 8
N_IMG = 64                      # images per core (512 / 8)
GROUP = int(os.environ.get("AFA_G", "4"))
K32_ENV = os.environ.get("AFA_K32")


# ---------------- host-side constants ----------------

def _ac_matrix(out_n, in_n):
    scale = (in_n - 1) / (out_n - 1)
    c = np.arange(out_n, dtype=np.float64) * scale
    i0 = np.clip(np.floor(c).astype(np.int64), 0, in_n - 2)
    w = c - i0
    M = np.zeros((out_n, in_n), dtype=np.float64)
    M[np.arange(out_n), i0] = 1.0 - w
    M[np.arange(out_n), i0 + 1] = w
    return M


def _toeplitz_same(h, n):
    T = np.zeros((n, n), dtype=np.float64)
    for u in range(len(h)):
        d = u - len(h) // 2
        if d >= 0:
            idx = np.arange(0, n - d)
        else:
            idx = np.arange(-d, n)
        T[idx, idx + d] += h[u]
    return T


def _shift_mat(n, d):
    S = np.zeros((n, n))
    idx = np.arange(0, n - d) if d >= 0 else np.arange(-d, n)
    S[idx, idx + d] = 1.0
    return S


def _weighted_rank(F, rank):
    """Rank-`rank` approx of F minimizing the end-to-end error under the
    signal model act ~ U x U^T with white x: err = ||B^1/2 (F-Fr) B^1/2||_F
    with B the Gram of the composed per-tap maps Z_u = D S_u U."""
    kf = F.shape[0]
    D = _ac_matrix(H, H2)
    Uu = _ac_matrix(H2, H)
    Zs = [D @ _shift_mat(H2, u - kf // 2) @ Uu for u in range(kf)]
    B = np.zeros((kf, kf))
    for u in range(kf):
        for v in range(u, kf):
            B[u, v] = B[v, u] = np.sum(Zs[u] * Zs[v])
    w, V = np.linalg.eigh(B)
    w = np.maximum(w, 1e-12)
    Bh = (V * np.sqrt(w)) @ V.T
    Bih = (V / np.sqrt(w)) @ V.T
    Gm = Bh @ F @ Bh
    U_, S_, Vt_ = np.linalg.svd(Gm)
    Fr = Bih @ (U_[:, :rank] * S_[:rank]) @ Vt_[:rank] @ Bih
    return Fr


def _sample_act(Uu, n=4):
    rng = np.random.default_rng(1234)
    X = rng.standard_normal((n, H, H))
    A = Uu @ X @ Uu.T
    return np.where(A >= 0, A, LRELU_SLOPE * A)


def _make_consts(filt, rank, k32):
    """Build device constants.

    Ranks [0, k32): nt fp16 in seg layout (columns (j, r_local)-major for the
    banded A-pass 2D APs), mt as RAW fp32 (used as float32r), rank-major.
    Ranks [k32, rank): nt8/mt8 fp8 DoubleRow-interleaved, with per-rank scale
    n_r chosen so that nt8, W8 (= s_a n_r act N^T, evac'd by a PLAIN copy)
    and mt8 (= M / (s_a n_r)) all sit inside the fp8e4m3 normal range.
    s_a is folded into uyt (lrelu is positively homogeneous) and 1/s_a into
    nt/mt-fp32, so every rank's B contribution lands in out PSUM at scale 1.
    """
    F = np.asarray(filt, dtype=np.float64)
    if rank < min(F.shape):
        F = _weighted_rank(F, rank)
    U, S, Vt = np.linalg.svd(F)
    D = _ac_matrix(H, H2)
    Uu = _ac_matrix(H2, H)
    uyt = np.ascontiguousarray(Uu.T)               # [128 y, 256 Y]

    nr8 = rank - k32
    Ms = [D @ _toeplitz_same(U[:, r] * np.sqrt(S[r]), H2) for r in range(rank)]
    Ns = [D @ _toeplitz_same(Vt[r, :] * np.sqrt(S[r]), H2) for r in range(rank)]

    # fp8 scales: act8 = s_a * act (s_a folded into uyt)
    act = _sample_act(Uu)
    actmax = np.abs(act).max() * 1.15
    s_a = 100.0 / actmax

    # fp16/fp32 ranks: nt gets 1/s_a (act arrives pre-scaled by s_a);
    # mt stays exact fp32.
    nt = np.zeros((2, H, k32 * H), dtype=np.float32)
    mt32 = np.zeros((2, H, k32 * H), dtype=np.float16)
    for r in range(k32):
        cols = np.arange(H) * k32 + r              # (j, r_local)-major
        for c in range(2):
            nt[c, :, cols] = (Ns[r][:, c * H:(c + 1) * H] / s_a).astype(np.float32)
            mt32[c, :, r * H:(r + 1) * H] = \
                Ms[r][:, c * H:(c + 1) * H].T.astype(np.float16)

    out = {"uyt": (uyt * s_a).astype(np.float32), "uxt": uyt.astype(np.float32),
           "nt": nt, "mt32": mt32, "k32": k32, "nr8": nr8, "s_a": float(s_a),
           "s_e": 1.0}
    if nr8 == 0:
        return out

    # ---- fp8 tail: per-rank scales (baseline scheme): wg8 = s_e * W8psum
    # (s_e applied for free by the ACT Copy evac), n_r/m_r sqrt-balanced so
    # act8, nt8, wg8, mt8 all sit inside the fp8e4m3 normal range.
    np8 = mybir.dt.np(mybir.dt.float8e4)
    nt8 = np.zeros((H, 2, nr8 * H), dtype=np.float64)
    mt8 = np.zeros((H, 2, nr8 * H), dtype=np.float64)
    wmaxs = [np.abs(act @ Ns[k32 + i].T).max() * 1.3 for i in range(nr8)]
    mmaxs = [np.abs(Ms[k32 + i]).max() for i in range(nr8)]
    nmaxs = [np.abs(Ns[k32 + i]).max() for i in range(nr8)]
    bias = 2.0
    tmp = [np.sqrt(wmaxs[i] * mmaxs[i]) * bias / (s_a * wmaxs[i])
           for i in range(nr8)]           # = n_r * s_e per rank
    s_e = float(np.exp(np.mean(np.log([tmp[i] * nmaxs[i]
                                       for i in range(nr8)]))) / 100.0)
    for i in range(nr8):
        r = k32 + i
        n_r = tmp[i] / s_e
        m_r = 1.0 / (s_a * n_r * s_e)
        for c in range(2):
            # nt8[p, xc, j*nr8+i] = n_r * N_r[j, xc*128+p]
            nt8[:, c, i::nr8] = (n_r * Ns[r][:, c * H:(c + 1) * H]).T
            # mt8[p, yc, i*H+k] = m_r * M_r[k, yc*128+p]
            mt8[:, c, i * H:(i + 1) * H] = (m_r * Ms[r][:, c * H:(c + 1) * H]).T
    out["s_e"] = s_e
    out["nt8"] = nt8.astype(np8).reshape(H, 2 * nr8 * H)
    out["mt8"] = mt8.astype(np8).reshape(H, 2 * nr8 * H)
    return out


# ---------------- device program ----------------

def _build_tile_program(tc, outs, ins, *, n_img, k32, nr8, group,
                        s_e=1.0, loop_reps=1):
    nc = tc.nc
    if nr8:
        x_d, uyt_d, uxt_d, nt_d, mt32_d, nt8_d, mt8_d = ins
    else:
        x_d, uyt_d, uxt_d, nt_d, mt32_d = ins
    out_d = outs[0]
    RC = k32 * H
    G = group
    GW = G * H
    assert G in (2, 4)
    assert n_img % G == 0
    f16 = mybir.dt.float16
    f32 = mybir.dt.float32
    f32r = mybir.dt.float32r
    fp8 = mybir.dt.float8e4
    DR = mybir.MatmulPerfMode.DoubleRow

    ctx = contextlib.ExitStack()
    with ctx:
        const_pool = ctx.enter_context(tc.tile_pool(name="consts", bufs=1))
        x_pool = ctx.enter_context(tc.tile_pool(name="x", bufs=2))
        tmp_pool = ctx.enter_context(tc.tile_pool(
            name="tmp", bufs=int(os.environ.get("AFA_TMPB", "3"))))
        act_pool = ctx.enter_context(tc.tile_pool(
            name="act", bufs=int(os.environ.get("AFA_ACTB", "8"))))
        w_pool = ctx.enter_context(tc.tile_pool(
            name="w", bufs=int(os.environ.get("AFA_WB", "2"))))
        osb_pool = ctx.enter_context(tc.tile_pool(name="osb", bufs=2))
        # PSUM banks: ps_s 3 + ps_w 2x2 + ps_out 1 = 8
        ps_s = ctx.enter_context(tc.tile_pool(
            name="ps_s", bufs=int(os.environ.get("AFA_PSS", "1")), space="PSUM"))
        ps_w = ctx.enter_context(tc.tile_pool(
            name="ps_w", bufs=int(os.environ.get("AFA_PSW", "3")), space="PSUM"))
        ps_out = ctx.enter_context(tc.tile_pool(
            name="ps_out", bufs=int(os.environ.get("AFA_PSO", "1")),
            space="PSUM"))

        uyt_sb = const_pool.tile([H, H2], f16, tag="uyt")
        nc.sync.dma_start(uyt_sb[:], uyt_d[:])
        uxt_sb = const_pool.tile([H, H2], f16, tag="uxt")
        nc.sync.dma_start(uxt_sb[:], uxt_d[:])
        nt_sb = []
        mt_sb = []
        for c in range(2):
            t = const_pool.tile([H, RC], f16, tag=f"nt{c}", name=f"nt{c}_sb")
            nc.sync.dma_start(t[:], nt_d[c])
            nt_sb.append(t)
            t = const_pool.tile([H, RC], f16, tag=f"mt{c}", name=f"mt{c}_sb")
            nc.sync.dma_start(t[:], mt32_d[c])
            mt_sb.append(t)
        if nr8:
            nt8_sb = const_pool.tile([H, 2 * nr8 * H], fp8, tag="nt8")
            nc.sync.dma_start(nt8_sb[:], nt8_d[:])
            mt8_sb = const_pool.tile([H, 2 * nr8 * H], fp8, tag="mt8")
            nc.sync.dma_start(mt8_sb[:], mt8_d[:])

        def _fetch_x(g):
            x_sb = x_pool.tile([H, GW], f16, tag="x", name=f"x_{g}")
            xg = x_d[g * G:(g + 1) * G].rearrange("g h w -> h g w")
            nc.sync.dma_start(x_sb[:].rearrange("h (g w) -> h g w", g=G), xg)
            return x_sb

        def _emit_s1a(g, half, x_sb, st):
            """S1a matmuls for an image pair + tmpT evac (DVE)."""
            tmpT_ps = ps_s.tile([H, 2 * H2], f32, tag="sp",
                                name=f"tp_{g}_{half}")
            for u in range(2):
                m = half * 2 + u
                nc.tensor.matmul(tmpT_ps[:, u * H2:(u + 1) * H2],
                                 x_sb[:, m * H:(m + 1) * H], uyt_sb[:],
                                 start=True, stop=True)
            t_sb = tmp_pool.tile([H, 2 * H2], f16, tag="tmpT")
            nc.vector.tensor_copy(t_sb[:], tmpT_ps[:])
            st["t_sb"] = t_sb

        def _emit_s1b(g, m, st):
            """S1b matmuls + lrelu (ACT) + act8 (Pool) for one image."""
            act_ps = ps_s.tile([H, 2 * H2], f32, tag="sp", name=f"ap_{g}_{m}")
            tw = st["t_sb"][:, (m % 2) * H2:(m % 2 + 1) * H2]
            for xc in range(2):
                nc.tensor.matmul(act_ps[:, xc * H2:(xc + 1) * H2],
                                 uxt_sb[:, xc * H:(xc + 1) * H], tw,
                                 start=True, stop=True)
            act_sb = act_pool.tile([H, 2 * H2], f16, tag="act")
            nc.scalar.activation(act_sb[:], act_ps[:],
                                 mybir.ActivationFunctionType.Lrelu,
                                 alpha=LRELU_SLOPE)
            st["act"].append(act_sb)
            if nr8:
                act8_sb = act_pool.tile([H, 2 * H2], fp8, tag="act8")
                nc.gpsimd.tensor_copy(act8_sb[:], act_sb[:])
                st["act8"].append(act8_sb)

        def _emit_group(g, pending_b, acts_cur, x_next):
            wg_sb = w_pool.tile([H, 2 * k32 * GW], f16, tag="wg",
                                name=f"wg_{g}")
            wgv = wg_sb[:].rearrange("p (c r g w) -> p c r g w", c=2, r=k32, g=G)
            wg8_sb = None
            if nr8:
                wg8_sb = w_pool.tile([H, 2 * nr8 * GW], fp8, tag="wg8",
                                     name=f"wg8_{g}")
            act_sbs = acts_cur["act"]
            act8_sbs = acts_cur["act8"]
            nxt = {"act": [], "act8": []} if x_next is not None else None

            # pass A for image m of this group; W evacs right after the
            # producing matmuls so ps_w recycles fast.
            # fp16 part: nt/W_ps columns are (j, r_local)-major, so the
            # Toeplitz j-band of each X-chunk is a CONTIGUOUS column window:
            # X-chunk0 only reaches j<=71, chunk1 only j>=56; j in [56,72)
            # accumulates, the rest first-write.
            # fp8 part: one DoubleRow matmul per yc contracts both X-chunks.
            def _emit_a_split(m):
                act_sb = act_sbs[m]
                jwin = ((0, 72), (56, H))
                dst8v = wg8_sb[:].rearrange(
                    "p (c r g w) -> p c r g w", c=2, r=nr8, g=G)
                act8 = act8_sbs[m][:].rearrange("p (x y) -> p x y", x=2)
                nt8v = nt8_sb[:].rearrange("p (x c) -> p x c", x=2)
                for yc in range(2):
                    w_ps8 = ps_w.tile([H, 512], f32, tag="wps",
                                      name=f"wps8_{g}_{m}_{yc}")
                    nc.tensor.matmul(
                        w_ps8[:], act8[:, :, yc * H:(yc + 1) * H], nt8v,
                        start=True, stop=True, perf_mode=DR)
                    src8 = w_ps8[:].rearrange("p (j r) -> p r j", r=nr8)
                    if yc == 0:
                        nc.scalar.activation(dst8v[:, yc, :, m], src8,
                                             mybir.ActivationFunctionType.Copy,
                                             scale=s_e)
                    else:
                        nc.vector.tensor_scalar_mul(dst8v[:, yc, :, m],
                                                    src8, s_e)
                for yc in range(2):
                    w_ps = ps_w.tile([H, 512], f32, tag="wps",
                                     name=f"wps_{g}_{m}_{yc}")
                    for xc in range(2):
                        j0, j1 = jwin[xc]
                        nc.tensor.matmul(
                            w_ps[:, j0 * k32:j1 * k32],
                            act_sb[:, xc * H2 + yc * H: xc * H2 + (yc + 1) * H],
                            nt_sb[xc][:, j0 * k32:j1 * k32],
                            start=(xc == 0), stop=(xc == 1),
                            skip_group_check=True)
                    src = w_ps[:, :k32 * H].rearrange("p (j r) -> p r j", r=k32)
                    if yc == 0:
                        nc.vector.tensor_copy(wgv[:, yc, :, m], src)
                    else:
                        nc.scalar.activation(wgv[:, yc, :, m], src,
                                             mybir.ActivationFunctionType.Copy)

            def _emit_a(m):
                if os.environ.get("AFA_WSPLIT", "0") == "1":
                    _emit_a_split(m)
                    return
                act_sb = act_sbs[m]
                if nr8:
                    act8 = act8_sbs[m][:].rearrange("p (x y) -> p x y", x=2)
                    nt8v = nt8_sb[:].rearrange("p (x c) -> p x c", x=2)
                    w_ps8 = ps_w.tile([H, 1024], f32, tag="wps",
                                      name=f"wps8_{g}_{m}")
                    for yc in range(2):
                        nc.tensor.matmul(
                            w_ps8[:, yc * 512:(yc + 1) * 512],
                            act8[:, :, yc * H:(yc + 1) * H],
                            nt8v,
                            start=True, stop=True, perf_mode=DR)
                    # W8 evac: PLAIN fp8 copy on ACT (scales folded into nt8)
                    dst8v = wg8_sb[:].rearrange(
                        "p (c r g w) -> p c r g w", c=2, r=nr8, g=G)
                    if nr8 == 4:
                        src8 = w_ps8[:].rearrange(
                            "p (c j r) -> p c r j", c=2, r=nr8)
                        nc.scalar.activation(dst8v[:, :, :, m], src8,
                                             mybir.ActivationFunctionType.Copy,
                                             scale=s_e)
                    else:
                        for yc in range(2):
                            src8 = w_ps8[:, yc * 512:yc * 512 + nr8 * H] \
                                .rearrange("p (j r) -> p r j", r=nr8)
                            nc.scalar.activation(
                                dst8v[:, yc, :, m], src8,
                                mybir.ActivationFunctionType.Copy, scale=s_e)
                w_ps = ps_w.tile([H, 1024], f32, tag="wps",
                                 name=f"wps_{g}_{m}")
                jwin = ((0, 72), (56, H))
                for yc in range(2):
                    for xc in range(2):
                        j0, j1 = jwin[xc]
                        nc.tensor.matmul(
                            w_ps[:, yc * 512 + j0 * k32:yc * 512 + j1 * k32],
                            act_sb[:, xc * H2 + yc * H: xc * H2 + (yc + 1) * H],
                            nt_sb[xc][:, j0 * k32:j1 * k32],
                            start=(xc == 0), stop=(xc == 1),
                            skip_group_check=True)
                # W32 evac: RAW fp32 copy on DVE (consumed as float32r)
                if k32 == 4:
                    src = w_ps[:].rearrange("p (c j r) -> p c r j", c=2, r=k32)
                    nc.vector.tensor_copy(wgv[:, :, :, m], src)
                else:
                    for yc in range(2):
                        src = w_ps[:, yc * 512:yc * 512 + k32 * H].rearrange(
                            "p (j r) -> p r j", r=k32)
                        nc.vector.tensor_copy(wgv[:, yc, :, m], src)

            # pass B over image PAIRS: each unit is a 256-col matmul touching
            # only images (p*2, p*2+1), so chunk ci only needs W of pair<=ci
            # from the previous group -> full-slot slack at pbd=0.
            state = {"nmm": [0, 0], "out_ps": None}
            if os.environ.get("AFA_BGRP", "0") == "1":
                npair = 1
                units = [(kind, 0, yc, r) for kind, yc, r in
                         ([("32", yc, r) for yc in range(2) for r in range(k32)]
                          + [("8", 0, rl) for rl in range(nr8)])]
            else:
                npair = G // 2
                units = [(kind, p, yc, r)
                         for p in range(npair)
                         for kind, yc, r in
                         ([("32", yc, r) for yc in range(2) for r in range(k32)]
                          + [("8", 0, rl) for rl in range(nr8)])]
            nu_pair = len(units) // npair

            def _pass_b(ci, chunks=G):
                if state["out_ps"] is None:
                    state["out_ps"] = ps_out.tile([H, GW], f32, tag="ops",
                                                  name=f"ops_{g}")
                out_ps = state["out_ps"]
                n0 = (ci * len(units)) // chunks
                n1 = ((ci + 1) * len(units)) // chunks
                pw = GW // npair
                for kind, p, yc, r in units[n0:n1]:
                    state["nmm"][p] += 1
                    ow = out_ps[:, p * pw:(p + 1) * pw]
                    first = state["nmm"][p] == 1
                    last = state["nmm"][p] == nu_pair
                    gpp = G // npair
                    if kind == "32":
                        nc.tensor.matmul(
                            ow,
                            mt_sb[yc][:, r * H:(r + 1) * H],
                            wgv[:, yc, r, p * gpp:(p + 1) * gpp],
                            start=first, stop=last, skip_group_check=True)
                    else:
                        mt8v = mt8_sb[:].rearrange("p (c ri) -> p c ri", c=2)
                        wg8v = wg8_sb[:].rearrange(
                            "p (c r g w) -> p c r g w", c=2, r=nr8, g=G)
                        nc.tensor.matmul(
                            ow,
                            mt8v[:, :, r * H:(r + 1) * H],
                            wg8v[:, :, r, p * gpp:(p + 1) * gpp],
                            start=first, stop=last,
                            perf_mode=DR, skip_group_check=True)
                if ci == chunks - 1:
                    og = out_d[g * G:(g + 1) * G].rearrange("g h w -> h g w")
                    out_sb = osb_pool.tile([H, GW], f32, tag="osb")
                    oe = os.environ.get("AFA_OE", "v")
                    if os.environ.get("AFA_OSPLIT", "0") == "1":
                        for p in range(G // 2):
                            nc.vector.tensor_copy(
                                out_sb[:, p * 2 * H:(p + 1) * 2 * H],
                                out_ps[:, p * 2 * H:(p + 1) * 2 * H])
                    elif oe == "s":
                        nc.scalar.activation(out_sb[:], out_ps[:],
                                             mybir.ActivationFunctionType.Copy)
                    else:
                        nc.vector.tensor_copy(out_sb[:], out_ps[:])
                    nc.sync.dma_start(
                        og, out_sb[:].rearrange("h (g w) -> h g w", g=G))

            # slot loop: A+evacs, then B(g-1) chunk, then next-group S1 piece
            s1pos = os.environ.get("AFA_S1POS", "late")
            bch = int(os.environ.get("AFA_BCH", str(G)))   # B chunk count
            bst = int(os.environ.get("AFA_BST", "0"))      # B start slot
            for m in range(G):
                if s1pos == "early" and nxt is not None:
                    if m % 2 == 0:
                        _emit_s1a(g + 1, m // 2, x_next, nxt)
                    _emit_s1b(g + 1, m, nxt)
                _emit_a(m)
                if pending_b is not None and bst <= m < bst + bch:
                    pending_b(m - bst, chunks=bch)
                if s1pos != "early" and nxt is not None:
                    if m % 2 == 0:
                        _emit_s1a(g + 1, m // 2, x_next, nxt)
                    _emit_s1b(g + 1, m, nxt)

            return _pass_b, nxt

        def _emit_all_groups():
            ngroups = n_img // G
            pending = None
            x_next = _fetch_x(0)
            # prologue: S1 for group 0
            acts_next = {"act": [], "act8": []}
            for half in range(G // 2):
                _emit_s1a(0, half, x_next, acts_next)
                _emit_s1b(0, half * 2, acts_next)
                _emit_s1b(0, half * 2 + 1, acts_next)
            for g in range(ngroups):
                x_cur = x_next
                x_next = _fetch_x(g + 1) if g + 1 < ngroups else None
                acts_cur = acts_next
                pending, acts_next = _emit_group(g, pending, acts_cur, x_next)
            if pending is not None:
                bch = int(os.environ.get("AFA_BCH", str(G)))
                for ci in range(bch):
                    pending(ci, chunks=bch)

        if loop_reps > 1:
            with tc.For_i(0, loop_reps, 1):
                _emit_all_groups()
        else:
            _emit_all_groups()


_NC_CACHE = {}


def _build_nc(n_img=N_IMG, k32=4, nr8=4, group=None, s_e=1.0, loop_reps=1):
    if group is None:
        group = GROUP
    key = (n_img, k32, nr8, group, loop_reps, round(s_e, 12),
           os.environ.get("AFA_PSO", ""), os.environ.get("AFA_S1POS", ""),
           os.environ.get("AFA_BCH", ""), os.environ.get("AFA_BST", ""),
           os.environ.get("AFA_OSPLIT", ""), os.environ.get("AFA_G", ""),
           os.environ.get("AFA_WSPLIT", ""), os.environ.get("AFA_OE", ""),
           os.environ.get("AFA_BGRP", ""),
           os.environ.get("AFA_PSW", ""), os.environ.get("AFA_PSS", ""),
           os.environ.get("AFA_PBD", ""), os.environ.get("AFA_TMPB", ""),
           os.environ.get("AFA_ACTB", ""), os.environ.get("AFA_WB", ""))
    if key in _NC_CACHE:
        return _NC_CACHE[key]
    nc = bacc.Bacc("TRN2", target_bir_lowering=False, debug=False)
    f16 = mybir.dt.float16
    f32 = mybir.dt.float32
    f32r = mybir.dt.float32r
    fp8 = mybir.dt.float8e4
    RC = k32 * H
    x_d = nc.dram_tensor("x", [n_img, H, H], f16, kind="ExternalInput").ap()
    uyt_d = nc.dram_tensor("uyt", [H, H2], f16, kind="ExternalInput").ap()
    uxt_d = nc.dram_tensor("uxt", [H, H2], f16, kind="ExternalInput").ap()
    nt_d = nc.dram_tensor("nt", [2, H, RC], f16, kind="ExternalInput").ap()
    mt32_d = nc.dram_tensor("mt32", [2, H, RC], f16, kind="ExternalInput").ap()
    ins = [x_d, uyt_d, uxt_d, nt_d, mt32_d]
    if nr8:
        nt8_d = nc.dram_tensor("nt8", [H, 2 * nr8 * H], fp8,
                               kind="ExternalInput").ap()
        mt8_d = nc.dram_tensor("mt8", [H, 2 * nr8 * H], fp8,
                               kind="ExternalInput").ap()
        ins += [nt8_d, mt8_d]
    out_d = nc.dram_tensor("out", [n_img, H, H], f32, kind="ExternalOutput").ap()
    with tile.TileContext(nc) as tc:
        _build_tile_program(tc, [out_d], ins, n_img=n_img, k32=k32, nr8=nr8,
                            group=group, s_e=s_e, loop_reps=loop_reps)
    nc.compile()
    _NC_CACHE[key] = nc
    return nc


def _pick_rank(filt):
    """Smallest rank whose weighted-truncation error estimate fits the
    error budget (harness gate 2e-2; leave room for fp16/fp8 quantization).
    For the reference's sinc filter this lands on 8."""
    if RANK_ENV:
        return int(RANK_ENV)
    F = np.asarray(filt, np.float64)
    kf = F.shape[0]
    D = _ac_matrix(H, H2)
    Uu = _ac_matrix(H2, H)
    Zs = [D @ _shift_mat(H2, u - kf // 2) @ Uu for u in range(kf)]
    B = np.zeros((kf, kf))
    for u in range(kf):
        for v in range(u, kf):
            B[u, v] = B[v, u] = np.sum(Zs[u] * Zs[v])
    w, V = np.linalg.eigh(B)
    Bh = (V * np.sqrt(np.maximum(w, 1e-12))) @ V.T
    s = np.linalg.svd(Bh @ F @ Bh, compute_uv=False)
    nrm = np.sqrt(np.sum(s * s))
    for r in range(4, 16):
        if r >= len(s) or np.sqrt(np.sum(s[r:] ** 2)) <= 4e-3 * nrm:
            return r
    return 16


def _split_rank(rank):
    k32 = int(K32_ENV) if K32_ENV else max(rank - 4, rank // 2)
    k32 = min(k32, rank)
    return k32, rank - k32


def _make_in_maps(x, filt, rank, consts=None):
    k32, nr8 = _split_rank(rank)
    if consts is None:
        consts = _make_consts(filt, rank, k32)
    f16 = np.float16
    imgs = x.reshape(N_CORES, N_IMG, H, H)
    base = {
        "uyt": consts["uyt"].astype(f16), "uxt": consts["uxt"].astype(f16),
        "nt": consts["nt"].astype(f16), "mt32": consts["mt32"],
    }
    if consts["nr8"]:
        base["nt8"] = consts["nt8"]
        base["mt8"] = consts["mt8"]
    return [{"x": np.ascontiguousarray(imgs[core]).astype(f16), **base}
            for core in range(N_CORES)]


_RUNNER_CACHE = {}


def _get_runner(nc):
    """Persistent jitted 8-core runner (mirrors bass2jax.run_bass_via_pjrt's
    multi-core path) so repeated kernel() calls reuse one compiled executable."""
    if id(nc) in _RUNNER_CACHE:
        return _RUNNER_CACHE[id(nc)]
    import jax
    from jax.sharding import Mesh, PartitionSpec
    from jax.experimental.shard_map import shard_map
    from concourse.bass2jax import (_bass_exec_p, install_neuronx_cc_hook,
                                    partition_id_tensor)
    install_neuronx_cc_hook()
    in_names, out_names, out_avals, zero_outs = [], [], [], []
    for alloc in nc.m.functions[0].allocations:
        if not isinstance(alloc, mybir.MemoryLocationSet):
            continue
        name = alloc.memorylocations[0].name
        if alloc.kind == "ExternalInput":
            if nc.partition_id_tensor is not None and name == nc.partition_id_tensor.name:
                continue
            in_names.append(name)
        elif alloc.kind == "ExternalOutput":
            out_names.append(name)
            shape = tuple(alloc.tensor_shape)
            dtype = mybir.dt.np(alloc.dtype)
            out_avals.append(jax.core.ShapedArray(shape, dtype))
            zero_outs.append(np.zeros(shape, dtype))
    n_params = len(in_names)
    all_in_names = in_names + out_names
    if nc.partition_id_tensor is not None:
        all_in_names = all_in_names + [nc.partition_id_tensor.name]

    def _body(*args):
        operands = list(args)
        if nc.partition_id_tensor is not None:
            operands.append(partition_id_tensor())
        return tuple(_bass_exec_p.bind(
            *operands,
            out_avals=tuple(out_avals),
            in_names=tuple(all_in_names),
            out_names=tuple(out_names),
            lowering_input_output_aliases=(),
            sim_require_finite=True,
            sim_require_nnan=True,
            nc=nc,
        ))

    donate = tuple(range(n_params, n_params + len(out_names)))
    devices = jax.devices()[:N_CORES]
    mesh = Mesh(np.asarray(devices), ("core",))
    in_specs = (PartitionSpec("core"),) * (n_params + len(out_names))
    out_specs = (PartitionSpec("core"),) * len(out_names)
    sharded = jax.jit(
        shard_map(_body, mesh=mesh, in_specs=in_specs, out_specs=out_specs,
                  check_rep=False),
        donate_argnums=donate, keep_unused=True)
    runner = (sharded, in_names, out_names, out_avals, zero_outs)
    _RUNNER_CACHE[id(nc)] = runner
    return runner


def run(x, filt):
    """Run on 8 cores. Returns out [B,C,H,W] f32."""
    x = np.ascontiguousarray(np.asarray(x, dtype=np.float32))
    filt = np.asarray(filt, dtype=np.float32)
    B, C, Hh, Ww = x.shape
    assert (Hh, Ww) == (H, H) and B * C == N_CORES * N_IMG
    rank = _pick_rank(filt)
    k32, nr8 = _split_rank(rank)
    consts = _make_consts(filt, rank, k32)
    in_maps = _make_in_maps(x, filt, rank, consts)
    nc = _build_nc(k32=k32, nr8=nr8, s_e=consts["s_e"])
    try:
        sharded, in_names, out_names, out_avals, zero_outs = _get_runner(nc)
        concat_in = [np.concatenate([in_maps[c][nm] for c in range(N_CORES)], axis=0)
                     for nm in in_names]
        concat_zero = [np.zeros((N_CORES * z.shape[0], *z.shape[1:]), z.dtype)
                       for z in zero_outs]
        outs = sharded(*concat_in, *concat_zero)
        oi = out_names.index("out")
        out = np.asarray(outs[oi]).reshape(N_CORES, *out_avals[oi].shape)
    except Exception:
        res = run_bass_kernel_spmd(nc, in_maps, core_ids=list(range(N_CORES)))
        out = np.stack([res.results[c]["out"] for c in range(N_CORES)])
    return out.reshape(B, C, H, H).astype(np.float32, copy=False)


def kernel(x, filt):
    return run(x, filt)


# revision 24
# speedup vs baseline: 1.7461x; 1.0229x over previous
"""AliasFreeActivation (upsample2x -> leaky_relu -> 31x31 depthwise sinc conv
-> downsample2x) as a Trainium2 Bass/Tile kernel, data-parallel over 8 cores.

Math (per [128,128] image; B*C = 512 images, 64 per core):
  out = Dy @ Conv_F(lrelu(Uy @ x @ Ux^T)) @ Dx^T
With F = sum_r g_r h_r^T (weighted SVD of the 31x31 filter, rank 8):
  out = sum_r M_r @ act @ N_r^T
  M_r = Dy @ Toeplitz(g_r) [128,256],  N_r = Dx @ Toeplitz(h_r) [128,256]
  act = lrelu(Uy @ x @ Ux^T) [256,256]

Precision/engine split (v2):
  ranks 0..k32-1 : A-pass fp16 (banded), W evac'd to fp16 by a plain DVE
                   copy, B-pass fp16 matmuls over image-pair units.
  ranks k32..    : A-pass fp8 DoubleRow, W evac = PLAIN Pool copy to fp8
                   (the old s_e scale is folded into nt8), B-pass fp8 DR.
PSUM evacuation engines: ACT = lrelu + tmpT + out; DVE = W32; Pool
(gpsimd, which CAN read PSUM on TRN2) = W8 + act8.

Device dataflow per image (out[m,n] = sum_k lhsT[k,m] rhs[k,n]):
  S1a: tmpT[c,Y]    = sum_y x[y,c] UyT[y,Y]
  S1b: actT[X,Y]    = sum_c UxT[c,X] tmpT[c,Y]   (+ lrelu on evac)
  A:   W[Y,(r,j)]   = sum_X actT[X,Y] NT[X,(r,j)]
  B:   out[i,(m,j)] = sum_{r,Yc} MT[Yc,(r,i)] W[Yc,(r,m,j)]   (PSUM accum)
"""
import contextlib
import os

import numpy as np

import concourse.bass as bass
import concourse.mybir as mybir
import concourse.tile as tile
from concourse import bacc
from concourse.bass_utils import run_bass_kernel_spmd

H = 128
H2 = 256
KF = 31
LRELU_SLOPE = 0.01
RANK_ENV = os.environ.get("AFA_RANK")
N_CORES = 8
N_IMG = 64                      # images per core (512 / 8)
GROUP = int(os.environ.get("AFA_G", "4"))
K32_ENV = os.environ.get("AFA_K32")


# ---------------- host-side constants ----------------

def _ac_matrix(out_n, in_n):
    scale = (in_n - 1) / (out_n - 1)
    c = np.arange(out_n, dtype=np.float64) * scale
    i0 = np.clip(np.floor(c).astype(np.int64), 0, in_n - 2)
    w = c - i0
    M = np.zeros((out_n, in_n), dtype=np.float64)
    M[np.arange(out_n), i0] = 1.0 - w
    M[np.arange(out_n), i0 + 1] = w
    return M


def _toeplitz_same(h, n):
    T = np.zeros((n, n), dtype=np.float64)
    for u in range(len(h)):
        d = u - len(h) // 2
        if d >= 0:
            idx = np.arange(0, n - d)
        else:
            idx = np.arange(-d, n)
        T[idx, idx + d] += h[u]
    return T


def _shift_mat(n, d):
    S = np.zeros((n, n))
    idx = np.arange(0, n - d) if d >= 0 else np.arange(-d, n)
    S[idx, idx + d] = 1.0
    return S


def _weighted_rank(F, rank):
    """Rank-`rank` approx of F minimizing the end-to-end error under the
    signal model act ~ U x U^T with white x: err = ||B^1/2 (F-Fr) B^1/2||_F
    with B the Gram of the composed per-tap maps Z_u = D S_u U."""
    kf = F.shape[0]
    D = _ac_matrix(H, H2)
    Uu = _ac_matrix(H2, H)
    Zs = [D @ _shift_mat(H2, u - kf // 2) @ Uu for u in range(kf)]
    B = np.zeros((kf, kf))
    for u in range(kf):
        for v in range(u, kf):
            B[u, v] = B[v, u] = np.sum(Zs[u] * Zs[v])
    w, V = np.linalg.eigh(B)
    w = np.maximum(w, 1e-12)
    Bh = (V * np.sqrt(w)) @ V.T
    Bih = (V / np.sqrt(w)) @ V.T
    Gm = Bh @ F @ Bh
    U_, S_, Vt_ = np.linalg.svd(Gm)
    Fr = Bih @ (U_[:, :rank] * S_[:rank]) @ Vt_[:rank] @ Bih
    return Fr


def _sample_act(Uu, n=4):
    rng = np.random.default_rng(1234)
    X = rng.standard_normal((n, H, H))
    A = Uu @ X @ Uu.T
    return np.where(A >= 0, A, LRELU_SLOPE * A)


def _make_consts(filt, rank, k32):
    """Build device constants.

    Ranks [0, k32): nt fp16 in seg layout (columns (j, r_local)-major for the
    banded A-pass 2D APs), mt as RAW fp32 (used as float32r), rank-major.
    Ranks [k32, rank): nt8/mt8 fp8 DoubleRow-interleaved, with per-rank scale
    n_r chosen so that nt8, W8 (= s_a n_r act N^T, evac'd by a PLAIN copy)
    and mt8 (= M / (s_a n_r)) all sit inside the fp8e4m3 normal range.
    s_a is folded into uyt (lrelu is positively homogeneous) and 1/s_a into
    nt/mt-fp32, so every rank's B contribution lands in out PSUM at scale 1.
    """
    F = np.asarray(filt, dtype=np.float64)
    if rank < min(F.shape):
        F = _weighted_rank(F, rank)
    U, S, Vt = np.linalg.svd(F)
    D = _ac_matrix(H, H2)
    Uu = _ac_matrix(H2, H)
    uyt = np.ascontiguousarray(Uu.T)               # [128 y, 256 Y]

    nr8 = rank - k32
    Ms = [D @ _toeplitz_same(U[:, r] * np.sqrt(S[r]), H2) for r in range(rank)]
    Ns = [D @ _toeplitz_same(Vt[r, :] * np.sqrt(S[r]), H2) for r in range(rank)]

    # fp8 scales: act8 = s_a * act (s_a folded into uyt)
    act = _sample_act(Uu)
    actmax = np.abs(act).max() * 1.15
    s_a = 100.0 / actmax

    # fp16/fp32 ranks: nt gets 1/s_a (act arrives pre-scaled by s_a);
    # mt stays exact fp32.
    nt = np.zeros((2, H, k32 * H), dtype=np.float32)
    mt32 = np.zeros((2, H, k32 * H), dtype=np.float16)
    for r in range(k32):
        cols = np.arange(H) * k32 + r              # (j, r_local)-major
        for c in range(2):
            nt[c, :, cols] = (Ns[r][:, c * H:(c + 1) * H] / s_a).astype(np.float32)
            mt32[c, :, r * H:(r + 1) * H] = \
                Ms[r][:, c * H:(c + 1) * H].T.astype(np.float16)

    out = {"uyt": (uyt * s_a).astype(np.float32), "uxt": uyt.astype(np.float32),
           "nt": nt, "mt32": mt32, "k32": k32, "nr8": nr8, "s_a": float(s_a),
           "s_e": 1.0}
    if nr8 == 0:
        return out

    # ---- fp8 tail: per-rank scales (baseline scheme): wg8 = s_e * W8psum
    # (s_e applied for free by the ACT Copy evac), n_r/m_r sqrt-balanced so
    # act8, nt8, wg8, mt8 all sit inside the fp8e4m3 normal range.
    np8 = mybir.dt.np(mybir.dt.float8e4)
    nt8 = np.zeros((H, 2, nr8 * H), dtype=np.float64)
    mt8 = np.zeros((H, 2, nr8 * H), dtype=np.float64)
    wmaxs = [np.abs(act @ Ns[k32 + i].T).max() * 1.3 for i in range(nr8)]
    mmaxs = [np.abs(Ms[k32 + i]).max() for i in range(nr8)]
    nmaxs = [np.abs(Ns[k32 + i]).max() for i in range(nr8)]
    bias = 2.0
    tmp = [np.sqrt(wmaxs[i] * mmaxs[i]) * bias / (s_a * wmaxs[i])
           for i in range(nr8)]           # = n_r * s_e per rank
    s_e = float(np.exp(np.mean(np.log([tmp[i] * nmaxs[i]
                                       for i in range(nr8)]))) / 100.0)
    for i in range(nr8):
        r = k32 + i
        n_r = tmp[i] / s_e
        m_r = 1.0 / (s_a * n_r * s_e)
        for c in range(2):
            # nt8[p, xc, j*nr8+i] = n_r * N_r[j, xc*128+p]
            nt8[:, c, i::nr8] = (n_r * Ns[r][:, c * H:(c + 1) * H]).T
            # mt8[p, yc, i*H+k] = m_r * M_r[k, yc*128+p]
            mt8[:, c, i * H:(i + 1) * H] = (m_r * Ms[r][:, c * H:(c + 1) * H]).T
    out["s_e"] = s_e
    out["nt8"] = nt8.astype(np8).reshape(H, 2 * nr8 * H)
    out["mt8"] = mt8.astype(np8).reshape(H, 2 * nr8 * H)
    return out


# ---------------- device program ----------------

def _build_tile_program(tc, outs, ins, *, n_img, k32, nr8, group,
                        s_e=1.0, loop_reps=1):
    nc = tc.nc
    if nr8:
        x_d, uyt_d, uxt_d, nt_d, mt32_d, nt8_d, mt8_d = ins
    else:
        x_d, uyt_d, uxt_d, nt_d, mt32_d = ins
    out_d = outs[0]
    RC = k32 * H
    G = group
    GW = G * H
    assert G in (2, 4)
    assert n_img % G == 0
    f16 = mybir.dt.float16
    f32 = mybir.dt.float32
    f32r = mybir.dt.float32r
    fp8 = mybir.dt.float8e4
    DR = mybir.MatmulPerfMode.DoubleRow

    ctx = contextlib.ExitStack()
    with ctx:
        const_pool = ctx.enter_context(tc.tile_pool(name="consts", bufs=1))
        x_pool = ctx.enter_context(tc.tile_pool(name="x", bufs=2))
        tmp_pool = ctx.enter_context(tc.tile_pool(
            name="tmp", bufs=int(os.environ.get("AFA_TMPB", "3"))))
        act_pool = ctx.enter_context(tc.tile_pool(
            name="act", bufs=int(os.environ.get("AFA_ACTB", "8"))))
        w_pool = ctx.enter_context(tc.tile_pool(
            name="w", bufs=int(os.environ.get("AFA_WB", "2"))))
        osb_pool = ctx.enter_context(tc.tile_pool(name="osb", bufs=2))
        # PSUM banks: ps_s 3 + ps_w 2x2 + ps_out 1 = 8
        ps_s = ctx.enter_context(tc.tile_pool(
            name="ps_s", bufs=int(os.environ.get("AFA_PSS", "1")), space="PSUM"))
        w32h = os.environ.get("AFA_W32H", "0") == "1"
        if w32h:
            ps_w = ctx.enter_context(tc.tile_pool(
                name="ps_w", bufs=int(os.environ.get("AFA_PSW", "2")),
                space="PSUM"))
            ps_wh = ctx.enter_context(tc.tile_pool(
                name="ps_wh", bufs=int(os.environ.get("AFA_PSWH", "2")),
                space="PSUM"))
        else:
            ps_w = ctx.enter_context(tc.tile_pool(
                name="ps_w", bufs=int(os.environ.get("AFA_PSW", "3")),
                space="PSUM"))
        ps_out = ctx.enter_context(tc.tile_pool(
            name="ps_out", bufs=int(os.environ.get("AFA_PSO", "1")),
            space="PSUM"))

        uyt_sb = const_pool.tile([H, H2], f16, tag="uyt")
        nc.sync.dma_start(uyt_sb[:], uyt_d[:])
        uxt_sb = const_pool.tile([H, H2], f16, tag="uxt")
        nc.sync.dma_start(uxt_sb[:], uxt_d[:])
        nt_sb = []
        mt_sb = []
        for c in range(2):
            t = const_pool.tile([H, RC], f16, tag=f"nt{c}", name=f"nt{c}_sb")
            nt_sb.append(t)
            t = const_pool.tile([H, RC], f16, tag=f"mt{c}", name=f"mt{c}_sb")
            mt_sb.append(t)
        nt8_sb = mt8_sb = None
        if nr8:
            nt8_sb = const_pool.tile([H, 2 * nr8 * H], fp8, tag="nt8")
            mt8_sb = const_pool.tile([H, 2 * nr8 * H], fp8, tag="mt8")

        def _load_heavy_consts():
            # issued AFTER x(0)/uyt/uxt so the serial HWDGE queue does not
            # gate the S1 prologue; these are only needed by pass A/B.
            for c in range(2):
                nc.sync.dma_start(nt_sb[c][:], nt_d[c])
            if nr8:
                nc.sync.dma_start(nt8_sb[:], nt8_d[:])
            for c in range(2):
                nc.sync.dma_start(mt_sb[c][:], mt32_d[c])
            if nr8:
                nc.sync.dma_start(mt8_sb[:], mt8_d[:])

        def _fetch_x(g):
            x_sb = x_pool.tile([H, GW], f16, tag="x", name=f"x_{g}")
            xg = x_d[g * G:(g + 1) * G].rearrange("g h w -> h g w")
            nc.sync.dma_start(x_sb[:].rearrange("h (g w) -> h g w", g=G), xg)
            return x_sb

        import contextlib as _ctl

        def _s1prio():
            off = os.environ.get("AFA_S1PRIO", "")
            if off == "":
                return _ctl.nullcontext()
            return tc.high_priority(None if off == "0" else int(off))

        def _emit_s1a(g, half, x_sb, st):
            """S1a matmuls for an image pair + tmpT evac (DVE)."""
            with _s1prio():
                tmpT_ps = ps_s.tile([H, 2 * H2], f32, tag="sp",
                                    name=f"tp_{g}_{half}")
                for u in range(2):
                    m = half * 2 + u
                    nc.tensor.matmul(tmpT_ps[:, u * H2:(u + 1) * H2],
                                     x_sb[:, m * H:(m + 1) * H], uyt_sb[:],
                                     start=True, stop=True)
                t_sb = tmp_pool.tile([H, 2 * H2], f16, tag="tmpT")
                nc.vector.tensor_copy(t_sb[:], tmpT_ps[:])
                st["t_sb"] = t_sb

        def _emit_s1b(g, m, st):
            """S1b matmuls + lrelu (ACT) + act8 (Pool) for one image."""
            with _s1prio():
                act_ps = ps_s.tile([H, 2 * H2], f32, tag="sp",
                                   name=f"ap_{g}_{m}")
                tw = st["t_sb"][:, (m % 2) * H2:(m % 2 + 1) * H2]
                for xc in range(2):
                    nc.tensor.matmul(act_ps[:, xc * H2:(xc + 1) * H2],
                                     uxt_sb[:, xc * H:(xc + 1) * H], tw,
                                     start=True, stop=True)
                act_sb = act_pool.tile([H, 2 * H2], f16, tag="act")
                nc.scalar.activation(act_sb[:], act_ps[:],
                                     mybir.ActivationFunctionType.Lrelu,
                                     alpha=LRELU_SLOPE)
                st["act"].append(act_sb)
                if nr8:
                    act8_sb = act_pool.tile([H, 2 * H2], fp8, tag="act8")
                    nc.gpsimd.tensor_copy(act8_sb[:], act_sb[:])
                    st["act8"].append(act8_sb)

        def _emit_group(g, pending_b, acts_cur, x_next):
            wg_sb = w_pool.tile([H, 2 * k32 * GW], f16, tag="wg",
                                name=f"wg_{g}")
            wgv = wg_sb[:].rearrange("p (c r g w) -> p c r g w", c=2, r=k32, g=G)
            wg8_sb = None
            if nr8:
                wg8_sb = w_pool.tile([H, 2 * nr8 * GW], fp8, tag="wg8",
                                     name=f"wg8_{g}")
            act_sbs = acts_cur["act"]
            act8_sbs = acts_cur["act8"]
            nxt = {"act": [], "act8": []} if x_next is not None else None

            # pass A for image m of this group; W evacs right after the
            # producing matmuls so ps_w recycles fast.
            # fp16 part: nt/W_ps columns are (j, r_local)-major, so the
            # Toeplitz j-band of each X-chunk is a CONTIGUOUS column window:
            # X-chunk0 only reaches j<=71, chunk1 only j>=56; j in [56,72)
            # accumulates, the rest first-write.
            # fp8 part: one DoubleRow matmul per yc contracts both X-chunks.
            def _emit_a_split(m):
                act_sb = act_sbs[m]
                jwin = ((0, 72), (56, H))
                dst8v = wg8_sb[:].rearrange(
                    "p (c r g w) -> p c r g w", c=2, r=nr8, g=G)
                act8 = act8_sbs[m][:].rearrange("p (x y) -> p x y", x=2)
                nt8v = nt8_sb[:].rearrange("p (x c) -> p x c", x=2)
                for yc in range(2):
                    w_ps8 = ps_w.tile([H, 512], f32, tag="wps",
                                      name=f"wps8_{g}_{m}_{yc}")
                    nc.tensor.matmul(
                        w_ps8[:], act8[:, :, yc * H:(yc + 1) * H], nt8v,
                        start=True, stop=True, perf_mode=DR)
                    src8 = w_ps8[:].rearrange("p (j r) -> p r j", r=nr8)
                    if yc == 0:
                        nc.scalar.activation(dst8v[:, yc, :, m], src8,
                                             mybir.ActivationFunctionType.Copy,
                                             scale=s_e)
                    else:
                        nc.vector.tensor_scalar_mul(dst8v[:, yc, :, m],
                                                    src8, s_e)
                for yc in range(2):
                    w_ps = ps_w.tile([H, 512], f32, tag="wps",
                                     name=f"wps_{g}_{m}_{yc}")
                    for xc in range(2):
                        j0, j1 = jwin[xc]
                        nc.tensor.matmul(
                            w_ps[:, j0 * k32:j1 * k32],
                            act_sb[:, xc * H2 + yc * H: xc * H2 + (yc + 1) * H],
                            nt_sb[xc][:, j0 * k32:j1 * k32],
                            start=(xc == 0), stop=(xc == 1),
                            skip_group_check=True)
                    src = w_ps[:, :k32 * H].rearrange("p (j r) -> p r j", r=k32)
                    if yc == 0:
                        nc.vector.tensor_copy(wgv[:, yc, :, m], src)
                    else:
                        nc.scalar.activation(wgv[:, yc, :, m], src,
                                             mybir.ActivationFunctionType.Copy)

            def _emit_a(m):
                if os.environ.get("AFA_WSPLIT", "0") == "1":
                    _emit_a_split(m)
                    return
                act_sb = act_sbs[m]
                if nr8:
                    act8 = act8_sbs[m][:].rearrange("p (x y) -> p x y", x=2)
                    nt8v = nt8_sb[:].rearrange("p (x c) -> p x c", x=2)
                    w_ps8 = ps_w.tile([H, 1024], f32, tag="wps",
                                      name=f"wps8_{g}_{m}")
                    for yc in range(2):
                        nc.tensor.matmul(
                            w_ps8[:, yc * 512:(yc + 1) * 512],
                            act8[:, :, yc * H:(yc + 1) * H],
                            nt8v,
                            start=True, stop=True, perf_mode=DR)
                    # W8 evac: PLAIN fp8 copy on ACT (scales folded into nt8)
                    dst8v = wg8_sb[:].rearrange(
                        "p (c r g w) -> p c r g w", c=2, r=nr8, g=G)
                    if nr8 == 4:
                        src8 = w_ps8[:].rearrange(
                            "p (c j r) -> p c r j", c=2, r=nr8)
                        nc.scalar.activation(dst8v[:, :, :, m], src8,
                                             mybir.ActivationFunctionType.Copy,
                                             scale=s_e)
                    else:
                        for yc in range(2):
                            src8 = w_ps8[:, yc * 512:yc * 512 + nr8 * H] \
                                .rearrange("p (j r) -> p r j", r=nr8)
                            nc.scalar.activation(
                                dst8v[:, yc, :, m], src8,
                                mybir.ActivationFunctionType.Copy, scale=s_e)
                jwin = ((0, 72), (56, H))
                if w32h:
                    for yc in range(2):
                        w_ph = ps_wh.tile([H, 512], f32, tag="wph",
                                          name=f"wph_{g}_{m}_{yc}")
                        for xc in range(2):
                            j0, j1 = jwin[xc]
                            nc.tensor.matmul(
                                w_ph[:, j0 * k32:j1 * k32],
                                act_sb[:, xc * H2 + yc * H:
                                       xc * H2 + (yc + 1) * H],
                                nt_sb[xc][:, j0 * k32:j1 * k32],
                                start=(xc == 0), stop=(xc == 1),
                                skip_group_check=True)
                        srch = w_ph[:, :k32 * H].rearrange(
                            "p (j r) -> p r j", r=k32)
                        nc.vector.tensor_copy(wgv[:, yc, :, m], srch)
                else:
                    w_ps = ps_w.tile([H, 1024], f32, tag="wps",
                                     name=f"wps_{g}_{m}")
                    for yc in range(2):
                        for xc in range(2):
                            j0, j1 = jwin[xc]
                            nc.tensor.matmul(
                                w_ps[:, yc * 512 + j0 * k32:yc * 512 + j1 * k32],
                                act_sb[:, xc * H2 + yc * H: xc * H2 + (yc + 1) * H],
                                nt_sb[xc][:, j0 * k32:j1 * k32],
                                start=(xc == 0), stop=(xc == 1),
                                skip_group_check=True)
                    # W32 evac: RAW fp32 copy on DVE
                    if k32 == 4:
                        src = w_ps[:].rearrange("p (c j r) -> p c r j", c=2, r=k32)
                        nc.vector.tensor_copy(wgv[:, :, :, m], src)
                    else:
                        for yc in range(2):
                            src = w_ps[:, yc * 512:yc * 512 + k32 * H].rearrange(
                                "p (j r) -> p r j", r=k32)
                            nc.vector.tensor_copy(wgv[:, yc, :, m], src)

            # pass B over image PAIRS: each unit is a 256-col matmul touching
            # only images (p*2, p*2+1), so chunk ci only needs W of pair<=ci
            # from the previous group -> full-slot slack at pbd=0.
            state = {"nmm": [0, 0], "out_ps": None}
            if os.environ.get("AFA_BGRP", "0") == "1":
                npair = 1
                units = [(kind, 0, yc, r) for kind, yc, r in
                         ([("32", yc, r) for yc in range(2) for r in range(k32)]
                          + [("8", 0, rl) for rl in range(nr8)])]
            else:
                npair = G // 2
                units = [(kind, p, yc, r)
                         for p in range(npair)
                         for kind, yc, r in
                         ([("32", yc, r) for yc in range(2) for r in range(k32)]
                          + [("8", 0, rl) for rl in range(nr8)])]
            nu_pair = len(units) // npair

            def _pass_b(ci, chunks=G):
                if state["out_ps"] is None:
                    state["out_ps"] = ps_out.tile([H, GW], f32, tag="ops",
                                                  name=f"ops_{g}")
                out_ps = state["out_ps"]
                n0 = (ci * len(units)) // chunks
                n1 = ((ci + 1) * len(units)) // chunks
                pw = GW // npair
                for kind, p, yc, r in units[n0:n1]:
                    state["nmm"][p] += 1
                    ow = out_ps[:, p * pw:(p + 1) * pw]
                    first = state["nmm"][p] == 1
                    last = state["nmm"][p] == nu_pair
                    gpp = G // npair
                    if kind == "32":
                        nc.tensor.matmul(
                            ow,
                            mt_sb[yc][:, r * H:(r + 1) * H],
                            wgv[:, yc, r, p * gpp:(p + 1) * gpp],
                            start=first, stop=last, skip_group_check=True)
                    else:
                        mt8v = mt8_sb[:].rearrange("p (c ri) -> p c ri", c=2)
                        wg8v = wg8_sb[:].rearrange(
                            "p (c r g w) -> p c r g w", c=2, r=nr8, g=G)
                        nc.tensor.matmul(
                            ow,
                            mt8v[:, :, r * H:(r + 1) * H],
                            wg8v[:, :, r, p * gpp:(p + 1) * gpp],
                            start=first, stop=last,
                            perf_mode=DR, skip_group_check=True)
                if ci == chunks - 1:
                    og = out_d[g * G:(g + 1) * G].rearrange("g h w -> h g w")
                    out_sb = osb_pool.tile([H, GW], f32, tag="osb")
                    oe = os.environ.get("AFA_OE", "v")
                    if os.environ.get("AFA_OSPLIT", "0") == "1":
                        for p in range(G // 2):
                            nc.vector.tensor_copy(
                                out_sb[:, p * 2 * H:(p + 1) * 2 * H],
                                out_ps[:, p * 2 * H:(p + 1) * 2 * H])
                    elif oe == "s":
                        nc.scalar.activation(out_sb[:], out_ps[:],
                                             mybir.ActivationFunctionType.Copy)
                    else:
                        nc.vector.tensor_copy(out_sb[:], out_ps[:])
                    nc.sync.dma_start(
                        og, out_sb[:].rearrange("h (g w) -> h g w", g=G))

            # slot loop: A+evacs, then B(g-1) chunk, then next-group S1 piece
            s1pos = os.environ.get("AFA_S1POS", "late")
            bch = int(os.environ.get("AFA_BCH", str(G)))   # B chunk count
            bst = int(os.environ.get("AFA_BST", "0"))      # B start slot
            for m in range(G):
                if s1pos == "early" and nxt is not None:
                    if m % 2 == 0:
                        _emit_s1a(g + 1, m // 2, x_next, nxt)
                    _emit_s1b(g + 1, m, nxt)
                _emit_a(m)
                if pending_b is not None and bst <= m < bst + bch:
                    pending_b(m - bst, chunks=bch)
                if s1pos != "early" and nxt is not None:
                    if m % 2 == 0:
                        _emit_s1a(g + 1, m // 2, x_next, nxt)
                    _emit_s1b(g + 1, m, nxt)

            return _pass_b, nxt

        def _emit_all_groups():
            ngroups = n_img // G
            pending = None
            x_next = _fetch_x(0)
            _load_heavy_consts()
            # prologue: S1 for group 0
            acts_next = {"act": [], "act8": []}
            for half in range(G // 2):
                _emit_s1a(0, half, x_next, acts_next)
                _emit_s1b(0, half * 2, acts_next)
                _emit_s1b(0, half * 2 + 1, acts_next)
            for g in range(ngroups):
                x_cur = x_next
                x_next = _fetch_x(g + 1) if g + 1 < ngroups else None
                acts_cur = acts_next
                pending, acts_next = _emit_group(g, pending, acts_cur, x_next)
            if pending is not None:
                bch = int(os.environ.get("AFA_BCH", str(G)))
                for ci in range(bch):
                    pending(ci, chunks=bch)

        if loop_reps > 1:
            with tc.For_i(0, loop_reps, 1):
                _emit_all_groups()
        else:
            _emit_all_groups()


_NC_CACHE = {}


def _build_nc(n_img=N_IMG, k32=4, nr8=4, group=None, s_e=1.0, loop_reps=1):
    if group is None:
        group = GROUP
    key = (n_img, k32, nr8, group, loop_reps, round(s_e, 12),
           os.environ.get("AFA_PSO", ""), os.environ.get("AFA_S1POS", ""),
           os.environ.get("AFA_BCH", ""), os.environ.get("AFA_BST", ""),
           os.environ.get("AFA_OSPLIT", ""), os.environ.get("AFA_G", ""),
           os.environ.get("AFA_WSPLIT", ""), os.environ.get("AFA_OE", ""),
           os.environ.get("AFA_BGRP", ""), os.environ.get("AFA_S1PRIO", ""),
           os.environ.get("AFA_W32H", ""), os.environ.get("AFA_PSWH", ""),
           os.environ.get("AFA_PSW", ""), os.environ.get("AFA_PSS", ""),
           os.environ.get("AFA_PBD", ""), os.environ.get("AFA_TMPB", ""),
           os.environ.get("AFA_ACTB", ""), os.environ.get("AFA_WB", ""))
    if key in _NC_CACHE:
        return _NC_CACHE[key]
    nc = bacc.Bacc("TRN2", target_bir_lowering=False, debug=False)
    f16 = mybir.dt.float16
    f32 = mybir.dt.float32
    f32r = mybir.dt.float32r
    fp8 = mybir.dt.float8e4
    RC = k32 * H
    x_d = nc.dram_tensor("x", [n_img, H, H], f16, kind="ExternalInput").ap()
    uyt_d = nc.dram_tensor("uyt", [H, H2], f16, kind="ExternalInput").ap()
    uxt_d = nc.dram_tensor("uxt", [H, H2], f16, kind="ExternalInput").ap()
    nt_d = nc.dram_tensor("nt", [2, H, RC], f16, kind="ExternalInput").ap()
    mt32_d = nc.dram_tensor("mt32", [2, H, RC], f16, kind="ExternalInput").ap()
    ins = [x_d, uyt_d, uxt_d, nt_d, mt32_d]
    if nr8:
        nt8_d = nc.dram_tensor("nt8", [H, 2 * nr8 * H], fp8,
                               kind="ExternalInput").ap()
        mt8_d = nc.dram_tensor("mt8", [H, 2 * nr8 * H], fp8,
                               kind="ExternalInput").ap()
        ins += [nt8_d, mt8_d]
    out_d = nc.dram_tensor("out", [n_img, H, H], f32, kind="ExternalOutput").ap()
    with tile.TileContext(nc) as tc:
        _build_tile_program(tc, [out_d], ins, n_img=n_img, k32=k32, nr8=nr8,
                            group=group, s_e=s_e, loop_reps=loop_reps)
    nc.compile()
    _NC_CACHE[key] = nc
    return nc


def _pick_rank(filt):
    """Smallest rank whose weighted-truncation error estimate fits the
    error budget (harness gate 2e-2; leave room for fp16/fp8 quantization).
    For the reference's sinc filter this lands on 8."""
    if RANK_ENV:
        return int(RANK_ENV)
    F = np.asarray(filt, np.float64)
    kf = F.shape[0]
    D = _ac_matrix(H, H2)
    Uu = _ac_matrix(H2, H)
    Zs = [D @ _shift_mat(H2, u - kf // 2) @ Uu for u in range(kf)]
    B = np.zeros((kf, kf))
    for u in range(kf):
        for v in range(u, kf):
            B[u, v] = B[v, u] = np.sum(Zs[u] * Zs[v])
    w, V = np.linalg.eigh(B)
    Bh = (V * np.sqrt(np.maximum(w, 1e-12))) @ V.T
    s = np.linalg.svd(Bh @ F @ Bh, compute_uv=False)
    nrm = np.sqrt(np.sum(s * s))
    for r in range(4, 16):
        if r >= len(s) or np.sqrt(np.sum(s[r:] ** 2)) <= 4e-3 * nrm:
            return r
    return 16


def _split_rank(rank):
    k32 = int(K32_ENV) if K32_ENV else max(rank - 4, rank // 2)
    k32 = min(k32, rank)
    return k32, rank - k32


def _make_in_maps(x, filt, rank, consts=None):
    k32, nr8 = _split_rank(rank)
    if consts is None:
        consts = _make_consts(filt, rank, k32)
    f16 = np.float16
    imgs = x.reshape(N_CORES, N_IMG, H, H)
    base = {
        "uyt": consts["uyt"].astype(f16), "uxt": consts["uxt"].astype(f16),
        "nt": consts["nt"].astype(f16), "mt32": consts["mt32"],
    }
    if consts["nr8"]:
        base["nt8"] = consts["nt8"]
        base["mt8"] = consts["mt8"]
    return [{"x": np.ascontiguousarray(imgs[core]).astype(f16), **base}
            for core in range(N_CORES)]


_RUNNER_CACHE = {}


def _get_runner(nc):
    """Persistent jitted 8-core runner (mirrors bass2jax.run_bass_via_pjrt's
    multi-core path) so repeated kernel() calls reuse one compiled executable."""
    if id(nc) in _RUNNER_CACHE:
        return _RUNNER_CACHE[id(nc)]
    import jax
    from jax.sharding import Mesh, PartitionSpec
    from jax.experimental.shard_map import shard_map
    from concourse.bass2jax import (_bass_exec_p, install_neuronx_cc_hook,
                                    partition_id_tensor)
    install_neuronx_cc_hook()
    in_names, out_names, out_avals, zero_outs = [], [], [], []
    for alloc in nc.m.functions[0].allocations:
        if not isinstance(alloc, mybir.MemoryLocationSet):
            continue
        name = alloc.memorylocations[0].name
        if alloc.kind == "ExternalInput":
            if nc.partition_id_tensor is not None and name == nc.partition_id_tensor.name:
                continue
            in_names.append(name)
        elif alloc.kind == "ExternalOutput":
            out_names.append(name)
            shape = tuple(alloc.tensor_shape)
            dtype = mybir.dt.np(alloc.dtype)
            out_avals.append(jax.core.ShapedArray(shape, dtype))
            zero_outs.append(np.zeros(shape, dtype))
    n_params = len(in_names)
    all_in_names = in_names + out_names
    if nc.partition_id_tensor is not None:
        all_in_names = all_in_names + [nc.partition_id_tensor.name]

    def _body(*args):
        operands = list(args)
        if nc.partition_id_tensor is not None:
            operands.append(partition_id_tensor())
        return tuple(_bass_exec_p.bind(
            *operands,
            out_avals=tuple(out_avals),
            in_names=tuple(all_in_names),
            out_names=tuple(out_names),
            lowering_input_output_aliases=(),
            sim_require_finite=True,
            sim_require_nnan=True,
            nc=nc,
        ))

    donate = tuple(range(n_params, n_params + len(out_names)))
    devices = jax.devices()[:N_CORES]
    mesh = Mesh(np.asarray(devices), ("core",))
    in_specs = (PartitionSpec("core"),) * (n_params + len(out_names))
    out_specs = (PartitionSpec("core"),) * len(out_names)
    sharded = jax.jit(
        shard_map(_body, mesh=mesh, in_specs=in_specs, out_specs=out_specs,
                  check_rep=False),
        donate_argnums=donate, keep_unused=True)
    runner = (sharded, in_names, out_names, out_avals, zero_outs)
    _RUNNER_CACHE[id(nc)] = runner
    return runner


def run(x, filt):
    """Run on 8 cores. Returns out [B,C,H,W] f32."""
    x = np.ascontiguousarray(np.asarray(x, dtype=np.float32))
    filt = np.asarray(filt, dtype=np.float32)
    B, C, Hh, Ww = x.shape
    assert (Hh, Ww) == (H, H) and B * C == N_CORES * N_IMG
    rank = _pick_rank(filt)
    k32, nr8 = _split_rank(rank)
    consts = _make_consts(filt, rank, k32)
    in_maps = _make_in_maps(x, filt, rank, consts)
    nc = _build_nc(k32=k32, nr8=nr8, s_e=consts["s_e"])
    try:
        sharded, in_names, out_names, out_avals, zero_outs = _get_runner(nc)
        concat_in = [np.concatenate([in_maps[c][nm] for c in range(N_CORES)], axis=0)
                     for nm in in_names]
        concat_zero = [np.zeros((N_CORES * z.shape[0], *z.shape[1:]), z.dtype)
                       for z in zero_outs]
        outs = sharded(*concat_in, *concat_zero)
        oi = out_names.index("out")
        out = np.asarray(outs[oi]).reshape(N_CORES, *out_avals[oi].shape)
    except Exception:
        res = run_bass_kernel_spmd(nc, in_maps, core_ids=list(range(N_CORES)))
        out = np.stack([res.results[c]["out"] for c in range(N_CORES)])
    return out.reshape(B, C, H, H).astype(np.float32, copy=False)


def kernel(x, filt):
    return run(x, filt)


# revision 28
# speedup vs baseline: 1.7678x; 1.0124x over previous
"""AliasFreeActivation (upsample2x -> leaky_relu -> 31x31 depthwise sinc conv
-> downsample2x) as a Trainium2 Bass/Tile kernel, data-parallel over 8 cores.

Math (per [128,128] image; B*C = 512 images, 64 per core):
  out = Dy @ Conv_F(lrelu(Uy @ x @ Ux^T)) @ Dx^T
With F = sum_r g_r h_r^T (weighted SVD of the 31x31 filter, rank 8):
  out = sum_r M_r @ act @ N_r^T
  M_r = Dy @ Toeplitz(g_r) [128,256],  N_r = Dx @ Toeplitz(h_r) [128,256]
  act = lrelu(Uy @ x @ Ux^T) [256,256]

Precision/engine split (v2):
  ranks 0..k32-1 : A-pass fp16 (banded), W evac'd to fp16 by a plain DVE
                   copy, B-pass fp16 matmuls over image-pair units.
  ranks k32..    : A-pass fp8 DoubleRow, W evac = PLAIN Pool copy to fp8
                   (the old s_e scale is folded into nt8), B-pass fp8 DR.
PSUM evacuation engines: ACT = lrelu + tmpT + out; DVE = W32; Pool
(gpsimd, which CAN read PSUM on TRN2) = W8 + act8.

Device dataflow per image (out[m,n] = sum_k lhsT[k,m] rhs[k,n]):
  S1a: tmpT[c,Y]    = sum_y x[y,c] UyT[y,Y]
  S1b: actT[X,Y]    = sum_c UxT[c,X] tmpT[c,Y]   (+ lrelu on evac)
  A:   W[Y,(r,j)]   = sum_X actT[X,Y] NT[X,(r,j)]
  B:   out[i,(m,j)] = sum_{r,Yc} MT[Yc,(r,i)] W[Yc,(r,m,j)]   (PSUM accum)
"""
import contextlib
import os

import numpy as np

import concourse.bass as bass
import concourse.mybir as mybir
import concourse.tile as tile
from concourse import bacc
from concourse.bass_utils import run_bass_kernel_spmd

H = 128
H2 = 256
KF = 31
LRELU_SLOPE = 0.01
RANK_ENV = os.environ.get("AFA_RANK")
N_CORES = 8
N_IMG = 64                      # images per core (512 / 8)
GROUP = int(os.environ.get("AFA_G", "4"))
K32_ENV = os.environ.get("AFA_K32")


# ---------------- host-side constants ----------------

def _ac_matrix(out_n, in_n):
    scale = (in_n - 1) / (out_n - 1)
    c = np.arange(out_n, dtype=np.float64) * scale
    i0 = np.clip(np.floor(c).astype(np.int64), 0, in_n - 2)
    w = c - i0
    M = np.zeros((out_n, in_n), dtype=np.float64)
    M[np.arange(out_n), i0] = 1.0 - w
    M[np.arange(out_n), i0 + 1] = w
    return M


def _toeplitz_same(h, n):
    T = np.zeros((n, n), dtype=np.float64)
    for u in range(len(h)):
        d = u - len(h) // 2
        if d >= 0:
            idx = np.arange(0, n - d)
        else:
            idx = np.arange(-d, n)
        T[idx, idx + d] += h[u]
    return T


def _shift_mat(n, d):
    S = np.zeros((n, n))
    idx = np.arange(0, n - d) if d >= 0 else np.arange(-d, n)
    S[idx, idx + d] = 1.0
    return S


def _weighted_rank(F, rank):
    """Rank-`rank` approx of F minimizing the end-to-end error under the
    signal model act ~ U x U^T with white x: err = ||B^1/2 (F-Fr) B^1/2||_F
    with B the Gram of the composed per-tap maps Z_u = D S_u U."""
    kf = F.shape[0]
    D = _ac_matrix(H, H2)
    Uu = _ac_matrix(H2, H)
    Zs = [D @ _shift_mat(H2, u - kf // 2) @ Uu for u in range(kf)]
    B = np.zeros((kf, kf))
    for u in range(kf):
        for v in range(u, kf):
            B[u, v] = B[v, u] = np.sum(Zs[u] * Zs[v])
    w, V = np.linalg.eigh(B)
    w = np.maximum(w, 1e-12)
    Bh = (V * np.sqrt(w)) @ V.T
    Bih = (V / np.sqrt(w)) @ V.T
    Gm = Bh @ F @ Bh
    U_, S_, Vt_ = np.linalg.svd(Gm)
    Fr = Bih @ (U_[:, :rank] * S_[:rank]) @ Vt_[:rank] @ Bih
    return Fr


def _sample_act(Uu, n=4):
    rng = np.random.default_rng(1234)
    X = rng.standard_normal((n, H, H))
    A = Uu @ X @ Uu.T
    return np.where(A >= 0, A, LRELU_SLOPE * A)


def _make_consts(filt, rank, k32):
    """Build device constants.

    Ranks [0, k32): nt fp16 in seg layout (columns (j, r_local)-major for the
    banded A-pass 2D APs), mt as RAW fp32 (used as float32r), rank-major.
    Ranks [k32, rank): nt8/mt8 fp8 DoubleRow-interleaved, with per-rank scale
    n_r chosen so that nt8, W8 (= s_a n_r act N^T, evac'd by a PLAIN copy)
    and mt8 (= M / (s_a n_r)) all sit inside the fp8e4m3 normal range.
    s_a is folded into uyt (lrelu is positively homogeneous) and 1/s_a into
    nt/mt-fp32, so every rank's B contribution lands in out PSUM at scale 1.
    """
    F = np.asarray(filt, dtype=np.float64)
    if rank < min(F.shape):
        F = _weighted_rank(F, rank)
    U, S, Vt = np.linalg.svd(F)
    D = _ac_matrix(H, H2)
    Uu = _ac_matrix(H2, H)
    uyt = np.ascontiguousarray(Uu.T)               # [128 y, 256 Y]

    nr8 = rank - k32
    Ms = [D @ _toeplitz_same(U[:, r] * np.sqrt(S[r]), H2) for r in range(rank)]
    Ns = [D @ _toeplitz_same(Vt[r, :] * np.sqrt(S[r]), H2) for r in range(rank)]

    # fp8 scales: act8 = s_a * act (s_a folded into uyt)
    act = _sample_act(Uu)
    actmax = np.abs(act).max() * 1.15
    s_a = 100.0 / actmax

    # fp16/fp32 ranks: nt gets 1/s_a (act arrives pre-scaled by s_a);
    # mt stays exact fp32.
    nt = np.zeros((2, H, k32 * H), dtype=np.float32)
    mt32 = np.zeros((2, H, k32 * H), dtype=np.float16)
    for r in range(k32):
        cols = np.arange(H) * k32 + r              # (j, r_local)-major
        for c in range(2):
            nt[c, :, cols] = (Ns[r][:, c * H:(c + 1) * H] / s_a).astype(np.float32)
            mt32[c, :, r * H:(r + 1) * H] = \
                Ms[r][:, c * H:(c + 1) * H].T.astype(np.float16)

    out = {"uyt": (uyt * s_a).astype(np.float32), "uxt": uyt.astype(np.float32),
           "nt": nt, "mt32": mt32, "k32": k32, "nr8": nr8, "s_a": float(s_a),
           "s_e": 1.0}
    if nr8 == 0:
        return out

    # ---- fp8 tail: per-rank scales (baseline scheme): wg8 = s_e * W8psum
    # (s_e applied for free by the ACT Copy evac), n_r/m_r sqrt-balanced so
    # act8, nt8, wg8, mt8 all sit inside the fp8e4m3 normal range.
    np8 = mybir.dt.np(mybir.dt.float8e4)
    nt8 = np.zeros((H, 2, nr8 * H), dtype=np.float64)
    mt8 = np.zeros((H, 2, nr8 * H), dtype=np.float64)
    wmaxs = [np.abs(act @ Ns[k32 + i].T).max() * 1.3 for i in range(nr8)]
    mmaxs = [np.abs(Ms[k32 + i]).max() for i in range(nr8)]
    nmaxs = [np.abs(Ns[k32 + i]).max() for i in range(nr8)]
    bias = 2.0
    tmp = [np.sqrt(wmaxs[i] * mmaxs[i]) * bias / (s_a * wmaxs[i])
           for i in range(nr8)]           # = n_r * s_e per rank
    s_e = float(np.exp(np.mean(np.log([tmp[i] * nmaxs[i]
                                       for i in range(nr8)]))) / 100.0)
    for i in range(nr8):
        r = k32 + i
        n_r = tmp[i] / s_e
        m_r = 1.0 / (s_a * n_r * s_e)
        for c in range(2):
            # nt8[p, xc, j*nr8+i] = n_r * N_r[j, xc*128+p]
            nt8[:, c, i::nr8] = (n_r * Ns[r][:, c * H:(c + 1) * H]).T
            # mt8[p, yc, i*H+k] = m_r * M_r[k, yc*128+p]
            mt8[:, c, i * H:(i + 1) * H] = (m_r * Ms[r][:, c * H:(c + 1) * H]).T
    out["s_e"] = s_e
    out["nt8"] = nt8.astype(np8).reshape(H, 2 * nr8 * H)
    out["mt8"] = mt8.astype(np8).reshape(H, 2 * nr8 * H)
    return out


# ---------------- device program ----------------

def _build_tile_program(tc, outs, ins, *, n_img, k32, nr8, group,
                        s_e=1.0, loop_reps=1):
    nc = tc.nc
    if nr8:
        x_d, uyt_d, uxt_d, nt_d, mt32_d, nt8_d, mt8_d = ins
    else:
        x_d, uyt_d, uxt_d, nt_d, mt32_d = ins
    out_d = outs[0]
    RC = k32 * H
    G = group
    GW = G * H
    assert G in (2, 4)
    assert n_img % G == 0
    f16 = mybir.dt.float16
    f32 = mybir.dt.float32
    f32r = mybir.dt.float32r
    fp8 = mybir.dt.float8e4
    DR = mybir.MatmulPerfMode.DoubleRow

    ctx = contextlib.ExitStack()
    with ctx:
        const_pool = ctx.enter_context(tc.tile_pool(name="consts", bufs=1))
        x_pool = ctx.enter_context(tc.tile_pool(name="x", bufs=2))
        tmp_pool = ctx.enter_context(tc.tile_pool(
            name="tmp", bufs=int(os.environ.get("AFA_TMPB", "3"))))
        act_pool = ctx.enter_context(tc.tile_pool(
            name="act", bufs=int(os.environ.get("AFA_ACTB", "8"))))
        w_pool = ctx.enter_context(tc.tile_pool(
            name="w", bufs=int(os.environ.get("AFA_WB", "2"))))
        osb_pool = ctx.enter_context(tc.tile_pool(name="osb", bufs=2))
        # PSUM banks: ps_s 3 + ps_w 2x2 + ps_out 1 = 8
        ps_s = ctx.enter_context(tc.tile_pool(
            name="ps_s", bufs=int(os.environ.get("AFA_PSS", "1")), space="PSUM"))
        w32h = os.environ.get("AFA_W32H", "0") == "1"
        if w32h:
            ps_w = ctx.enter_context(tc.tile_pool(
                name="ps_w", bufs=int(os.environ.get("AFA_PSW", "2")),
                space="PSUM"))
            ps_wh = ctx.enter_context(tc.tile_pool(
                name="ps_wh", bufs=int(os.environ.get("AFA_PSWH", "2")),
                space="PSUM"))
        else:
            ps_w = ctx.enter_context(tc.tile_pool(
                name="ps_w", bufs=int(os.environ.get("AFA_PSW", "3")),
                space="PSUM"))
        ps_out = ctx.enter_context(tc.tile_pool(
            name="ps_out", bufs=int(os.environ.get("AFA_PSO", "1")),
            space="PSUM"))

        uyt_sb = const_pool.tile([H, H2], f16, tag="uyt")
        uxt_sb = const_pool.tile([H, H2], f16, tag="uxt")

        def _load_s1_consts():
            nc.sync.dma_start(uyt_sb[:], uyt_d[:])
            nc.sync.dma_start(uxt_sb[:], uxt_d[:])
        nt_sb = []
        mt_sb = []
        for c in range(2):
            t = const_pool.tile([H, RC], f16, tag=f"nt{c}", name=f"nt{c}_sb")
            nt_sb.append(t)
            t = const_pool.tile([H, RC], f16, tag=f"mt{c}", name=f"mt{c}_sb")
            mt_sb.append(t)
        nt8_sb = mt8_sb = None
        if nr8:
            nt8_sb = const_pool.tile([H, 2 * nr8 * H], fp8, tag="nt8")
            mt8_sb = const_pool.tile([H, 2 * nr8 * H], fp8, tag="mt8")

        def _load_heavy_consts():
            # issued AFTER x(0)/uyt/uxt so the serial HWDGE queue does not
            # gate the S1 prologue; these are only needed by pass A/B.
            for c in range(2):
                nc.sync.dma_start(nt_sb[c][:], nt_d[c])
            if nr8:
                nc.sync.dma_start(nt8_sb[:], nt8_d[:])
            for c in range(2):
                nc.sync.dma_start(mt_sb[c][:], mt32_d[c])
            if nr8:
                nc.sync.dma_start(mt8_sb[:], mt8_d[:])

        def _fetch_x(g):
            x_sb = x_pool.tile([H, GW], f16, tag="x", name=f"x_{g}")
            xg = x_d[g * G:(g + 1) * G].rearrange("g h w -> h g w")
            nc.sync.dma_start(x_sb[:].rearrange("h (g w) -> h g w", g=G), xg)
            return x_sb

        import contextlib as _ctl

        def _s1prio():
            off = os.environ.get("AFA_S1PRIO", "")
            if off == "":
                return _ctl.nullcontext()
            return tc.high_priority(None if off == "0" else int(off))

        def _emit_s1a(g, half, x_sb, st, pool=None):
            """S1a matmuls for an image pair + tmpT evac (DVE)."""
            with _s1prio():
                tmpT_ps = (pool or ps_s).tile([H, 2 * H2], f32,
                                              tag="sp" if pool is None else "wps",
                                              name=f"tp_{g}_{half}")
                for u in range(2):
                    m = half * 2 + u
                    nc.tensor.matmul(tmpT_ps[:, u * H2:(u + 1) * H2],
                                     x_sb[:, m * H:(m + 1) * H], uyt_sb[:],
                                     start=True, stop=True)
                t_sb = tmp_pool.tile([H, 2 * H2], f16, tag="tmpT")
                nc.vector.tensor_copy(t_sb[:], tmpT_ps[:])
                st["t_sb"] = t_sb

        def _emit_s1b(g, m, st, pool=None):
            """S1b matmuls + lrelu (ACT) + act8 (Pool) for one image."""
            with _s1prio():
                act_ps = (pool or ps_s).tile(
                    [H, 2 * H2], f32,
                    tag="sp" if pool is None else "wps",
                    name=f"ap_{g}_{m}")
                tw = st["t_sb"][:, (m % 2) * H2:(m % 2 + 1) * H2]
                for xc in range(2):
                    nc.tensor.matmul(act_ps[:, xc * H2:(xc + 1) * H2],
                                     uxt_sb[:, xc * H:(xc + 1) * H], tw,
                                     start=True, stop=True)
                act_sb = act_pool.tile([H, 2 * H2], f16, tag="act")
                nc.scalar.activation(act_sb[:], act_ps[:],
                                     mybir.ActivationFunctionType.Lrelu,
                                     alpha=LRELU_SLOPE)
                st["act"].append(act_sb)
                if nr8:
                    act8_sb = act_pool.tile([H, 2 * H2], fp8, tag="act8")
                    nc.gpsimd.tensor_copy(act8_sb[:], act_sb[:])
                    st["act8"].append(act8_sb)

        def _emit_group(g, pending_b, acts_cur, x_next):
            wg_sb = w_pool.tile([H, 2 * k32 * GW], f16, tag="wg",
                                name=f"wg_{g}")
            wgv = wg_sb[:].rearrange("p (c r g w) -> p c r g w", c=2, r=k32, g=G)
            wg8_sb = None
            if nr8:
                wg8_sb = w_pool.tile([H, 2 * nr8 * GW], fp8, tag="wg8",
                                     name=f"wg8_{g}")
            act_sbs = acts_cur["act"]
            act8_sbs = acts_cur["act8"]
            nxt = {"act": [], "act8": []} if x_next is not None else None

            # pass A for image m of this group; W evacs right after the
            # producing matmuls so ps_w recycles fast.
            # fp16 part: nt/W_ps columns are (j, r_local)-major, so the
            # Toeplitz j-band of each X-chunk is a CONTIGUOUS column window:
            # X-chunk0 only reaches j<=71, chunk1 only j>=56; j in [56,72)
            # accumulates, the rest first-write.
            # fp8 part: one DoubleRow matmul per yc contracts both X-chunks.
            def _emit_a_split(m):
                act_sb = act_sbs[m]
                jwin = ((0, 72), (56, H))
                dst8v = wg8_sb[:].rearrange(
                    "p (c r g w) -> p c r g w", c=2, r=nr8, g=G)
                act8 = act8_sbs[m][:].rearrange("p (x y) -> p x y", x=2)
                nt8v = nt8_sb[:].rearrange("p (x c) -> p x c", x=2)
                for yc in range(2):
                    w_ps8 = ps_w.tile([H, 512], f32, tag="wps",
                                      name=f"wps8_{g}_{m}_{yc}")
                    nc.tensor.matmul(
                        w_ps8[:], act8[:, :, yc * H:(yc + 1) * H], nt8v,
                        start=True, stop=True, perf_mode=DR)
                    src8 = w_ps8[:].rearrange("p (j r) -> p r j", r=nr8)
                    if yc == 0:
                        nc.scalar.activation(dst8v[:, yc, :, m], src8,
                                             mybir.ActivationFunctionType.Copy,
                                             scale=s_e)
                    else:
                        nc.vector.tensor_scalar_mul(dst8v[:, yc, :, m],
                                                    src8, s_e)
                for yc in range(2):
                    w_ps = ps_w.tile([H, 512], f32, tag="wps",
                                     name=f"wps_{g}_{m}_{yc}")
                    for xc in range(2):
                        j0, j1 = jwin[xc]
                        nc.tensor.matmul(
                            w_ps[:, j0 * k32:j1 * k32],
                            act_sb[:, xc * H2 + yc * H: xc * H2 + (yc + 1) * H],
                            nt_sb[xc][:, j0 * k32:j1 * k32],
                            start=(xc == 0), stop=(xc == 1),
                            skip_group_check=True)
                    src = w_ps[:, :k32 * H].rearrange("p (j r) -> p r j", r=k32)
                    if yc == 0:
                        nc.vector.tensor_copy(wgv[:, yc, :, m], src)
                    else:
                        nc.scalar.activation(wgv[:, yc, :, m], src,
                                             mybir.ActivationFunctionType.Copy)

            def _emit_a(m):
                if os.environ.get("AFA_WSPLIT", "0") == "1":
                    _emit_a_split(m)
                    return
                act_sb = act_sbs[m]
                if nr8:
                    act8 = act8_sbs[m][:].rearrange("p (x y) -> p x y", x=2)
                    nt8v = nt8_sb[:].rearrange("p (x c) -> p x c", x=2)
                    w_ps8 = ps_w.tile([H, 1024], f32, tag="wps",
                                      name=f"wps8_{g}_{m}")
                    for yc in range(2):
                        nc.tensor.matmul(
                            w_ps8[:, yc * 512:(yc + 1) * 512],
                            act8[:, :, yc * H:(yc + 1) * H],
                            nt8v,
                            start=True, stop=True, perf_mode=DR)
                    # W8 evac: PLAIN fp8 copy on ACT (scales folded into nt8)
                    dst8v = wg8_sb[:].rearrange(
                        "p (c r g w) -> p c r g w", c=2, r=nr8, g=G)
                    if nr8 == 4:
                        src8 = w_ps8[:].rearrange(
                            "p (c j r) -> p c r j", c=2, r=nr8)
                        nc.scalar.activation(dst8v[:, :, :, m], src8,
                                             mybir.ActivationFunctionType.Copy,
                                             scale=s_e)
                    else:
                        for yc in range(2):
                            src8 = w_ps8[:, yc * 512:yc * 512 + nr8 * H] \
                                .rearrange("p (j r) -> p r j", r=nr8)
                            nc.scalar.activation(
                                dst8v[:, yc, :, m], src8,
                                mybir.ActivationFunctionType.Copy, scale=s_e)
                jwin = ((0, 72), (56, H))
                if w32h:
                    for yc in range(2):
                        w_ph = ps_wh.tile([H, 512], f32, tag="wph",
                                          name=f"wph_{g}_{m}_{yc}")
                        for xc in range(2):
                            j0, j1 = jwin[xc]
                            nc.tensor.matmul(
                                w_ph[:, j0 * k32:j1 * k32],
                                act_sb[:, xc * H2 + yc * H:
                                       xc * H2 + (yc + 1) * H],
                                nt_sb[xc][:, j0 * k32:j1 * k32],
                                start=(xc == 0), stop=(xc == 1),
                                skip_group_check=True)
                        srch = w_ph[:, :k32 * H].rearrange(
                            "p (j r) -> p r j", r=k32)
                        nc.vector.tensor_copy(wgv[:, yc, :, m], srch)
                else:
                    w_ps = ps_w.tile([H, 1024], f32, tag="wps",
                                     name=f"wps_{g}_{m}")
                    for yc in range(2):
                        for xc in range(2):
                            j0, j1 = jwin[xc]
                            nc.tensor.matmul(
                                w_ps[:, yc * 512 + j0 * k32:yc * 512 + j1 * k32],
                                act_sb[:, xc * H2 + yc * H: xc * H2 + (yc + 1) * H],
                                nt_sb[xc][:, j0 * k32:j1 * k32],
                                start=(xc == 0), stop=(xc == 1),
                                skip_group_check=True)
                    # W32 evac: RAW fp32 copy on DVE
                    if k32 == 4:
                        src = w_ps[:].rearrange("p (c j r) -> p c r j", c=2, r=k32)
                        nc.vector.tensor_copy(wgv[:, :, :, m], src)
                    else:
                        for yc in range(2):
                            src = w_ps[:, yc * 512:yc * 512 + k32 * H].rearrange(
                                "p (j r) -> p r j", r=k32)
                            nc.vector.tensor_copy(wgv[:, yc, :, m], src)

            # pass B over image PAIRS: each unit is a 256-col matmul touching
            # only images (p*2, p*2+1), so chunk ci only needs W of pair<=ci
            # from the previous group -> full-slot slack at pbd=0.
            state = {"nmm": [0, 0], "out_ps": None}
            if os.environ.get("AFA_BGRP", "0") == "1":
                npair = 1
                units = [(kind, 0, yc, r) for kind, yc, r in
                         ([("32", yc, r) for yc in range(2) for r in range(k32)]
                          + [("8", 0, rl) for rl in range(nr8)])]
            else:
                npair = G // 2
                units = [(kind, p, yc, r)
                         for p in range(npair)
                         for kind, yc, r in
                         ([("32", yc, r) for yc in range(2) for r in range(k32)]
                          + [("8", 0, rl) for rl in range(nr8)])]
            nu_pair = len(units) // npair

            def _pass_b(ci, chunks=G):
                if state["out_ps"] is None:
                    state["out_ps"] = ps_out.tile([H, GW], f32, tag="ops",
                                                  name=f"ops_{g}")
                out_ps = state["out_ps"]
                n0 = (ci * len(units)) // chunks
                n1 = ((ci + 1) * len(units)) // chunks
                pw = GW // npair
                for kind, p, yc, r in units[n0:n1]:
                    state["nmm"][p] += 1
                    ow = out_ps[:, p * pw:(p + 1) * pw]
                    first = state["nmm"][p] == 1
                    last = state["nmm"][p] == nu_pair
                    gpp = G // npair
                    if kind == "32":
                        nc.tensor.matmul(
                            ow,
                            mt_sb[yc][:, r * H:(r + 1) * H],
                            wgv[:, yc, r, p * gpp:(p + 1) * gpp],
                            start=first, stop=last, skip_group_check=True)
                    else:
                        mt8v = mt8_sb[:].rearrange("p (c ri) -> p c ri", c=2)
                        wg8v = wg8_sb[:].rearrange(
                            "p (c r g w) -> p c r g w", c=2, r=nr8, g=G)
                        nc.tensor.matmul(
                            ow,
                            mt8v[:, :, r * H:(r + 1) * H],
                            wg8v[:, :, r, p * gpp:(p + 1) * gpp],
                            start=first, stop=last,
                            perf_mode=DR, skip_group_check=True)
                if ci == chunks - 1:
                    og = out_d[g * G:(g + 1) * G].rearrange("g h w -> h g w")
                    out_sb = osb_pool.tile([H, GW], f32, tag="osb")
                    oe = os.environ.get("AFA_OE", "v")
                    if os.environ.get("AFA_OSPLIT", "0") == "1":
                        for p in range(G // 2):
                            nc.vector.tensor_copy(
                                out_sb[:, p * 2 * H:(p + 1) * 2 * H],
                                out_ps[:, p * 2 * H:(p + 1) * 2 * H])
                    elif oe == "s":
                        nc.scalar.activation(out_sb[:], out_ps[:],
                                             mybir.ActivationFunctionType.Copy)
                    else:
                        nc.vector.tensor_copy(out_sb[:], out_ps[:])
                    nc.sync.dma_start(
                        og, out_sb[:].rearrange("h (g w) -> h g w", g=G))

            # slot loop: A+evacs, then B(g-1) chunk, then next-group S1 piece
            s1pos = os.environ.get("AFA_S1POS", "late")
            bch = int(os.environ.get("AFA_BCH", str(G)))   # B chunk count
            bst = int(os.environ.get("AFA_BST", "0"))      # B start slot
            for m in range(G):
                if s1pos == "early" and nxt is not None:
                    if m % 2 == 0:
                        _emit_s1a(g + 1, m // 2, x_next, nxt)
                    _emit_s1b(g + 1, m, nxt)
                _emit_a(m)
                if pending_b is not None and bst <= m < bst + bch:
                    pending_b(m - bst, chunks=bch)
                if s1pos != "early" and nxt is not None:
                    if m % 2 == 0:
                        _emit_s1a(g + 1, m // 2, x_next, nxt)
                    _emit_s1b(g + 1, m, nxt)

            return _pass_b, nxt

        def _emit_all_groups():
            ngroups = n_img // G
            pending = None
            x_next = _fetch_x(0)
            _load_s1_consts()
            _load_heavy_consts()
            # prologue: S1 for group 0
            acts_next = {"act": [], "act8": []}
            for half in range(G // 2):
                _emit_s1a(0, half, x_next, acts_next)
                _emit_s1b(0, half * 2, acts_next)
                _emit_s1b(0, half * 2 + 1, acts_next)
            for g in range(ngroups):
                x_cur = x_next
                x_next = _fetch_x(g + 1) if g + 1 < ngroups else None
                acts_cur = acts_next
                pending, acts_next = _emit_group(g, pending, acts_cur, x_next)
            if pending is not None:
                bch = int(os.environ.get("AFA_BCH", str(G)))
                for ci in range(bch):
                    pending(ci, chunks=bch)

        if loop_reps > 1:
            with tc.For_i(0, loop_reps, 1):
                _emit_all_groups()
        else:
            _emit_all_groups()


_NC_CACHE = {}


def _build_nc(n_img=N_IMG, k32=4, nr8=4, group=None, s_e=1.0, loop_reps=1):
    if group is None:
        group = GROUP
    key = (n_img, k32, nr8, group, loop_reps, round(s_e, 12),
           os.environ.get("AFA_PSO", ""), os.environ.get("AFA_S1POS", ""),
           os.environ.get("AFA_BCH", ""), os.environ.get("AFA_BST", ""),
           os.environ.get("AFA_OSPLIT", ""), os.environ.get("AFA_G", ""),
           os.environ.get("AFA_WSPLIT", ""), os.environ.get("AFA_OE", ""),
           os.environ.get("AFA_BGRP", ""), os.environ.get("AFA_S1PRIO", ""),
           os.environ.get("AFA_W32H", ""), os.environ.get("AFA_PSWH", ""),
           os.environ.get("AFA_PSW", ""), os.environ.get("AFA_PSS", ""),
           os.environ.get("AFA_PBD", ""), os.environ.get("AFA_TMPB", ""),
           os.environ.get("AFA_ACTB", ""), os.environ.get("AFA_WB", ""))
    if key in _NC_CACHE:
        return _NC_CACHE[key]
    nc = bacc.Bacc("TRN2", target_bir_lowering=False, debug=False)
    f16 = mybir.dt.float16
    f32 = mybir.dt.float32
    f32r = mybir.dt.float32r
    fp8 = mybir.dt.float8e4
    RC = k32 * H
    x_d = nc.dram_tensor("x", [n_img, H, H], f16, kind="ExternalInput").ap()
    uyt_d = nc.dram_tensor("uyt", [H, H2], f16, kind="ExternalInput").ap()
    uxt_d = nc.dram_tensor("uxt", [H, H2], f16, kind="ExternalInput").ap()
    nt_d = nc.dram_tensor("nt", [2, H, RC], f16, kind="ExternalInput").ap()
    mt32_d = nc.dram_tensor("mt32", [2, H, RC], f16, kind="ExternalInput").ap()
    ins = [x_d, uyt_d, uxt_d, nt_d, mt32_d]
    if nr8:
        nt8_d = nc.dram_tensor("nt8", [H, 2 * nr8 * H], fp8,
                               kind="ExternalInput").ap()
        mt8_d = nc.dram_tensor("mt8", [H, 2 * nr8 * H], fp8,
                               kind="ExternalInput").ap()
        ins += [nt8_d, mt8_d]
    out_d = nc.dram_tensor("out", [n_img, H, H], f32, kind="ExternalOutput").ap()
    with tile.TileContext(nc) as tc:
        _build_tile_program(tc, [out_d], ins, n_img=n_img, k32=k32, nr8=nr8,
                            group=group, s_e=s_e, loop_reps=loop_reps)
    nc.compile()
    _NC_CACHE[key] = nc
    return nc


def _pick_rank(filt):
    """Smallest rank whose weighted-truncation error estimate fits the
    error budget (harness gate 2e-2; leave room for fp16/fp8 quantization).
    For the reference's sinc filter this lands on 8."""
    if RANK_ENV:
        return int(RANK_ENV)
    F = np.asarray(filt, np.float64)
    kf = F.shape[0]
    D = _ac_matrix(H, H2)
    Uu = _ac_matrix(H2, H)
    Zs = [D @ _shift_mat(H2, u - kf // 2) @ Uu for u in range(kf)]
    B = np.zeros((kf, kf))
    for u in range(kf):
        for v in range(u, kf):
            B[u, v] = B[v, u] = np.sum(Zs[u] * Zs[v])
    w, V = np.linalg.eigh(B)
    Bh = (V * np.sqrt(np.maximum(w, 1e-12))) @ V.T
    s = np.linalg.svd(Bh @ F @ Bh, compute_uv=False)
    nrm = np.sqrt(np.sum(s * s))
    for r in range(4, 16):
        if r >= len(s) or np.sqrt(np.sum(s[r:] ** 2)) <= 4e-3 * nrm:
            return r
    return 16


def _split_rank(rank):
    k32 = int(K32_ENV) if K32_ENV else max(rank - 4, rank // 2)
    k32 = min(k32, rank)
    return k32, rank - k32


def _make_in_maps(x, filt, rank, consts=None):
    k32, nr8 = _split_rank(rank)
    if consts is None:
        consts = _make_consts(filt, rank, k32)
    f16 = np.float16
    imgs = x.reshape(N_CORES, N_IMG, H, H)
    base = {
        "uyt": consts["uyt"].astype(f16), "uxt": consts["uxt"].astype(f16),
        "nt": consts["nt"].astype(f16), "mt32": consts["mt32"],
    }
    if consts["nr8"]:
        base["nt8"] = consts["nt8"]
        base["mt8"] = consts["mt8"]
    return [{"x": np.ascontiguousarray(imgs[core]).astype(f16), **base}
            for core in range(N_CORES)]


_RUNNER_CACHE = {}


def _get_runner(nc):
    """Persistent jitted 8-core runner (mirrors bass2jax.run_bass_via_pjrt's
    multi-core path) so repeated kernel() calls reuse one compiled executable."""
    if id(nc) in _RUNNER_CACHE:
        return _RUNNER_CACHE[id(nc)]
    import jax
    from jax.sharding import Mesh, PartitionSpec
    from jax.experimental.shard_map import shard_map
    from concourse.bass2jax import (_bass_exec_p, install_neuronx_cc_hook,
                                    partition_id_tensor)
    install_neuronx_cc_hook()
    in_names, out_names, out_avals, zero_outs = [], [], [], []
    for alloc in nc.m.functions[0].allocations:
        if not isinstance(alloc, mybir.MemoryLocationSet):
            continue
        name = alloc.memorylocations[0].name
        if alloc.kind == "ExternalInput":
            if nc.partition_id_tensor is not None and name == nc.partition_id_tensor.name:
                continue
            in_names.append(name)
        elif alloc.kind == "ExternalOutput":
            out_names.append(name)
            shape = tuple(alloc.tensor_shape)
            dtype = mybir.dt.np(alloc.dtype)
            out_avals.append(jax.core.ShapedArray(shape, dtype))
            zero_outs.append(np.zeros(shape, dtype))
    n_params = len(in_names)
    all_in_names = in_names + out_names
    if nc.partition_id_tensor is not None:
        all_in_names = all_in_names + [nc.partition_id_tensor.name]

    def _body(*args):
        operands = list(args)
        if nc.partition_id_tensor is not None:
            operands.append(partition_id_tensor())
        return tuple(_bass_exec_p.bind(
            *operands,
            out_avals=tuple(out_avals),
            in_names=tuple(all_in_names),
            out_names=tuple(out_names),
            lowering_input_output_aliases=(),
            sim_require_finite=True,
            sim_require_nnan=True,
            nc=nc,
        ))

    donate = tuple(range(n_params, n_params + len(out_names)))
    devices = jax.devices()[:N_CORES]
    mesh = Mesh(np.asarray(devices), ("core",))
    in_specs = (PartitionSpec("core"),) * (n_params + len(out_names))
    out_specs = (PartitionSpec("core"),) * len(out_names)
    sharded = jax.jit(
        shard_map(_body, mesh=mesh, in_specs=in_specs, out_specs=out_specs,
                  check_rep=False),
        donate_argnums=donate, keep_unused=True)
    runner = (sharded, in_names, out_names, out_avals, zero_outs)
    _RUNNER_CACHE[id(nc)] = runner
    return runner


def run(x, filt):
    """Run on 8 cores. Returns out [B,C,H,W] f32."""
    x = np.ascontiguousarray(np.asarray(x, dtype=np.float32))
    filt = np.asarray(filt, dtype=np.float32)
    B, C, Hh, Ww = x.shape
    assert (Hh, Ww) == (H, H) and B * C == N_CORES * N_IMG
    rank = _pick_rank(filt)
    k32, nr8 = _split_rank(rank)
    consts = _make_consts(filt, rank, k32)
    in_maps = _make_in_maps(x, filt, rank, consts)
    nc = _build_nc(k32=k32, nr8=nr8, s_e=consts["s_e"])
    try:
        sharded, in_names, out_names, out_avals, zero_outs = _get_runner(nc)
        concat_in = [np.concatenate([in_maps[c][nm] for c in range(N_CORES)], axis=0)
                     for nm in in_names]
        concat_zero = [np.zeros((N_CORES * z.shape[0], *z.shape[1:]), z.dtype)
                       for z in zero_outs]
        outs = sharded(*concat_in, *concat_zero)
        oi = out_names.index("out")
        out = np.asarray(outs[oi]).reshape(N_CORES, *out_avals[oi].shape)
    except Exception:
        res = run_bass_kernel_spmd(nc, in_maps, core_ids=list(range(N_CORES)))
        out = np.stack([res.results[c]["out"] for c in range(N_CORES)])
    return out.reshape(B, C, H, H).astype(np.float32, copy=False)


def kernel(x, filt):
    return run(x, filt)


# revision 31
# speedup vs baseline: 1.7803x; 1.0071x over previous
"""AliasFreeActivation (upsample2x -> leaky_relu -> 31x31 depthwise sinc conv
-> downsample2x) as a Trainium2 Bass/Tile kernel, data-parallel over 8 cores.

Math (per [128,128] image; B*C = 512 images, 64 per core):
  out = Dy @ Conv_F(lrelu(Uy @ x @ Ux^T)) @ Dx^T
With F = sum_r g_r h_r^T (weighted SVD of the 31x31 filter, rank 8):
  out = sum_r M_r @ act @ N_r^T
  M_r = Dy @ Toeplitz(g_r) [128,256],  N_r = Dx @ Toeplitz(h_r) [128,256]
  act = lrelu(Uy @ x @ Ux^T) [256,256]

Precision/engine split (v2):
  ranks 0..k32-1 : A-pass fp16 (banded), W evac'd to fp16 by a plain DVE
                   copy, B-pass fp16 matmuls over image-pair units.
  ranks k32..    : A-pass fp8 DoubleRow, W evac = PLAIN Pool copy to fp8
                   (the old s_e scale is folded into nt8), B-pass fp8 DR.
PSUM evacuation engines: ACT = lrelu + tmpT + out; DVE = W32; Pool
(gpsimd, which CAN read PSUM on TRN2) = W8 + act8.

Device dataflow per image (out[m,n] = sum_k lhsT[k,m] rhs[k,n]):
  S1a: tmpT[c,Y]    = sum_y x[y,c] UyT[y,Y]
  S1b: actT[X,Y]    = sum_c UxT[c,X] tmpT[c,Y]   (+ lrelu on evac)
  A:   W[Y,(r,j)]   = sum_X actT[X,Y] NT[X,(r,j)]
  B:   out[i,(m,j)] = sum_{r,Yc} MT[Yc,(r,i)] W[Yc,(r,m,j)]   (PSUM accum)
"""
import contextlib
import os

import numpy as np

import concourse.bass as bass
import concourse.mybir as mybir
import concourse.tile as tile
from concourse import bacc
from concourse.bass_utils import run_bass_kernel_spmd

H = 128
H2 = 256
KF = 31
LRELU_SLOPE = 0.01
RANK_ENV = os.environ.get("AFA_RANK")
N_CORES = 8
N_IMG = 64                      # images per core (512 / 8)
GROUP = int(os.environ.get("AFA_G", "4"))
K32_ENV = os.environ.get("AFA_K32")


# ---------------- host-side constants ----------------

def _ac_matrix(out_n, in_n):
    scale = (in_n - 1) / (out_n - 1)
    c = np.arange(out_n, dtype=np.float64) * scale
    i0 = np.clip(np.floor(c).astype(np.int64), 0, in_n - 2)
    w = c - i0
    M = np.zeros((out_n, in_n), dtype=np.float64)
    M[np.arange(out_n), i0] = 1.0 - w
    M[np.arange(out_n), i0 + 1] = w
    return M


def _toeplitz_same(h, n):
    T = np.zeros((n, n), dtype=np.float64)
    for u in range(len(h)):
        d = u - len(h) // 2
        if d >= 0:
            idx = np.arange(0, n - d)
        else:
            idx = np.arange(-d, n)
        T[idx, idx + d] += h[u]
    return T


def _shift_mat(n, d):
    S = np.zeros((n, n))
    idx = np.arange(0, n - d) if d >= 0 else np.arange(-d, n)
    S[idx, idx + d] = 1.0
    return S


def _weighted_rank(F, rank):
    """Rank-`rank` approx of F minimizing the end-to-end error under the
    signal model act ~ U x U^T with white x: err = ||B^1/2 (F-Fr) B^1/2||_F
    with B the Gram of the composed per-tap maps Z_u = D S_u U."""
    kf = F.shape[0]
    D = _ac_matrix(H, H2)
    Uu = _ac_matrix(H2, H)
    Zs = [D @ _shift_mat(H2, u - kf // 2) @ Uu for u in range(kf)]
    B = np.zeros((kf, kf))
    for u in range(kf):
        for v in range(u, kf):
            B[u, v] = B[v, u] = np.sum(Zs[u] * Zs[v])
    w, V = np.linalg.eigh(B)
    w = np.maximum(w, 1e-12)
    Bh = (V * np.sqrt(w)) @ V.T
    Bih = (V / np.sqrt(w)) @ V.T
    Gm = Bh @ F @ Bh
    U_, S_, Vt_ = np.linalg.svd(Gm)
    Fr = Bih @ (U_[:, :rank] * S_[:rank]) @ Vt_[:rank] @ Bih
    return Fr


def _sample_act(Uu, n=4):
    rng = np.random.default_rng(1234)
    X = rng.standard_normal((n, H, H))
    A = Uu @ X @ Uu.T
    return np.where(A >= 0, A, LRELU_SLOPE * A)


def _make_consts(filt, rank, k32):
    """Build device constants.

    Ranks [0, k32): nt fp16 in seg layout (columns (j, r_local)-major for the
    banded A-pass 2D APs), mt as RAW fp32 (used as float32r), rank-major.
    Ranks [k32, rank): nt8/mt8 fp8 DoubleRow-interleaved, with per-rank scale
    n_r chosen so that nt8, W8 (= s_a n_r act N^T, evac'd by a PLAIN copy)
    and mt8 (= M / (s_a n_r)) all sit inside the fp8e4m3 normal range.
    s_a is folded into uyt (lrelu is positively homogeneous) and 1/s_a into
    nt/mt-fp32, so every rank's B contribution lands in out PSUM at scale 1.
    """
    F = np.asarray(filt, dtype=np.float64)
    if rank < min(F.shape):
        F = _weighted_rank(F, rank)
    U, S, Vt = np.linalg.svd(F)
    D = _ac_matrix(H, H2)
    Uu = _ac_matrix(H2, H)
    uyt = np.ascontiguousarray(Uu.T)               # [128 y, 256 Y]

    nr8 = rank - k32
    Ms = [D @ _toeplitz_same(U[:, r] * np.sqrt(S[r]), H2) for r in range(rank)]
    Ns = [D @ _toeplitz_same(Vt[r, :] * np.sqrt(S[r]), H2) for r in range(rank)]

    # fp8 scales: act8 = s_a * act (s_a folded into uyt)
    act = _sample_act(Uu)
    actmax = np.abs(act).max() * 1.15
    s_a = 100.0 / actmax

    # fp16/fp32 ranks: nt gets 1/s_a (act arrives pre-scaled by s_a);
    # mt stays exact fp32.
    nt = np.zeros((2, H, k32 * H), dtype=np.float32)
    mt32 = np.zeros((2, H, k32 * H), dtype=np.float16)
    for r in range(k32):
        cols = np.arange(H) * k32 + r              # (j, r_local)-major
        for c in range(2):
            nt[c, :, cols] = (Ns[r][:, c * H:(c + 1) * H] / s_a).astype(np.float32)
            mt32[c, :, r * H:(r + 1) * H] = \
                Ms[r][:, c * H:(c + 1) * H].T.astype(np.float16)

    out = {"uyt": (uyt * s_a).astype(np.float32), "uxt": uyt.astype(np.float32),
           "nt": nt, "mt32": mt32, "k32": k32, "nr8": nr8, "s_a": float(s_a),
           "s_e": 1.0}
    if nr8 == 0:
        return out

    # ---- fp8 tail: per-rank scales (baseline scheme): wg8 = s_e * W8psum
    # (s_e applied for free by the ACT Copy evac), n_r/m_r sqrt-balanced so
    # act8, nt8, wg8, mt8 all sit inside the fp8e4m3 normal range.
    np8 = mybir.dt.np(mybir.dt.float8e4)
    nt8 = np.zeros((H, 2, nr8 * H), dtype=np.float64)
    mt8 = np.zeros((H, 2, nr8 * H), dtype=np.float64)
    wmaxs = [np.abs(act @ Ns[k32 + i].T).max() * 1.3 for i in range(nr8)]
    mmaxs = [np.abs(Ms[k32 + i]).max() for i in range(nr8)]
    nmaxs = [np.abs(Ns[k32 + i]).max() for i in range(nr8)]
    bias = 2.0
    tmp = [np.sqrt(wmaxs[i] * mmaxs[i]) * bias / (s_a * wmaxs[i])
           for i in range(nr8)]           # = n_r * s_e per rank
    s_e = float(np.exp(np.mean(np.log([tmp[i] * nmaxs[i]
                                       for i in range(nr8)]))) / 100.0)
    for i in range(nr8):
        r = k32 + i
        n_r = tmp[i] / s_e
        m_r = 1.0 / (s_a * n_r * s_e)
        for c in range(2):
            # nt8[p, xc, j*nr8+i] = n_r * N_r[j, xc*128+p]
            nt8[:, c, i::nr8] = (n_r * Ns[r][:, c * H:(c + 1) * H]).T
            # mt8[p, yc, i*H+k] = m_r * M_r[k, yc*128+p]
            mt8[:, c, i * H:(i + 1) * H] = (m_r * Ms[r][:, c * H:(c + 1) * H]).T
    out["s_e"] = s_e
    out["nt8"] = nt8.astype(np8).reshape(H, 2 * nr8 * H)
    out["mt8"] = mt8.astype(np8).reshape(H, 2 * nr8 * H)
    return out


# ---------------- device program ----------------

def _build_tile_program(tc, outs, ins, *, n_img, k32, nr8, group,
                        s_e=1.0, loop_reps=1):
    nc = tc.nc
    if nr8:
        x_d, uyt_d, uxt_d, nt_d, mt32_d, nt8_d, mt8_d = ins
    else:
        x_d, uyt_d, uxt_d, nt_d, mt32_d = ins
    out_d = outs[0]
    RC = k32 * H
    G = group
    GW = G * H
    assert G in (2, 4)
    assert n_img % G == 0
    f16 = mybir.dt.float16
    f32 = mybir.dt.float32
    f32r = mybir.dt.float32r
    fp8 = mybir.dt.float8e4
    DR = mybir.MatmulPerfMode.DoubleRow

    ctx = contextlib.ExitStack()
    with ctx:
        const_pool = ctx.enter_context(tc.tile_pool(name="consts", bufs=1))
        x_pool = ctx.enter_context(tc.tile_pool(name="x", bufs=2))
        tmp_pool = ctx.enter_context(tc.tile_pool(
            name="tmp", bufs=int(os.environ.get("AFA_TMPB", "3"))))
        act_pool = ctx.enter_context(tc.tile_pool(
            name="act", bufs=int(os.environ.get("AFA_ACTB", "8"))))
        w_pool = ctx.enter_context(tc.tile_pool(
            name="w", bufs=int(os.environ.get("AFA_WB", "2"))))
        osb_pool = ctx.enter_context(tc.tile_pool(name="osb", bufs=2))
        # PSUM banks: ps_s 3 + ps_w 2x2 + ps_out 1 = 8
        ps_s = ctx.enter_context(tc.tile_pool(
            name="ps_s", bufs=int(os.environ.get("AFA_PSS", "1")), space="PSUM"))
        w32h = os.environ.get("AFA_W32H", "0") == "1"
        if w32h:
            ps_w = ctx.enter_context(tc.tile_pool(
                name="ps_w", bufs=int(os.environ.get("AFA_PSW", "2")),
                space="PSUM"))
            ps_wh = ctx.enter_context(tc.tile_pool(
                name="ps_wh", bufs=int(os.environ.get("AFA_PSWH", "2")),
                space="PSUM"))
        else:
            ps_w = ctx.enter_context(tc.tile_pool(
                name="ps_w", bufs=int(os.environ.get("AFA_PSW", "3")),
                space="PSUM"))
        ps_out = ctx.enter_context(tc.tile_pool(
            name="ps_out", bufs=int(os.environ.get("AFA_PSO", "1")),
            space="PSUM"))

        uyt_sb = const_pool.tile([H, H2], f16, tag="uyt")
        uxt_sb = const_pool.tile([H, H2], f16, tag="uxt")

        def _load_s1_consts():
            nc.sync.dma_start(uyt_sb[:], uyt_d[:])
            nc.sync.dma_start(uxt_sb[:], uxt_d[:])
        nt_sb = []
        mt_sb = []
        for c in range(2):
            t = const_pool.tile([H, RC], f16, tag=f"nt{c}", name=f"nt{c}_sb")
            nt_sb.append(t)
            t = const_pool.tile([H, RC], f16, tag=f"mt{c}", name=f"mt{c}_sb")
            mt_sb.append(t)
        nt8_sb = mt8_sb = None
        if nr8:
            nt8_sb = const_pool.tile([H, 2 * nr8 * H], fp8, tag="nt8")
            mt8_sb = const_pool.tile([H, 2 * nr8 * H], fp8, tag="mt8")

        def _load_heavy_consts():
            # issued AFTER x(0)/uyt/uxt so the serial HWDGE queue does not
            # gate the S1 prologue; these are only needed by pass A/B.
            for c in range(2):
                nc.sync.dma_start(nt_sb[c][:], nt_d[c])
            if nr8:
                nc.sync.dma_start(nt8_sb[:], nt8_d[:])
            for c in range(2):
                nc.sync.dma_start(mt_sb[c][:], mt32_d[c])
            if nr8:
                nc.sync.dma_start(mt8_sb[:], mt8_d[:])

        def _fetch_x(g):
            x_sb = x_pool.tile([H, GW], f16, tag="x", name=f"x_{g}")
            xg = x_d[g * G:(g + 1) * G].rearrange("g h w -> h g w")
            nc.sync.dma_start(x_sb[:].rearrange("h (g w) -> h g w", g=G), xg)
            return x_sb

        import contextlib as _ctl

        def _s1prio():
            off = os.environ.get("AFA_S1PRIO", "")
            if off == "":
                return _ctl.nullcontext()
            return tc.high_priority(None if off == "0" else int(off))

        def _emit_s1a(g, half, x_sb, st, pool=None):
            """S1a matmuls for an image pair + tmpT evac (DVE)."""
            with _s1prio():
                tmpT_ps = (pool or ps_s).tile([H, 2 * H2], f32,
                                              tag="sp" if pool is None else "ops",
                                              name=f"tp_{g}_{half}")
                for u in range(2):
                    m = half * 2 + u
                    nc.tensor.matmul(tmpT_ps[:, u * H2:(u + 1) * H2],
                                     x_sb[:, m * H:(m + 1) * H], uyt_sb[:],
                                     start=True, stop=True)
                t_sb = tmp_pool.tile([H, 2 * H2], f16, tag="tmpT")
                nc.vector.tensor_copy(t_sb[:], tmpT_ps[:])
                st["t_sb"] = t_sb

        def _emit_s1b(g, m, st, pool=None):
            """S1b matmuls + lrelu (ACT) + act8 (Pool) for one image."""
            with _s1prio():
                act_ps = (pool or ps_s).tile(
                    [H, 2 * H2], f32,
                    tag="sp" if pool is None else "ops",
                    name=f"ap_{g}_{m}")
                tw = st["t_sb"][:, (m % 2) * H2:(m % 2 + 1) * H2]
                for xc in range(2):
                    nc.tensor.matmul(act_ps[:, xc * H2:(xc + 1) * H2],
                                     uxt_sb[:, xc * H:(xc + 1) * H], tw,
                                     start=True, stop=True)
                act_sb = act_pool.tile([H, 2 * H2], f16, tag="act")
                nc.scalar.activation(act_sb[:], act_ps[:],
                                     mybir.ActivationFunctionType.Lrelu,
                                     alpha=LRELU_SLOPE)
                st["act"].append(act_sb)
                if nr8:
                    act8_sb = act_pool.tile([H, 2 * H2], fp8, tag="act8")
                    nc.gpsimd.tensor_copy(act8_sb[:], act_sb[:])
                    st["act8"].append(act8_sb)

        def _emit_group(g, pending_b, acts_cur, x_next):
            wg_sb = w_pool.tile([H, 2 * k32 * GW], f16, tag="wg",
                                name=f"wg_{g}")
            wgv = wg_sb[:].rearrange("p (c r g w) -> p c r g w", c=2, r=k32, g=G)
            wg8_sb = None
            if nr8:
                wg8_sb = w_pool.tile([H, 2 * nr8 * GW], fp8, tag="wg8",
                                     name=f"wg8_{g}")
            act_sbs = acts_cur["act"]
            act8_sbs = acts_cur["act8"]
            nxt = {"act": [], "act8": []} if x_next is not None else None

            # pass A for image m of this group; W evacs right after the
            # producing matmuls so ps_w recycles fast.
            # fp16 part: nt/W_ps columns are (j, r_local)-major, so the
            # Toeplitz j-band of each X-chunk is a CONTIGUOUS column window:
            # X-chunk0 only reaches j<=71, chunk1 only j>=56; j in [56,72)
            # accumulates, the rest first-write.
            # fp8 part: one DoubleRow matmul per yc contracts both X-chunks.
            def _emit_a_split(m):
                act_sb = act_sbs[m]
                jwin = ((0, 72), (56, H))
                dst8v = wg8_sb[:].rearrange(
                    "p (c r g w) -> p c r g w", c=2, r=nr8, g=G)
                act8 = act8_sbs[m][:].rearrange("p (x y) -> p x y", x=2)
                nt8v = nt8_sb[:].rearrange("p (x c) -> p x c", x=2)
                for yc in range(2):
                    w_ps8 = ps_w.tile([H, 512], f32, tag="wps",
                                      name=f"wps8_{g}_{m}_{yc}")
                    nc.tensor.matmul(
                        w_ps8[:], act8[:, :, yc * H:(yc + 1) * H], nt8v,
                        start=True, stop=True, perf_mode=DR)
                    src8 = w_ps8[:].rearrange("p (j r) -> p r j", r=nr8)
                    if yc == 0:
                        nc.scalar.activation(dst8v[:, yc, :, m], src8,
                                             mybir.ActivationFunctionType.Copy,
                                             scale=s_e)
                    else:
                        nc.vector.tensor_scalar_mul(dst8v[:, yc, :, m],
                                                    src8, s_e)
                for yc in range(2):
                    w_ps = ps_w.tile([H, 512], f32, tag="wps",
                                     name=f"wps_{g}_{m}_{yc}")
                    for xc in range(2):
                        j0, j1 = jwin[xc]
                        nc.tensor.matmul(
                            w_ps[:, j0 * k32:j1 * k32],
                            act_sb[:, xc * H2 + yc * H: xc * H2 + (yc + 1) * H],
                            nt_sb[xc][:, j0 * k32:j1 * k32],
                            start=(xc == 0), stop=(xc == 1),
                            skip_group_check=True)
                    src = w_ps[:, :k32 * H].rearrange("p (j r) -> p r j", r=k32)
                    if yc == 0:
                        nc.vector.tensor_copy(wgv[:, yc, :, m], src)
                    else:
                        nc.scalar.activation(wgv[:, yc, :, m], src,
                                             mybir.ActivationFunctionType.Copy)

            def _emit_a(m):
                if os.environ.get("AFA_WSPLIT", "0") == "1":
                    _emit_a_split(m)
                    return
                act_sb = act_sbs[m]
                if nr8:
                    act8 = act8_sbs[m][:].rearrange("p (x y) -> p x y", x=2)
                    nt8v = nt8_sb[:].rearrange("p (x c) -> p x c", x=2)
                    w_ps8 = ps_w.tile([H, 1024], f32, tag="wps",
                                      name=f"wps8_{g}_{m}")
                    for yc in range(2):
                        nc.tensor.matmul(
                            w_ps8[:, yc * 512:(yc + 1) * 512],
                            act8[:, :, yc * H:(yc + 1) * H],
                            nt8v,
                            start=True, stop=True, perf_mode=DR)
                    # W8 evac: PLAIN fp8 copy on ACT (scales folded into nt8)
                    dst8v = wg8_sb[:].rearrange(
                        "p (c r g w) -> p c r g w", c=2, r=nr8, g=G)
                    if nr8 == 4:
                        src8 = w_ps8[:].rearrange(
                            "p (c j r) -> p c r j", c=2, r=nr8)
                        nc.scalar.activation(dst8v[:, :, :, m], src8,
                                             mybir.ActivationFunctionType.Copy,
                                             scale=s_e)
                    else:
                        for yc in range(2):
                            src8 = w_ps8[:, yc * 512:yc * 512 + nr8 * H] \
                                .rearrange("p (j r) -> p r j", r=nr8)
                            nc.scalar.activation(
                                dst8v[:, yc, :, m], src8,
                                mybir.ActivationFunctionType.Copy, scale=s_e)
                jwin = ((0, 72), (56, H))
                if w32h:
                    for yc in range(2):
                        w_ph = ps_wh.tile([H, 512], f32, tag="wph",
                                          name=f"wph_{g}_{m}_{yc}")
                        for xc in range(2):
                            j0, j1 = jwin[xc]
                            nc.tensor.matmul(
                                w_ph[:, j0 * k32:j1 * k32],
                                act_sb[:, xc * H2 + yc * H:
                                       xc * H2 + (yc + 1) * H],
                                nt_sb[xc][:, j0 * k32:j1 * k32],
                                start=(xc == 0), stop=(xc == 1),
                                skip_group_check=True)
                        srch = w_ph[:, :k32 * H].rearrange(
                            "p (j r) -> p r j", r=k32)
                        nc.vector.tensor_copy(wgv[:, yc, :, m], srch)
                else:
                    w_ps = ps_w.tile([H, 1024], f32, tag="wps",
                                     name=f"wps_{g}_{m}")
                    for yc in range(2):
                        for xc in range(2):
                            j0, j1 = jwin[xc]
                            nc.tensor.matmul(
                                w_ps[:, yc * 512 + j0 * k32:yc * 512 + j1 * k32],
                                act_sb[:, xc * H2 + yc * H: xc * H2 + (yc + 1) * H],
                                nt_sb[xc][:, j0 * k32:j1 * k32],
                                start=(xc == 0), stop=(xc == 1),
                                skip_group_check=True)
                    # W32 evac: RAW fp32 copy on DVE
                    if k32 == 4:
                        src = w_ps[:].rearrange("p (c j r) -> p c r j", c=2, r=k32)
                        nc.vector.tensor_copy(wgv[:, :, :, m], src)
                    else:
                        for yc in range(2):
                            src = w_ps[:, yc * 512:yc * 512 + k32 * H].rearrange(
                                "p (j r) -> p r j", r=k32)
                            nc.vector.tensor_copy(wgv[:, yc, :, m], src)

            # pass B over image PAIRS: each unit is a 256-col matmul touching
            # only images (p*2, p*2+1), so chunk ci only needs W of pair<=ci
            # from the previous group -> full-slot slack at pbd=0.
            state = {"nmm": [0, 0], "out_ps": None}
            if os.environ.get("AFA_BGRP", "0") == "1":
                npair = 1
                units = [(kind, 0, yc, r) for kind, yc, r in
                         ([("32", yc, r) for yc in range(2) for r in range(k32)]
                          + [("8", 0, rl) for rl in range(nr8)])]
            else:
                npair = G // 2
                units = [(kind, p, yc, r)
                         for p in range(npair)
                         for kind, yc, r in
                         ([("32", yc, r) for yc in range(2) for r in range(k32)]
                          + [("8", 0, rl) for rl in range(nr8)])]
            nu_pair = len(units) // npair

            def _pass_b(ci, chunks=G):
                if state["out_ps"] is None:
                    state["out_ps"] = ps_out.tile([H, GW], f32, tag="ops",
                                                  name=f"ops_{g}")
                out_ps = state["out_ps"]
                n0 = (ci * len(units)) // chunks
                n1 = ((ci + 1) * len(units)) // chunks
                pw = GW // npair
                for kind, p, yc, r in units[n0:n1]:
                    state["nmm"][p] += 1
                    ow = out_ps[:, p * pw:(p + 1) * pw]
                    first = state["nmm"][p] == 1
                    last = state["nmm"][p] == nu_pair
                    gpp = G // npair
                    if kind == "32":
                        nc.tensor.matmul(
                            ow,
                            mt_sb[yc][:, r * H:(r + 1) * H],
                            wgv[:, yc, r, p * gpp:(p + 1) * gpp],
                            start=first, stop=last, skip_group_check=True)
                    else:
                        mt8v = mt8_sb[:].rearrange("p (c ri) -> p c ri", c=2)
                        wg8v = wg8_sb[:].rearrange(
                            "p (c r g w) -> p c r g w", c=2, r=nr8, g=G)
                        nc.tensor.matmul(
                            ow,
                            mt8v[:, :, r * H:(r + 1) * H],
                            wg8v[:, :, r, p * gpp:(p + 1) * gpp],
                            start=first, stop=last,
                            perf_mode=DR, skip_group_check=True)
                if ci == chunks - 1:
                    og = out_d[g * G:(g + 1) * G].rearrange("g h w -> h g w")
                    out_sb = osb_pool.tile([H, GW], f32, tag="osb")
                    oe = os.environ.get("AFA_OE", "v")
                    if os.environ.get("AFA_OSPLIT", "0") == "1":
                        for p in range(G // 2):
                            nc.vector.tensor_copy(
                                out_sb[:, p * 2 * H:(p + 1) * 2 * H],
                                out_ps[:, p * 2 * H:(p + 1) * 2 * H])
                    elif oe == "s":
                        nc.scalar.activation(out_sb[:], out_ps[:],
                                             mybir.ActivationFunctionType.Copy)
                    else:
                        nc.vector.tensor_copy(out_sb[:], out_ps[:])
                    nc.sync.dma_start(
                        og, out_sb[:].rearrange("h (g w) -> h g w", g=G))

            # slot loop: A+evacs, then B(g-1) chunk, then next-group S1 piece
            s1pos = os.environ.get("AFA_S1POS", "late")
            bch = int(os.environ.get("AFA_BCH", str(G)))   # B chunk count
            bst = int(os.environ.get("AFA_BST", "0"))      # B start slot
            for m in range(G):
                if s1pos == "early" and nxt is not None:
                    if m % 2 == 0:
                        _emit_s1a(g + 1, m // 2, x_next, nxt)
                    _emit_s1b(g + 1, m, nxt)
                _emit_a(m)
                if pending_b is not None and bst <= m < bst + bch:
                    pending_b(m - bst, chunks=bch)
                if s1pos != "early" and nxt is not None:
                    if m % 2 == 0:
                        _emit_s1a(g + 1, m // 2, x_next, nxt)
                    sp_pool = ps_out if (m == int(os.environ.get(
                        "AFA_S1OUT", "-1"))) else None
                    _emit_s1b(g + 1, m, nxt, pool=sp_pool)

            return _pass_b, nxt

        def _emit_all_groups():
            ngroups = n_img // G
            pending = None
            x_next = _fetch_x(0)
            _load_s1_consts()
            _load_heavy_consts()
            # prologue: S1 for group 0
            acts_next = {"act": [], "act8": []}
            pseq = [None, ps_out, None, ps_out, None, ps_out]
            pi = 0
            for half in range(G // 2):
                _emit_s1a(0, half, x_next, acts_next, pool=pseq[pi]); pi += 1
                _emit_s1b(0, half * 2, acts_next, pool=pseq[pi]); pi += 1
                _emit_s1b(0, half * 2 + 1, acts_next, pool=pseq[pi]); pi += 1
            for g in range(ngroups):
                x_cur = x_next
                x_next = _fetch_x(g + 1) if g + 1 < ngroups else None
                acts_cur = acts_next
                pending, acts_next = _emit_group(g, pending, acts_cur, x_next)
            if pending is not None:
                bch = int(os.environ.get("AFA_BCH", str(G)))
                for ci in range(bch):
                    pending(ci, chunks=bch)

        if loop_reps > 1:
            with tc.For_i(0, loop_reps, 1):
                _emit_all_groups()
        else:
            _emit_all_groups()


_NC_CACHE = {}


def _build_nc(n_img=N_IMG, k32=4, nr8=4, group=None, s_e=1.0, loop_reps=1):
    if group is None:
        group = GROUP
    key = (n_img, k32, nr8, group, loop_reps, round(s_e, 12),
           os.environ.get("AFA_PSO", ""), os.environ.get("AFA_S1POS", ""),
           os.environ.get("AFA_BCH", ""), os.environ.get("AFA_BST", ""),
           os.environ.get("AFA_OSPLIT", ""), os.environ.get("AFA_G", ""),
           os.environ.get("AFA_WSPLIT", ""), os.environ.get("AFA_OE", ""),
           os.environ.get("AFA_BGRP", ""), os.environ.get("AFA_S1PRIO", ""),
           os.environ.get("AFA_W32H", ""), os.environ.get("AFA_PSWH", ""),
           os.environ.get("AFA_S1OUT", ""),
           os.environ.get("AFA_PSW", ""), os.environ.get("AFA_PSS", ""),
           os.environ.get("AFA_PBD", ""), os.environ.get("AFA_TMPB", ""),
           os.environ.get("AFA_ACTB", ""), os.environ.get("AFA_WB", ""))
    if key in _NC_CACHE:
        return _NC_CACHE[key]
    nc = bacc.Bacc("TRN2", target_bir_lowering=False, debug=False)
    f16 = mybir.dt.float16
    f32 = mybir.dt.float32
    f32r = mybir.dt.float32r
    fp8 = mybir.dt.float8e4
    RC = k32 * H
    x_d = nc.dram_tensor("x", [n_img, H, H], f16, kind="ExternalInput").ap()
    uyt_d = nc.dram_tensor("uyt", [H, H2], f16, kind="ExternalInput").ap()
    uxt_d = nc.dram_tensor("uxt", [H, H2], f16, kind="ExternalInput").ap()
    nt_d = nc.dram_tensor("nt", [2, H, RC], f16, kind="ExternalInput").ap()
    mt32_d = nc.dram_tensor("mt32", [2, H, RC], f16, kind="ExternalInput").ap()
    ins = [x_d, uyt_d, uxt_d, nt_d, mt32_d]
    if nr8:
        nt8_d = nc.dram_tensor("nt8", [H, 2 * nr8 * H], fp8,
                               kind="ExternalInput").ap()
        mt8_d = nc.dram_tensor("mt8", [H, 2 * nr8 * H], fp8,
                               kind="ExternalInput").ap()
        ins += [nt8_d, mt8_d]
    out_d = nc.dram_tensor("out", [n_img, H, H], f32, kind="ExternalOutput").ap()
    with tile.TileContext(nc) as tc:
        _build_tile_program(tc, [out_d], ins, n_img=n_img, k32=k32, nr8=nr8,
                            group=group, s_e=s_e, loop_reps=loop_reps)
    nc.compile()
    _NC_CACHE[key] = nc
    return nc


def _pick_rank(filt):
    """Smallest rank whose weighted-truncation error estimate fits the
    error budget (harness gate 2e-2; leave room for fp16/fp8 quantization).
    For the reference's sinc filter this lands on 8."""
    if RANK_ENV:
        return int(RANK_ENV)
    F = np.asarray(filt, np.float64)
    kf = F.shape[0]
    D = _ac_matrix(H, H2)
    Uu = _ac_matrix(H2, H)
    Zs = [D @ _shift_mat(H2, u - kf // 2) @ Uu for u in range(kf)]
    B = np.zeros((kf, kf))
    for u in range(kf):
        for v in range(u, kf):
            B[u, v] = B[v, u] = np.sum(Zs[u] * Zs[v])
    w, V = np.linalg.eigh(B)
    Bh = (V * np.sqrt(np.maximum(w, 1e-12))) @ V.T
    s = np.linalg.svd(Bh @ F @ Bh, compute_uv=False)
    nrm = np.sqrt(np.sum(s * s))
    for r in range(4, 16):
        if r >= len(s) or np.sqrt(np.sum(s[r:] ** 2)) <= 4e-3 * nrm:
            return r
    return 16


def _split_rank(rank):
    k32 = int(K32_ENV) if K32_ENV else max(rank - 4, rank // 2)
    k32 = min(k32, rank)
    return k32, rank - k32


def _make_in_maps(x, filt, rank, consts=None):
    k32, nr8 = _split_rank(rank)
    if consts is None:
        consts = _make_consts(filt, rank, k32)
    f16 = np.float16
    imgs = x.reshape(N_CORES, N_IMG, H, H)
    base = {
        "uyt": consts["uyt"].astype(f16), "uxt": consts["uxt"].astype(f16),
        "nt": consts["nt"].astype(f16), "mt32": consts["mt32"],
    }
    if consts["nr8"]:
        base["nt8"] = consts["nt8"]
        base["mt8"] = consts["mt8"]
    return [{"x": np.ascontiguousarray(imgs[core]).astype(f16), **base}
            for core in range(N_CORES)]


_RUNNER_CACHE = {}


def _get_runner(nc):
    """Persistent jitted 8-core runner (mirrors bass2jax.run_bass_via_pjrt's
    multi-core path) so repeated kernel() calls reuse one compiled executable."""
    if id(nc) in _RUNNER_CACHE:
        return _RUNNER_CACHE[id(nc)]
    import jax
    from jax.sharding import Mesh, PartitionSpec
    from jax.experimental.shard_map import shard_map
    from concourse.bass2jax import (_bass_exec_p, install_neuronx_cc_hook,
                                    partition_id_tensor)
    install_neuronx_cc_hook()
    in_names, out_names, out_avals, zero_outs = [], [], [], []
    for alloc in nc.m.functions[0].allocations:
        if not isinstance(alloc, mybir.MemoryLocationSet):
            continue
        name = alloc.memorylocations[0].name
        if alloc.kind == "ExternalInput":
            if nc.partition_id_tensor is not None and name == nc.partition_id_tensor.name:
                continue
            in_names.append(name)
        elif alloc.kind == "ExternalOutput":
            out_names.append(name)
            shape = tuple(alloc.tensor_shape)
            dtype = mybir.dt.np(alloc.dtype)
            out_avals.append(jax.core.ShapedArray(shape, dtype))
            zero_outs.append(np.zeros(shape, dtype))
    n_params = len(in_names)
    all_in_names = in_names + out_names
    if nc.partition_id_tensor is not None:
        all_in_names = all_in_names + [nc.partition_id_tensor.name]

    def _body(*args):
        operands = list(args)
        if nc.partition_id_tensor is not None:
            operands.append(partition_id_tensor())
        return tuple(_bass_exec_p.bind(
            *operands,
            out_avals=tuple(out_avals),
            in_names=tuple(all_in_names),
            out_names=tuple(out_names),
            lowering_input_output_aliases=(),
            sim_require_finite=True,
            sim_require_nnan=True,
            nc=nc,
        ))

    donate = tuple(range(n_params, n_params + len(out_names)))
    devices = jax.devices()[:N_CORES]
    mesh = Mesh(np.asarray(devices), ("core",))
    in_specs = (PartitionSpec("core"),) * (n_params + len(out_names))
    out_specs = (PartitionSpec("core"),) * len(out_names)
    sharded = jax.jit(
        shard_map(_body, mesh=mesh, in_specs=in_specs, out_specs=out_specs,
                  check_rep=False),
        donate_argnums=donate, keep_unused=True)
    runner = (sharded, in_names, out_names, out_avals, zero_outs)
    _RUNNER_CACHE[id(nc)] = runner
    return runner


def run(x, filt):
    """Run on 8 cores. Returns out [B,C,H,W] f32."""
    x = np.ascontiguousarray(np.asarray(x, dtype=np.float32))
    filt = np.asarray(filt, dtype=np.float32)
    B, C, Hh, Ww = x.shape
    assert (Hh, Ww) == (H, H) and B * C == N_CORES * N_IMG
    rank = _pick_rank(filt)
    k32, nr8 = _split_rank(rank)
    consts = _make_consts(filt, rank, k32)
    in_maps = _make_in_maps(x, filt, rank, consts)
    nc = _build_nc(k32=k32, nr8=nr8, s_e=consts["s_e"])
    try:
        sharded, in_names, out_names, out_avals, zero_outs = _get_runner(nc)
        concat_in = [np.concatenate([in_maps[c][nm] for c in range(N_CORES)], axis=0)
                     for nm in in_names]
        concat_zero = [np.zeros((N_CORES * z.shape[0], *z.shape[1:]), z.dtype)
                       for z in zero_outs]
        outs = sharded(*concat_in, *concat_zero)
        oi = out_names.index("out")
        out = np.asarray(outs[oi]).reshape(N_CORES, *out_avals[oi].shape)
    except Exception:
        res = run_bass_kernel_spmd(nc, in_maps, core_ids=list(range(N_CORES)))
        out = np.stack([res.results[c]["out"] for c in range(N_CORES)])
    return out.reshape(B, C, H, H).astype(np.float32, copy=False)


def kernel(x, filt):
    return run(x, filt)
